# revision 1
# baseline (speedup 1.0000x reference)
"""Trainium2 Bass kernel for retrieval-knn attention classifier (nn_MA_51866025067137).

Strategy (8 NeuronCores):
  Phase 1 — memory_keys sharded along N (12800 keys/core, padded 100000->102400
  with dummy rows).  Each core computes cosine-similarity ranking values for all
  256 queries against its shard (fp32r matmuls on the PE; keys pre-normalized so
  the matmul directly yields cosine ranking values) and extracts its local
  top-32 (value, index) per query with DVE max8/max_index/match_replace, using a
  low-9-mantissa-bit packing trick to recover in-segment indices.
  Host — merges the 8x40 candidates per row, re-scores them exactly in fp32,
  and gathers the global top-32 key vectors.
  Phase 2 — batch sharded (32 queries/core): memory-attention module
  (tanh(qWq + knnWm + b) -> scores -> softmax -> weighted sum) and classifier,
  via small fp32r matmuls; the softmax-weighted sum is a block-diagonal matmul.
"""

import numpy as np

import concourse.bacc as bacc
import concourse.mybir as mybir
from concourse.tile import TileContext
from concourse.bass_utils import run_bass_kernel_spmd
from concourse.masks import make_identity

# problem dims (hardcoded per harness contract)
B, N, D = 256, 100000, 512
A, C, K = 256, 100, 32
NC_CORES = 8
NPAD = 102400             # 8 * 12800
SHARD = NPAD // NC_CORES  # 12800
CHUNK = 512               # keys per inner loop step
NCHUNK = SHARD // CHUNK   # 25
SEG = 512                 # max8 segment width (9-bit in-segment index)
NSEG = SHARD // SEG       # 25
L1W = NSEG * 8            # 200
BROWS = B // NC_CORES     # 32 rows per core in phase 2
KLOC = 40                 # local candidates per core per row
CAND = NC_CORES * KLOC    # 320 merged candidates per row

f32 = mybir.dt.float32
f32r = mybir.dt.float32r
u32 = mybir.dt.uint32

_PH1 = None
_PH2 = None


def _build_phase1():
    nc = bacc.Bacc("TRN2", target_bir_lowering=False)
    khatT = nc.dram_tensor("khatT", [NCHUNK, 128, 4 * CHUNK], f32r, kind="ExternalInput")
    qT = nc.dram_tensor("qT", [D, B], f32r, kind="ExternalInput")
    win_out = nc.dram_tensor("win", [B, KLOC], f32, kind="ExternalOutput")
    pos_out = nc.dram_tensor("pos", [B, KLOC], u32, kind="ExternalOutput")

    with TileContext(nc) as tc:
        with (
            tc.tile_pool(name="const", bufs=1) as constp,
            tc.tile_pool(name="qpool", bufs=1) as qpool,
            tc.tile_pool(name="keys", bufs=6) as keyp,
            tc.tile_pool(name="packed", bufs=8) as packp,
            tc.tile_pool(name="l1", bufs=1) as l1p,
            tc.tile_pool(name="small", bufs=1) as smallp,
            tc.tile_pool(name="psum", bufs=2, space="PSUM") as psump,
        ):
            # constants: AND-mask (0xFFFFFE00) per partition; iota 0..511
            mask_t = constp.tile([128, 1], u32, tag="mask")
            nc.vector.memset(mask_t[:], 0xFFFFFE00)
            iota_t = constp.tile([128, CHUNK], u32, tag="iota")
            nc.gpsimd.iota(iota_t[:], pattern=[[1, CHUNK]], base=0,
                           channel_multiplier=0)

            # load qT and relu in place
            qTr = []
            for dc in range(4):
                t = qpool.tile([128, B], f32r, tag=f"qt{dc}")
                nc.sync.dma_start(out=t[:], in_=qT[dc * 128:(dc + 1) * 128, :])
                nc.scalar.activation(t[:], t[:], mybir.ActivationFunctionType.Relu)
                qTr.append(t)

            L1 = [l1p.tile([128, L1W], f32, tag=f"l1_{qt}", name=f"l1_{qt}") for qt in range(2)]

            for c in range(NCHUNK):
                kt = keyp.tile([128, 4 * CHUNK], f32r, tag="kt")
                nc.sync.dma_start(out=kt[:], in_=khatT[c, :, :])
                for qt in range(2):
                    ps = psump.tile([128, CHUNK], f32, tag=f"sim{qt}")
                    for dc in range(4):
                        nc.tensor.matmul(
                            ps[:],
                            lhsT=qTr[dc][:, qt * 128:(qt + 1) * 128],
                            rhs=kt[:, dc * CHUNK:(dc + 1) * CHUNK],
                            start=(dc == 0), stop=(dc == 3),
                        )
                    # evict (ACT), pack on GPSIMD: packed = (sim & mask) | iota
                    ev = packp.tile([128, CHUNK], f32, tag=f"ev{qt}")
                    nc.scalar.copy(out=ev[:], in_=ps[:])
                    pk = packp.tile([128, CHUNK], f32, tag=f"pk{qt}")
                    nc.vector.scalar_tensor_tensor(
                        out=pk[:].bitcast(u32), in0=ev[:].bitcast(u32),
                        scalar=mask_t[:], in1=iota_t[:],
                        op0=mybir.AluOpType.bitwise_and,
                        op1=mybir.AluOpType.bitwise_or,
                    )
                    nc.vector.max(out=L1[qt][:, c * 8:(c + 1) * 8], in_=pk[:])

            # extraction: 5 rounds of top-8 from L1 (400 wide)
            for qt in range(2):
                win = smallp.tile([128, KLOC], f32, tag=f"win{qt}")
                pos = smallp.tile([128, KLOC], u32, tag=f"pos{qt}")
                for r in range(5):
                    w8 = win[:, r * 8:(r + 1) * 8]
                    nc.vector.max(out=w8, in_=L1[qt][:])
                    nc.vector.max_index(out=pos[:, r * 8:(r + 1) * 8],
                                        in_max=w8, in_values=L1[qt][:])
                    if r < 4:
                        nc.vector.match_replace(out=L1[qt][:], in_to_replace=w8,
                                                in_values=L1[qt][:],
                                                imm_value=-3.0e38)
                nc.sync.dma_start(out=win_out[qt * 128:(qt + 1) * 128, :], in_=win[:])
                nc.sync.dma_start(out=pos_out[qt * 128:(qt + 1) * 128, :], in_=pos[:])
    nc.finalize()
    return nc


def _build_phase2():
    nc = bacc.Bacc("TRN2", target_bir_lowering=False)
    qT_in = nc.dram_tensor("qT", [D, BROWS], f32r, kind="ExternalInput")       # pre-relu
    knn_in = nc.dram_tensor("knn", [BROWS * K, D], f32r, kind="ExternalInput")
    knnT_in = nc.dram_tensor("knnT", [D, BROWS * K], f32r, kind="ExternalInput")
    Wq_in = nc.dram_tensor("Wq", [D, A], f32r, kind="ExternalInput")
    Wm_in = nc.dram_tensor("Wm", [D, A], f32r, kind="ExternalInput")
    Ws_in = nc.dram_tensor("Ws", [A, 1], f32r, kind="ExternalInput")
    bqm_in = nc.dram_tensor("bqm", [A, 1], f32, kind="ExternalInput")          # bq+bm
    Wc_in = nc.dram_tensor("Wc", [2 * D, C], f32r, kind="ExternalInput")
    out_d = nc.dram_tensor("out", [BROWS, C], f32, kind="ExternalOutput")      # +bc host
    escratch = nc.dram_tensor("escratch", [1, BROWS * K], f32)                 # bounce

    NCD = BROWS * K  # 1024

    with TileContext(nc) as tc:
        with (
            tc.tile_pool(name="big", bufs=1) as bigp,
            tc.tile_pool(name="small", bufs=1) as smallp,
            tc.tile_pool(name="psum", bufs=1, space="PSUM") as psump,
        ):
            # ---- load inputs (M-padded tiles to satisfy fp32r col_grp=0xf) ----
            qT = [smallp.tile([128, 128], f32r, tag=f"qT{dc}", name=f"qTt{dc}") for dc in range(4)]
            for dc in range(4):
                nc.vector.memset(qT[dc][:].bitcast(u32), 0)
                nc.sync.dma_start(out=qT[dc][:, :BROWS],
                                  in_=qT_in[dc * 128:(dc + 1) * 128, :])
                nc.scalar.activation(qT[dc][:, :BROWS], qT[dc][:, :BROWS],
                                     mybir.ActivationFunctionType.Relu)
            knnall = bigp.tile([128, 8 * D], f32r, tag="knnall")
            nc.sync.dma_start(out=knnall[:].rearrange("p (t d) -> p t d", t=8),
                              in_=knn_in[:].rearrange("(t p) d -> p t d", p=128))
            knn = [knnall[:, t * D:(t + 1) * D] for t in range(8)]
            knnTall = bigp.tile([128, 4 * NCD], f32r, tag="knnTall")
            nc.sync.dma_start(out=knnTall[:].rearrange("p (dc c) -> p dc c", dc=4),
                              in_=knnT_in[:].rearrange("(dc p) c -> p dc c", p=128))
            knnT = [knnTall[:, dc * NCD:(dc + 1) * NCD] for dc in range(4)]
            Wqall = smallp.tile([128, 4 * A], f32r, tag="Wqall")
            nc.sync.dma_start(out=Wqall[:].rearrange("p (dc a) -> p dc a", dc=4),
                              in_=Wq_in[:].rearrange("(dc p) a -> p dc a", p=128))
            Wq = [Wqall[:, dc * A:(dc + 1) * A] for dc in range(4)]
            Wmall = smallp.tile([128, 4 * A], f32r, tag="Wmall")
            nc.sync.dma_start(out=Wmall[:].rearrange("p (dc a) -> p dc a", dc=4),
                              in_=Wm_in[:].rearrange("(dc p) a -> p dc a", p=128))
            Wm = [Wmall[:, dc * A:(dc + 1) * A] for dc in range(4)]
            Ws = [smallp.tile([128, 128], f32r, tag=f"Ws{at}", name=f"Wst{at}") for at in range(2)]
            bqm = [smallp.tile([128, 1], f32, tag=f"bqm{at}", name=f"bqmt{at}") for at in range(2)]
            for at in range(2):
                nc.vector.memset(Ws[at][:].bitcast(u32), 0)
                nc.sync.dma_start(out=Ws[at][:, :1],
                                  in_=Ws_in[at * 128:(at + 1) * 128, :])
                nc.sync.dma_start(out=bqm[at][:],
                                  in_=bqm_in[at * 128:(at + 1) * 128, :])
            Wcall = smallp.tile([128, 8 * C], f32r, tag="Wcall")
            nc.sync.dma_start(out=Wcall[:].rearrange("p (m j) -> p m j", m=8),
                              in_=Wc_in[:].rearrange("(m p) j -> p m j", p=128))
            Wc = [Wcall[:, m * C:(m + 1) * C] for m in range(8)]
            ones = smallp.tile([128, 2], f32r, tag="ones")
            nc.vector.memset(ones[:].bitcast(u32), 0)
            nc.vector.memset(ones[:, :1].bitcast(u32), 0x3F800000)
            # mask4[p, j] = 1.0 iff j == p // 32
            mask4 = smallp.tile([128, 4], f32, tag="mask4")
            nc.vector.memset(mask4[:], 1.0)
            nc.gpsimd.affine_select(out=mask4[:], in_=mask4[:],
                                    compare_op=mybir.AluOpType.is_ge, fill=0.0,
                                    base=0, pattern=[[-32, 4]], channel_multiplier=1)
            nc.gpsimd.affine_select(out=mask4[:], in_=mask4[:],
                                    compare_op=mybir.AluOpType.is_ge, fill=0.0,
                                    base=31, pattern=[[32, 4]], channel_multiplier=-1)
            ident = smallp.tile([128, 128], f32, tag="ident")
            make_identity(nc, ident[:])

            # ---- qprojT [2][128a, 32] ----
            qprojT = [smallp.tile([128, BROWS], f32, tag=f"qp{at}", name=f"qpt{at}") for at in range(2)]
            for at in range(2):
                ps = psump.tile([128, BROWS], f32, tag="ps_a")
                for dc in range(4):
                    nc.tensor.matmul(
                        ps[:],
                        lhsT=Wq[dc][:, at * 128:(at + 1) * 128],
                        rhs=qT[dc][:, :BROWS],
                        start=(dc == 0), stop=(dc == 3))
                nc.scalar.copy(out=qprojT[at][:], in_=ps[:])

            # ---- hT = tanh(kprojT + qprojT_bcast + bqm) ; scores ----
            sc_ps = psump.tile([128, NCD], f32, tag="ps_sc")
            for at in range(2):
                kp = psump.tile([128, NCD], f32, tag="ps_kp", bufs=2)
                for dc in range(4):
                    for half in range(2):
                        nc.tensor.matmul(
                            kp[:, half * 512:(half + 1) * 512],
                            lhsT=Wm[dc][:, at * 128:(at + 1) * 128],
                            rhs=knnT[dc][:, half * 512:(half + 1) * 512],
                            start=(dc == 0), stop=(dc == 3))
                hT = bigp.tile([128, NCD], f32r, tag=f"hT{at}")
                qb = qprojT[at][:, :, None].to_broadcast([128, BROWS, K])
                nc.vector.tensor_tensor(
                    hT[:].rearrange("p (q k) -> p q k", k=K),
                    kp[:].rearrange("p (q k) -> p q k", k=K),
                    qb, mybir.AluOpType.add)
                nc.scalar.activation(hT[:], hT[:], mybir.ActivationFunctionType.Tanh,
                                     bias=bqm[at][:])
                for half in range(2):
                    nc.tensor.matmul(
                        sc_ps[:, half * 512:(half + 1) * 512],
                        lhsT=Ws[at][:],
                        rhs=hT[:, half * 512:(half + 1) * 512],
                        start=(at == 0), stop=(at == 1))
            e_row = smallp.tile([1, NCD], f32, tag="e_row")
            nc.scalar.activation(e_row[:], sc_ps[:1, :],
                                 mybir.ActivationFunctionType.Exp)
            # bounce through DRAM to redistribute [1, 1024] -> [128, 8]
            nc.sync.dma_start(out=escratch[:, :], in_=e_row[:, :])
            e_col = smallp.tile([128, 8], f32, tag="e_col")
            nc.sync.dma_start(out=e_col[:],
                              in_=escratch[0, :].rearrange("(t p) -> p t", p=128))

            # ---- block-diag softmax weights (M-padded), den, attended ----
            w2 = [bigp.tile([128, 128], f32r, tag=f"w2_{t}", name=f"w2t{t}") for t in range(8)]
            for t in range(8):
                nc.vector.memset(w2[t][:].bitcast(u32), 0)
                nc.vector.tensor_scalar_mul(w2[t][:, 4 * t:4 * t + 4], mask4[:],
                                            e_col[:, t:t + 1])
            den_ps = psump.tile([128, 2], f32, tag="ps_a")
            for t in range(8):
                nc.tensor.matmul(den_ps[:], lhsT=w2[t][:], rhs=ones[:],
                                 start=(t == 0), stop=(t == 7))
            att_ps = psump.tile([128, D], f32, tag="ps_kp", bufs=2)
            for t in range(8):
                nc.tensor.matmul(att_ps[:], lhsT=w2[t][:], rhs=knn[t],
                                 start=(t == 0), stop=(t == 7))
            rden = smallp.tile([BROWS, 1], f32, tag="rden")
            nc.vector.reciprocal(rden[:], den_ps[:BROWS, :1])
            att = smallp.tile([BROWS, D], f32, tag="att_sb")
            nc.vector.tensor_scalar_mul(att[:], att_ps[:BROWS, :], rden[:])

            # ---- attendedT via PE transpose (plain fp32) ----
            attT = [smallp.tile([128, 128], f32r, tag=f"attT{dc}", name=f"attTt{dc}") for dc in range(4)]
            for dc in range(4):
                tp = psump.tile([128, BROWS], f32, tag="ps_a")
                nc.tensor.transpose(tp[:], att[:, dc * 128:(dc + 1) * 128],
                                    ident[:BROWS, :BROWS])
                nc.vector.memset(attT[dc][:].bitcast(u32), 0)
                nc.scalar.copy(out=attT[dc][:, :BROWS], in_=tp[:])

            # ---- classifier ----
            out_ps = psump.tile([128, C], f32, tag="ps_out")
            for m in range(8):
                lhsT = qT[m] if m < 4 else attT[m - 4]
                nc.tensor.matmul(out_ps[:], lhsT=lhsT[:], rhs=Wc[m],
                                 start=(m == 0), stop=(m == 7))
            out_sb = smallp.tile([BROWS, C], f32, tag="out_sb")
            nc.scalar.copy(out=out_sb[:], in_=out_ps[:BROWS, :])
            nc.sync.dma_start(out=out_d[:, :], in_=out_sb[:])
    nc.finalize()
    return nc


def _phase1_nc():
    global _PH1
    if _PH1 is None:
        _PH1 = _build_phase1()
    return _PH1


def _phase2_nc():
    global _PH2
    if _PH2 is None:
        _PH2 = _build_phase2()
    return _PH2


def kernel(query_feat, memory_keys, Wq, bq, Wm, bm, Ws, bs, Wc, bc):
    query_feat = np.asarray(query_feat, np.float32)
    memory_keys = np.asarray(memory_keys, np.float32)

    # ---- host prep: pad + normalize + transpose + shard keys ----
    kn = np.sqrt((memory_keys ** 2).sum(axis=1))
    khat = memory_keys * (1.0 / kn)[:, None]
    pad = np.full((NPAD - N, D), -1.0 / np.sqrt(D), np.float32)
    khat_pad = np.concatenate([khat.astype(np.float32), pad], axis=0)
    qT_full = np.ascontiguousarray(query_feat.T)  # [512, 256]

    ph1 = _phase1_nc()
    in_maps = []
    for c in range(NC_CORES):
        sh = khat_pad[c * SHARD:(c + 1) * SHARD]          # [12800, 512]
        arr = np.ascontiguousarray(
            sh.reshape(NCHUNK, CHUNK, 4, 128).transpose(0, 3, 2, 1)
        ).reshape(NCHUNK, 128, 4 * CHUNK)
        in_maps.append({"khatT": arr, "qT": qT_full})
    res1 = run_bass_kernel_spmd(ph1, in_maps, core_ids=list(range(NC_CORES)))

    # ---- host merge: recover indices, exact re-score of candidates ----
    all_gidx = np.zeros((B, NC_CORES, KLOC), np.int64)
    for c in range(NC_CORES):
        win = res1.results[c]["win"].view(np.uint32)
        pos = res1.results[c]["pos"].astype(np.int64)   # 0..399 in L1
        seg = pos // 8
        within = (win & np.uint32(0x1FF)).astype(np.int64)
        all_gidx[:, c, :] = seg * SEG + within + c * SHARD
    gidx = all_gidx.reshape(B, CAND)
    safe = np.minimum(gidx, N - 1)
    q32 = np.maximum(query_feat, 0)
    cand_keys = memory_keys[safe]                       # [256, 320, 512]
    dots = np.einsum("bd,bcd->bc", q32, cand_keys, optimize=True)
    cos = dots / np.maximum(
        np.linalg.norm(q32, axis=1)[:, None] * kn[safe], np.float32(1e-8))
    cos[gidx >= N] = -np.inf                            # mask dummy-pad hits
    order = np.argsort(-cos, axis=1, kind="stable")[:, :K]
    top_idx = np.take_along_axis(safe, order, axis=1)   # [256, 32]
    knn = memory_keys[top_idx]                          # [256, 32, 512]

    # ---- phase 2 (batch sharded) ----
    ph2 = _phase2_nc()
    bqm = (np.asarray(bq, np.float32) + np.asarray(bm, np.float32)).reshape(A, 1)
    Wq_a = np.ascontiguousarray(np.asarray(Wq, np.float32))
    Wm_a = np.ascontiguousarray(np.asarray(Wm, np.float32))
    Ws_a = np.ascontiguousarray(np.asarray(Ws, np.float32))
    Wc_a = np.ascontiguousarray(np.asarray(Wc, np.float32))
    in_maps2 = []
    for c in range(NC_CORES):
        rows = slice(c * BROWS, (c + 1) * BROWS)
        knn_c = knn[rows].reshape(BROWS * K, D)
        in_maps2.append({
            "qT": np.ascontiguousarray(query_feat[rows].T),
            "knn": np.ascontiguousarray(knn_c),
            "knnT": np.ascontiguousarray(knn_c.T),
            "Wq": Wq_a, "Wm": Wm_a, "Ws": Ws_a, "bqm": bqm, "Wc": Wc_a,
        })
    res2 = run_bass_kernel_spmd(ph2, in_maps2, core_ids=list(range(NC_CORES)))
    out = np.concatenate([res2.results[c]["out"] for c in range(NC_CORES)], axis=0)
    return (out + np.asarray(bc, np.float32)[None, :]).astype(np.float32)



# revision 3
# speedup vs baseline: 1.2737x; 1.2737x over previous
"""Trainium2 Bass kernel for retrieval-knn attention classifier (nn_MA_51866025067137).

Strategy (8 NeuronCores):
  Phase 1 — memory_keys sharded along N (12800 keys/core, padded 100000->102400
  with dummy rows), fed in bf16 (keys pre-normalized on host so the matmul
  directly yields cosine ranking values; host re-scores candidates in exact
  fp32 afterwards, so ranking precision only has to preserve the top-32 set).
  Per chunk of 512 keys: PE computes sims for all 256 queries (bf16 matmuls,
  fp32 PSUM), ACT evicts both query-halves in one [128,1024] copy, GPSIMD
  packs a 9-bit in-chunk index into the sim mantissa, DVE max8 keeps the
  top-8 per chunk.  Tail: 4 rounds of max8/max_index/match_replace extract
  the per-core top-32 (value, position) per query row.
  Host — merges the 8x32 candidates per row, re-scores them exactly in fp32,
  and gathers the global top-32 key vectors.
  Phase 2 — batch sharded (32 queries/core), all inputs packed into one bf16
  blob (2 logical DMAs): memory-attention module via bf16 matmuls; the
  softmax-score row is transposed with 8 tiny PE transposes (no DRAM bounce);
  attended@Wc is reassociated as sum_k w_k * (knn_k @ Wc2) so the weighted
  sum runs over a precomputed [1024,100] knnWc instead of [1024,512] knn
  (no knn tile, no attT transpose); normalization by sum(e) happens on host.
"""

import numpy as np
import ml_dtypes

import concourse.bacc as bacc
import concourse.mybir as mybir
from concourse.tile import TileContext
from concourse.bass_utils import run_bass_kernel_spmd

# problem dims (hardcoded per harness contract)
B, N, D = 256, 100000, 512
A, C, K = 256, 100, 32
NC_CORES = 8
NPAD = 102400             # 8 * 12800
SHARD = NPAD // NC_CORES  # 12800
CHUNK = 512               # keys per inner loop step
NCHUNK = SHARD // CHUNK   # 25
L1W = NCHUNK * 8          # 200
BROWS = B // NC_CORES     # 32 rows per core in phase 2
KLOC = 32                 # local candidates per core per row
NROUND = KLOC // 8        # 4 extraction rounds
CAND = NC_CORES * KLOC    # 256 merged candidates per row

f32 = mybir.dt.float32
f32r = mybir.dt.float32r
bf16 = mybir.dt.bfloat16
u32 = mybir.dt.uint32
BF = ml_dtypes.bfloat16

# ---- phase-2 blob layout (bf16 columns) ----
NCD = BROWS * K           # 1024
# ktile: [128, 4096], col = half*2048 + dc*512 + i  (bk = half*512 + i)
KT_W = 4096
# wtile: [128, 2984]
W_WM = 0                  # 4 dc x 256
W_WQ = 1024               # 4 dc x 256
W_WS = 2048               # 2 (col at)
W_BQM = 2050              # 4 bf16 cols = [128,2] f32 (byte offset 4100, 4-aligned)
W_QT = 2054               # 4 dc x 32
W_WC = 2182               # 8 m x 100
W_W = 2984                # padded (2982 used)
BLOB_W = KT_W + W_W       # 7080
CP4 = C + 4               # 104: knnWc stride (100 vals, col 100 = 1.0)

_PH1 = None
_PH2 = None


def _build_phase1():
    nc = bacc.Bacc("TRN2", target_bir_lowering=False)
    khatT = nc.dram_tensor("khatT", [NCHUNK, 128, 4 * CHUNK], bf16, kind="ExternalInput")
    qT = nc.dram_tensor("qT", [D, B], bf16, kind="ExternalInput")
    win_out = nc.dram_tensor("win", [B, KLOC], f32, kind="ExternalOutput")
    pos_out = nc.dram_tensor("pos", [B, KLOC], u32, kind="ExternalOutput")

    with TileContext(nc) as tc:
        with (
            tc.tile_pool(name="const", bufs=1) as constp,
            tc.tile_pool(name="qpool", bufs=1) as qpool,
            tc.tile_pool(name="keys", bufs=5) as keyp,
            tc.tile_pool(name="ev", bufs=3) as evp,
            tc.tile_pool(name="pk", bufs=3) as pkp,
            tc.tile_pool(name="l1", bufs=1) as l1p,
            tc.tile_pool(name="small", bufs=1) as smallp,
            tc.tile_pool(name="psum", bufs=3, space="PSUM") as psump,
        ):
            # constants: AND-mask (0xFFFFFE00) per partition; iota 0..511 twice
            mask_t = constp.tile([128, 1], u32, tag="mask")
            nc.vector.memset(mask_t[:], 0xFFFFFE00)
            iota_t = constp.tile([128, 2 * CHUNK], u32, tag="iota")
            nc.gpsimd.iota(iota_t[:].rearrange("p (a b) -> p a b", a=2),
                           pattern=[[0, 2], [1, CHUNK]], base=0,
                           channel_multiplier=0)

            # qT already relu'd + bf16 on host
            qTr = []
            for dc in range(4):
                t = qpool.tile([128, B], bf16, tag=f"qt{dc}")
                nc.sync.dma_start(out=t[:], in_=qT[dc * 128:(dc + 1) * 128, :])
                qTr.append(t)

            L1 = [l1p.tile([128, L1W], f32, tag=f"l1_{qt}", name=f"l1_{qt}")
                  for qt in range(2)]

            for c in range(NCHUNK):
                kt = keyp.tile([128, 4 * CHUNK], bf16, tag="kt")
                nc.sync.dma_start(out=kt[:], in_=khatT[c, :, :])
                ps = psump.tile([128, 2 * CHUNK], f32, tag="sim")
                for qt in range(2):
                    for dc in range(4):
                        nc.tensor.matmul(
                            ps[:, qt * CHUNK:(qt + 1) * CHUNK],
                            lhsT=qTr[dc][:, qt * 128:(qt + 1) * 128],
                            rhs=kt[:, dc * CHUNK:(dc + 1) * CHUNK],
                            start=(dc == 0), stop=(dc == 3),
                        )
                # evict both query-halves at once (ACT), pack on GPSIMD
                ev = evp.tile([128, 2 * CHUNK], f32, tag="ev")
                nc.scalar.copy(out=ev[:], in_=ps[:])
                pk = pkp.tile([128, 2 * CHUNK], f32, tag="pk")
                nc.vector.scalar_tensor_tensor(
                    out=pk[:].bitcast(u32), in0=ev[:].bitcast(u32),
                    scalar=mask_t[:], in1=iota_t[:],
                    op0=mybir.AluOpType.bitwise_and,
                    op1=mybir.AluOpType.bitwise_or,
                )
                for qt in range(2):
                    nc.vector.max(out=L1[qt][:, c * 8:(c + 1) * 8],
                                  in_=pk[:, qt * CHUNK:(qt + 1) * CHUNK])

            # extraction: NROUND rounds of top-8 from L1 (200 wide)
            for qt in range(2):
                win = smallp.tile([128, KLOC], f32, tag=f"win{qt}")
                pos = smallp.tile([128, KLOC], u32, tag=f"pos{qt}")
                for r in range(NROUND):
                    w8 = win[:, r * 8:(r + 1) * 8]
                    nc.vector.max(out=w8, in_=L1[qt][:])
                    nc.vector.max_index(out=pos[:, r * 8:(r + 1) * 8],
                                        in_max=w8, in_values=L1[qt][:])
                    if r < NROUND - 1:
                        nc.vector.match_replace(out=L1[qt][:], in_to_replace=w8,
                                                in_values=L1[qt][:],
                                                imm_value=-3.0e38)
                nc.sync.dma_start(out=win_out[qt * 128:(qt + 1) * 128, :], in_=win[:])
                nc.sync.dma_start(out=pos_out[qt * 128:(qt + 1) * 128, :], in_=pos[:])
    nc.finalize()
    return nc


def _build_phase2():
    nc = bacc.Bacc("TRN2", target_bir_lowering=False)
    blob = nc.dram_tensor("blob", [128, BLOB_W], bf16, kind="ExternalInput")
    out_d = nc.dram_tensor("out", [2 * BROWS, CP4], f32, kind="ExternalOutput")

    with TileContext(nc) as tc:
        with (
            tc.tile_pool(name="big", bufs=1) as bigp,
            tc.tile_pool(name="small", bufs=1) as smallp,
            tc.tile_pool(name="pskp", bufs=2, space="PSUM") as pskp,
            tc.tile_pool(name="pssc", bufs=1, space="PSUM") as pssc,
            tc.tile_pool(name="psmi", bufs=2, space="PSUM") as psmi,
        ):
            # ---- DMAs: weights first (small), then knnT halves ----
            wt = bigp.tile([128, W_W], bf16, tag="wt")
            nc.sync.dma_start(out=wt[:, :2982], in_=blob[:, KT_W:KT_W + 2982])
            kt = bigp.tile([128, KT_W], bf16, tag="ktile")
            for half in range(2):
                nc.sync.dma_start(out=kt[:, half * 2048:(half + 1) * 2048],
                                  in_=blob[:, half * 2048:(half + 1) * 2048])

            Wm = [wt[:, W_WM + dc * A:W_WM + (dc + 1) * A] for dc in range(4)]
            Wq = [wt[:, W_WQ + dc * A:W_WQ + (dc + 1) * A] for dc in range(4)]
            Ws = [wt[:, W_WS + at:W_WS + at + 1] for at in range(2)]
            bqm = wt[:, W_BQM:W_BQM + 4].bitcast(f32)            # [128, 2]
            qT = [wt[:, W_QT + dc * BROWS:W_QT + (dc + 1) * BROWS] for dc in range(4)]
            Wc = [wt[:, W_WC + m * C:W_WC + (m + 1) * C] for m in range(8)]

            # ---- small constants ----
            ident1 = smallp.tile([1, 1], f32, tag="id1")
            nc.vector.memset(ident1[:], 1.0)
            # mask4[p, j] = 1.0 iff j == p // 32
            mask4 = smallp.tile([128, 4], f32, tag="mask4")
            nc.vector.memset(mask4[:], 1.0)
            nc.gpsimd.affine_select(out=mask4[:], in_=mask4[:],
                                    compare_op=mybir.AluOpType.is_ge, fill=0.0,
                                    base=0, pattern=[[-32, 4]], channel_multiplier=1)
            nc.gpsimd.affine_select(out=mask4[:], in_=mask4[:],
                                    compare_op=mybir.AluOpType.is_ge, fill=0.0,
                                    base=31, pattern=[[32, 4]], channel_multiplier=-1)
            w2 = [smallp.tile([128, BROWS], bf16, tag=f"w2_{t}", name=f"w2t{t}")
                  for t in range(8)]
            for t in range(8):
                nc.vector.memset(w2[t][:], 0.0)
            kwS = smallp.tile([128, 8 * CP4], bf16, tag="kwS")   # knn@Wc2 (+ones col)
            for t in range(8):
                nc.vector.memset(kwS[:, t * CP4 + C:t * CP4 + C + 1], 1.0)

            # ---- qprojT [128(a), 2*32] ----
            qp_ps = psmi.tile([128, CP4], f32, tag="mi")
            for at in range(2):
                for dc in range(4):
                    nc.tensor.matmul(
                        qp_ps[:, at * BROWS:(at + 1) * BROWS],
                        lhsT=Wq[dc][:, at * 128:(at + 1) * 128], rhs=qT[dc],
                        start=(dc == 0), stop=(dc == 3))
            qprojT = smallp.tile([128, 2 * BROWS], f32, tag="qprojT")
            nc.scalar.copy(out=qprojT[:], in_=qp_ps[:, :2 * BROWS])

            # ---- kprojT + h = tanh(. + qproj + bqm) -> scores row ----
            sc_ps = pssc.tile([128, NCD], f32, tag="sc")
            hT = [bigp.tile([128, NCD], bf16, tag=f"hT{at}", name=f"hTt{at}")
                  for at in range(2)]
            for at in range(2):
                kp = pskp.tile([128, NCD], f32, tag="kp")
                for half in range(2):
                    for dc in range(4):
                        nc.tensor.matmul(
                            kp[:, half * 512:(half + 1) * 512],
                            lhsT=Wm[dc][:, at * 128:(at + 1) * 128],
                            rhs=kt[:, half * 2048 + dc * 512:half * 2048 + (dc + 1) * 512],
                            start=(dc == 0), stop=(dc == 3))
                for half in range(2):
                    cols = slice(half * 512, (half + 1) * 512)
                    qb = qprojT[:, at * BROWS + half * 16:at * BROWS + half * 16 + 16,
                                None].to_broadcast([128, 16, K])
                    nc.vector.tensor_tensor(
                        hT[at][:, cols].rearrange("p (b k) -> p b k", k=K),
                        kp[:, cols].rearrange("p (b k) -> p b k", k=K),
                        qb, mybir.AluOpType.add)
                    nc.scalar.activation(hT[at][:, cols], hT[at][:, cols],
                                         mybir.ActivationFunctionType.Tanh,
                                         bias=bqm[:, at:at + 1])
                for half in range(2):
                    nc.tensor.matmul(
                        sc_ps[:1, half * 512:(half + 1) * 512],
                        lhsT=Ws[at],
                        rhs=hT[at][:, half * 512:(half + 1) * 512],
                        start=(at == 0), stop=(at == 1))

            # ---- knnWc[t] = knn_block_t @ Wc2 (overlaps scores tail) ----
            for t in range(8):
                kw_ps = psmi.tile([128, CP4], f32, tag="mi")
                half, blk = t // 4, t % 4
                for dc in range(4):
                    nc.tensor.matmul(
                        kw_ps[:, :C],
                        lhsT=kt[:, half * 2048 + dc * 512 + blk * 128:
                                half * 2048 + dc * 512 + (blk + 1) * 128],
                        rhs=Wc[4 + dc],
                        start=(dc == 0), stop=(dc == 3))
                nc.vector.tensor_copy(kwS[:, t * CP4:t * CP4 + C], kw_ps[:, :C])

            # ---- e row -> e_col via PE transposes; softmax weights ----
            e_row = smallp.tile([1, NCD], f32, tag="e_row")
            nc.scalar.activation(e_row[:], sc_ps[:1, :],
                                 mybir.ActivationFunctionType.Exp)
            ecT_ps = psmi.tile([128, CP4], f32, tag="mi")
            for t in range(8):
                nc.tensor.transpose(ecT_ps[:, t:t + 1],
                                    e_row[:, t * 128:(t + 1) * 128], ident1[:])
            e_col = smallp.tile([128, 8], f32, tag="e_col")
            nc.scalar.copy(out=e_col[:], in_=ecT_ps[:, :8])
            for t in range(8):
                nc.vector.tensor_scalar_mul(w2[t][:, 4 * t:4 * t + 4], mask4[:],
                                            e_col[:, t:t + 1])

            # ---- y2[b,:] = sum_k e * knnWc ; col C = sum_k e (den) ----
            y2_ps = psmi.tile([128, CP4], f32, tag="mi")
            for t in range(8):
                nc.tensor.matmul(y2_ps[:BROWS, :C + 1], lhsT=w2[t][:],
                                 rhs=kwS[:, t * CP4:t * CP4 + C + 1],
                                 start=(t == 0), stop=(t == 7))
            # ---- y1 = relu(q) @ Wc1 ----
            y1_ps = psmi.tile([128, CP4], f32, tag="mi")
            for dc in range(4):
                nc.tensor.matmul(y1_ps[:BROWS, :C], lhsT=qT[dc], rhs=Wc[dc],
                                 start=(dc == 0), stop=(dc == 3))

            osb = smallp.tile([2 * BROWS, CP4], f32, tag="osb")
            nc.scalar.copy(out=osb[BROWS:, :C + 1], in_=y2_ps[:BROWS, :C + 1])
            nc.scalar.copy(out=osb[:BROWS, :C], in_=y1_ps[:BROWS, :C])
            nc.sync.dma_start(out=out_d[:, :], in_=osb[:])
    nc.finalize()
    return nc


def _phase1_nc():
    global _PH1
    if _PH1 is None:
        _PH1 = _build_phase1()
    return _PH1


def _phase2_nc():
    global _PH2
    if _PH2 is None:
        _PH2 = _build_phase2()
    return _PH2


def kernel(query_feat, memory_keys, Wq, bq, Wm, bm, Ws, bs, Wc, bc):
    query_feat = np.asarray(query_feat, np.float32)
    memory_keys = np.asarray(memory_keys, np.float32)

    # ---- host prep: pad + normalize + transpose + shard keys (bf16) ----
    kn = np.sqrt((memory_keys ** 2).sum(axis=1))
    khat = memory_keys * (1.0 / kn)[:, None]
    pad = np.full((NPAD - N, D), -1.0 / np.sqrt(D), np.float32)
    khat_pad = np.concatenate([khat.astype(np.float32), pad], axis=0)
    q32 = np.maximum(query_feat, 0)
    qT_full = np.ascontiguousarray(q32.T.astype(BF))  # [512, 256] bf16

    ph1 = _phase1_nc()
    in_maps = []
    for c in range(NC_CORES):
        sh = khat_pad[c * SHARD:(c + 1) * SHARD]          # [12800, 512]
        arr = np.ascontiguousarray(
            sh.reshape(NCHUNK, CHUNK, 4, 128).transpose(0, 3, 2, 1).astype(BF)
        ).reshape(NCHUNK, 128, 4 * CHUNK)
        in_maps.append({"khatT": arr, "qT": qT_full})
    res1 = run_bass_kernel_spmd(ph1, in_maps, core_ids=list(range(NC_CORES)))

    # ---- host merge: recover indices, exact re-score of candidates ----
    all_gidx = np.zeros((B, NC_CORES, KLOC), np.int64)
    for c in range(NC_CORES):
        win = np.asarray(res1.results[c]["win"]).view(np.uint32)
        pos = np.asarray(res1.results[c]["pos"]).astype(np.int64)  # 0..199
        seg = pos // 8
        within = (win & np.uint32(0x1FF)).astype(np.int64)
        all_gidx[:, c, :] = seg * CHUNK + within + c * SHARD
    gidx = all_gidx.reshape(B, CAND)
    safe = np.minimum(gidx, N - 1)
    cand_keys = memory_keys[safe]                       # [256, 256, 512]
    dots = np.einsum("bd,bcd->bc", q32, cand_keys, optimize=True)
    cos = dots / np.maximum(
        np.linalg.norm(q32, axis=1)[:, None] * kn[safe], np.float32(1e-8))
    cos[gidx >= N] = -np.inf                            # mask dummy-pad hits
    order = np.argsort(-cos, axis=1, kind="stable")[:, :K]
    top_idx = np.take_along_axis(safe, order, axis=1)   # [256, 32]

    # ---- phase 2 (batch sharded): pack one bf16 blob per core ----
    ph2 = _phase2_nc()
    bqm_f = (np.asarray(bq, np.float32) + np.asarray(bm, np.float32))
    Wm_b = np.asarray(Wm, np.float32).reshape(4, 128, A).transpose(1, 0, 2).reshape(128, 1024)
    Wq_b = np.asarray(Wq, np.float32).reshape(4, 128, A).transpose(1, 0, 2).reshape(128, 1024)
    Ws_b = np.asarray(Ws, np.float32)[:, 0].reshape(2, 128).T         # [128, 2]
    Wc_b = np.asarray(Wc, np.float32).reshape(8, 128, C).transpose(1, 0, 2).reshape(128, 800)
    bqm_u16 = np.ascontiguousarray(
        bqm_f.reshape(2, 128).T.astype(np.float32)).view(np.uint16)   # [128, 4]

    wpart = np.zeros((128, W_W), np.uint16)
    wpart[:, W_WM:W_WM + 1024] = Wm_b.astype(BF).view(np.uint16)
    wpart[:, W_WQ:W_WQ + 1024] = Wq_b.astype(BF).view(np.uint16)
    wpart[:, W_WS:W_WS + 2] = Ws_b.astype(BF).view(np.uint16)
    wpart[:, W_BQM:W_BQM + 4] = bqm_u16
    wpart[:, W_WC:W_WC + 800] = Wc_b.astype(BF).view(np.uint16)

    in_maps2 = []
    for c in range(NC_CORES):
        rows = slice(c * BROWS, (c + 1) * BROWS)
        knn_rows = memory_keys[top_idx[rows]].reshape(NCD, D)
        ktp = np.ascontiguousarray(
            knn_rows.reshape(2, 512, 4, 128).transpose(3, 0, 2, 1)
        ).reshape(128, KT_W).astype(BF).view(np.uint16)
        qTc = np.ascontiguousarray(
            q32[rows].T.reshape(4, 128, BROWS).transpose(1, 0, 2)
        ).reshape(128, 128).astype(BF).view(np.uint16)
        blob = np.zeros((128, BLOB_W), np.uint16)
        blob[:, :KT_W] = ktp
        blob[:, KT_W:] = wpart
        blob[:, KT_W + W_QT:KT_W + W_QT + 128] = qTc
        in_maps2.append({"blob": blob.view(BF)})
    res2 = run_bass_kernel_spmd(ph2, in_maps2, core_ids=list(range(NC_CORES)))

    out = np.zeros((B, C), np.float32)
    for c in range(NC_CORES):
        r = np.asarray(res2.results[c]["out"], np.float32)   # [64, 104]
        y1 = r[:BROWS, :C]
        y2 = r[BROWS:, :C]
        den = r[BROWS:, C]
        out[c * BROWS:(c + 1) * BROWS] = y1 + y2 / den[:, None]
    return (out + np.asarray(bc, np.float32)[None, :]).astype(np.float32)


# revision 6
# speedup vs baseline: 1.3830x; 1.0858x over previous
"""Trainium2 Bass kernel for retrieval-knn attention classifier (nn_MA_51866025067137).

Strategy (8 NeuronCores):
  Phase 1 — memory_keys sharded along N (12800 keys/core, padded 100000->102400
  with dummy rows), fed in bf16 (keys pre-normalized on host so the matmul
  directly yields cosine ranking values; host re-scores candidates in exact
  fp32 afterwards, so ranking precision only has to preserve the top-32 set).
  Per chunk of 512 keys: PE computes sims for all 256 queries (bf16 matmuls,
  fp32 PSUM), ACT evicts both query-halves in one [128,1024] copy, GPSIMD
  packs a 9-bit in-chunk index into the sim mantissa, DVE max8 keeps the
  top-8 per chunk.  Tail: 4 rounds of max8/max_index/match_replace extract
  the per-core top-32 (value, position) per query row.
  Host — merges the 8x32 candidates per row, re-scores them exactly in fp32,
  and gathers the global top-32 key vectors.
  Phase 2 — batch sharded (32 queries/core), all inputs packed into one bf16
  blob (2 logical DMAs): memory-attention module via bf16 matmuls; the
  softmax-score row is transposed with 8 tiny PE transposes (no DRAM bounce);
  attended@Wc is reassociated as sum_k w_k * (knn_k @ Wc2) so the weighted
  sum runs over a precomputed [1024,100] knnWc instead of [1024,512] knn
  (no knn tile, no attT transpose); normalization by sum(e) happens on host.
"""

import numpy as np
import ml_dtypes

import concourse.bacc as bacc
import concourse.mybir as mybir
from concourse.tile import TileContext
from concourse.bass_utils import run_bass_kernel_spmd

# problem dims (hardcoded per harness contract)
B, N, D = 256, 100000, 512
A, C, K = 256, 100, 32
NC_CORES = 8
NPAD = 102400             # 8 * 12800
SHARD = NPAD // NC_CORES  # 12800
CHUNK = 512               # keys per inner loop step
NCHUNK = SHARD // CHUNK   # 25
L1W = NCHUNK * 8          # 200
BROWS = B // NC_CORES     # 32 rows per core in phase 2
KLOC = 32                 # local candidates per core per row
NROUND = KLOC // 8        # 4 extraction rounds
CAND = NC_CORES * KLOC    # 256 merged candidates per row

f32 = mybir.dt.float32
f32r = mybir.dt.float32r
bf16 = mybir.dt.bfloat16
u32 = mybir.dt.uint32
BF = ml_dtypes.bfloat16

# ---- phase-2 blob layout (bf16 columns) ----
NCD = BROWS * K           # 1024
# ktile: [128, 4096], col = half*2048 + dc*512 + i  (bk = half*512 + i)
KT_W = 4096
# wtile: [128, 2984]
W_WM = 0                  # 4 dc x 256
W_WQ = 1024               # 4 dc x 256
W_WS = 2048               # 2 (col at)
W_BQM = 2050              # 4 bf16 cols = [128,2] f32 (byte offset 4100, 4-aligned)
W_QT = 2054               # 4 dc x 32
W_WC = 2182               # 8 m x 100
W_W = 2984                # padded (2982 used)
BLOB_W = KT_W + W_W       # 7080
CP4 = C + 4               # 104: knnWc stride (100 vals, col 100 = 1.0)

_PH1 = None
_PH2 = None


def _build_phase1():
    nc = bacc.Bacc("TRN2", target_bir_lowering=False)
    khatT = nc.dram_tensor("khatT", [NCHUNK, 128, 4 * CHUNK], bf16, kind="ExternalInput")
    qT = nc.dram_tensor("qT", [D, B], bf16, kind="ExternalInput")
    win_out = nc.dram_tensor("win", [B, KLOC], f32, kind="ExternalOutput")

    with TileContext(nc) as tc:
        with (
            tc.tile_pool(name="const", bufs=1) as constp,
            tc.tile_pool(name="qpool", bufs=1) as qpool,
            tc.tile_pool(name="keys", bufs=5) as keyp,
            tc.tile_pool(name="ev", bufs=4) as evp,
            tc.tile_pool(name="pk", bufs=4) as pkp,
            tc.tile_pool(name="iota", bufs=4) as iotap,
            tc.tile_pool(name="l1", bufs=1) as l1p,
            tc.tile_pool(name="small", bufs=1) as smallp,
            tc.tile_pool(name="psum", bufs=3, space="PSUM") as psump,
            tc.tile_pool(name="pswarm", bufs=1, space="PSUM") as pswarm,
        ):
            # AND-mask keeps sign+exp+9 mantissa bits; low 14 bits carry the
            # in-shard key index (chunk*512 + j, < 16384).
            mask_t = constp.tile([128, 1], u32, tag="mask")
            nc.vector.memset(mask_t[:], 0xFFFFC000)

            # PE p-state warmup: keep the tensor engine queue non-empty from
            # ~1us until the first key chunk lands (~7us) so the clock ramps
            # to 2.4GHz before real matmuls start.
            warm = constp.tile([128, CHUNK], bf16, tag="warm")
            nc.vector.memset(warm[:], 0.0)
            warm_ps = pswarm.tile([128, CHUNK], f32, tag="wps")
            for i in range(14):
                nc.tensor.matmul(warm_ps[:], lhsT=warm[:, :128], rhs=warm[:],
                                 start=True, stop=True)

            # qT already relu'd + bf16 on host
            qTr = []
            for dc in range(4):
                t = qpool.tile([128, B], bf16, tag=f"qt{dc}")
                nc.sync.dma_start(out=t[:], in_=qT[dc * 128:(dc + 1) * 128, :])
                qTr.append(t)

            L1 = [l1p.tile([128, L1W], f32, tag=f"l1_{qt}", name=f"l1_{qt}")
                  for qt in range(2)]

            for c in range(NCHUNK):
                kt = keyp.tile([128, 4 * CHUNK], bf16, tag="kt")
                nc.sync.dma_start(out=kt[:], in_=khatT[c, :, :])
                iota_t = iotap.tile([128, CHUNK], u32, tag="iota")
                nc.gpsimd.iota(iota_t[:], pattern=[[1, CHUNK]], base=c * CHUNK,
                               channel_multiplier=0)
                ps = psump.tile([128, 2 * CHUNK], f32, tag="sim")
                for qt in range(2):
                    for dc in range(4):
                        nc.tensor.matmul(
                            ps[:, qt * CHUNK:(qt + 1) * CHUNK],
                            lhsT=qTr[dc][:, qt * 128:(qt + 1) * 128],
                            rhs=kt[:, dc * CHUNK:(dc + 1) * CHUNK],
                            start=(dc == 0), stop=(dc == 3),
                        )
                # evict both query-halves at once (ACT), pack index bits (DVE)
                ev = evp.tile([128, 2 * CHUNK], f32, tag="ev")
                nc.scalar.copy(out=ev[:], in_=ps[:])
                pk = pkp.tile([128, 2 * CHUNK], f32, tag="pk")
                nc.vector.scalar_tensor_tensor(
                    out=pk[:].bitcast(u32).rearrange("p (a b) -> p a b", a=2),
                    in0=ev[:].bitcast(u32).rearrange("p (a b) -> p a b", a=2),
                    scalar=mask_t[:],
                    in1=iota_t[:, None, :].to_broadcast([128, 2, CHUNK]),
                    op0=mybir.AluOpType.bitwise_and,
                    op1=mybir.AluOpType.bitwise_or,
                )
                for qt in range(2):
                    nc.vector.max(out=L1[qt][:, c * 8:(c + 1) * 8],
                                  in_=pk[:, qt * CHUNK:(qt + 1) * CHUNK])

            # extraction: NROUND rounds of top-8 from L1 (200 wide); winners
            # carry their in-shard index in the low 14 bits -> no max_index
            for qt in range(2):
                win = smallp.tile([128, KLOC], f32, tag=f"win{qt}")
                for r in range(NROUND):
                    w8 = win[:, r * 8:(r + 1) * 8]
                    nc.vector.max(out=w8, in_=L1[qt][:])
                    if r < NROUND - 1:
                        nc.vector.match_replace(out=L1[qt][:], in_to_replace=w8,
                                                in_values=L1[qt][:],
                                                imm_value=-3.0e38)
                nc.sync.dma_start(out=win_out[qt * 128:(qt + 1) * 128, :], in_=win[:])
    nc.finalize()
    return nc


def _build_phase2():
    nc = bacc.Bacc("TRN2", target_bir_lowering=False)
    blob = nc.dram_tensor("blob", [128, BLOB_W], bf16, kind="ExternalInput")
    out_d = nc.dram_tensor("out", [2 * BROWS, CP4], f32, kind="ExternalOutput")

    with TileContext(nc) as tc:
        with (
            tc.tile_pool(name="big", bufs=1) as bigp,
            tc.tile_pool(name="small", bufs=1) as smallp,
            tc.tile_pool(name="pskp", bufs=2, space="PSUM") as pskp,
            tc.tile_pool(name="pssc", bufs=1, space="PSUM") as pssc,
            tc.tile_pool(name="psmi", bufs=2, space="PSUM") as psmi,
        ):
            # ---- DMAs: weights first (small), then knnT halves ----
            wt = bigp.tile([128, W_W], bf16, tag="wt")
            nc.sync.dma_start(out=wt[:, :2982], in_=blob[:, KT_W:KT_W + 2982])
            kt = bigp.tile([128, KT_W], bf16, tag="ktile")
            for half in range(2):
                nc.sync.dma_start(out=kt[:, half * 2048:(half + 1) * 2048],
                                  in_=blob[:, half * 2048:(half + 1) * 2048])

            Wm = [wt[:, W_WM + dc * A:W_WM + (dc + 1) * A] for dc in range(4)]
            Wq = [wt[:, W_WQ + dc * A:W_WQ + (dc + 1) * A] for dc in range(4)]
            Ws = [wt[:, W_WS + at:W_WS + at + 1] for at in range(2)]
            bqm = wt[:, W_BQM:W_BQM + 4].bitcast(f32)            # [128, 2]
            qT = [wt[:, W_QT + dc * BROWS:W_QT + (dc + 1) * BROWS] for dc in range(4)]
            Wc = [wt[:, W_WC + m * C:W_WC + (m + 1) * C] for m in range(8)]

            # ---- small constants ----
            ident1 = smallp.tile([1, 1], f32, tag="id1")
            nc.vector.memset(ident1[:], 1.0)
            # mask4[p, j] = 1.0 iff j == p // 32
            mask4 = smallp.tile([128, 4], f32, tag="mask4")
            nc.vector.memset(mask4[:], 1.0)
            nc.gpsimd.affine_select(out=mask4[:], in_=mask4[:],
                                    compare_op=mybir.AluOpType.is_ge, fill=0.0,
                                    base=0, pattern=[[-32, 4]], channel_multiplier=1)
            nc.gpsimd.affine_select(out=mask4[:], in_=mask4[:],
                                    compare_op=mybir.AluOpType.is_ge, fill=0.0,
                                    base=31, pattern=[[32, 4]], channel_multiplier=-1)
            w2 = [smallp.tile([128, BROWS], bf16, tag=f"w2_{t}", name=f"w2t{t}")
                  for t in range(8)]
            for t in range(8):
                nc.vector.memset(w2[t][:], 0.0)
            kwS = smallp.tile([128, 8 * CP4], bf16, tag="kwS")   # knn@Wc2 (+ones col)
            for t in range(8):
                nc.vector.memset(kwS[:, t * CP4 + C:t * CP4 + C + 1], 1.0)

            # ---- qprojT [128(a), 2*32] ----
            qp_ps = psmi.tile([128, CP4], f32, tag="mi")
            for at in range(2):
                for dc in range(4):
                    nc.tensor.matmul(
                        qp_ps[:, at * BROWS:(at + 1) * BROWS],
                        lhsT=Wq[dc][:, at * 128:(at + 1) * 128], rhs=qT[dc],
                        start=(dc == 0), stop=(dc == 3))
            qprojT = smallp.tile([128, 2 * BROWS], f32, tag="qprojT")
            nc.scalar.copy(out=qprojT[:], in_=qp_ps[:, :2 * BROWS])

            # ---- kprojT + h = tanh(. + qproj + bqm) -> scores row ----
            sc_ps = pssc.tile([128, NCD], f32, tag="sc")
            hT = [bigp.tile([128, NCD], bf16, tag=f"hT{at}", name=f"hTt{at}")
                  for at in range(2)]
            for at in range(2):
                kp = pskp.tile([128, NCD], f32, tag="kp")
                for half in range(2):
                    for dc in range(4):
                        nc.tensor.matmul(
                            kp[:, half * 512:(half + 1) * 512],
                            lhsT=Wm[dc][:, at * 128:(at + 1) * 128],
                            rhs=kt[:, half * 2048 + dc * 512:half * 2048 + (dc + 1) * 512],
                            start=(dc == 0), stop=(dc == 3))
                for half in range(2):
                    cols = slice(half * 512, (half + 1) * 512)
                    qb = qprojT[:, at * BROWS + half * 16:at * BROWS + half * 16 + 16,
                                None].to_broadcast([128, 16, K])
                    nc.vector.tensor_tensor(
                        hT[at][:, cols].rearrange("p (b k) -> p b k", k=K),
                        kp[:, cols].rearrange("p (b k) -> p b k", k=K),
                        qb, mybir.AluOpType.add)
                    nc.scalar.activation(hT[at][:, cols], hT[at][:, cols],
                                         mybir.ActivationFunctionType.Tanh,
                                         bias=bqm[:, at:at + 1])
                for half in range(2):
                    nc.tensor.matmul(
                        sc_ps[:1, half * 512:(half + 1) * 512],
                        lhsT=Ws[at],
                        rhs=hT[at][:, half * 512:(half + 1) * 512],
                        start=(at == 0), stop=(at == 1))

            # ---- knnWc[t] = knn_block_t @ Wc2 (overlaps scores tail) ----
            for t in range(8):
                kw_ps = psmi.tile([128, CP4], f32, tag="mi")
                half, blk = t // 4, t % 4
                for dc in range(4):
                    nc.tensor.matmul(
                        kw_ps[:, :C],
                        lhsT=kt[:, half * 2048 + dc * 512 + blk * 128:
                                half * 2048 + dc * 512 + (blk + 1) * 128],
                        rhs=Wc[4 + dc],
                        start=(dc == 0), stop=(dc == 3))
                nc.vector.tensor_copy(kwS[:, t * CP4:t * CP4 + C], kw_ps[:, :C])

            # ---- e row -> e_col via PE transposes; softmax weights ----
            e_row = smallp.tile([1, NCD], f32, tag="e_row")
            nc.scalar.activation(e_row[:], sc_ps[:1, :],
                                 mybir.ActivationFunctionType.Exp)
            ecT_ps = psmi.tile([128, CP4], f32, tag="mi")
            for t in range(8):
                nc.tensor.transpose(ecT_ps[:, t:t + 1],
                                    e_row[:, t * 128:(t + 1) * 128], ident1[:])
            e_col = smallp.tile([128, 8], f32, tag="e_col")
            nc.scalar.copy(out=e_col[:], in_=ecT_ps[:, :8])
            for t in range(8):
                nc.vector.tensor_scalar_mul(w2[t][:, 4 * t:4 * t + 4], mask4[:],
                                            e_col[:, t:t + 1])

            # ---- y2[b,:] = sum_k e * knnWc ; col C = sum_k e (den) ----
            y2_ps = psmi.tile([128, CP4], f32, tag="mi")
            for t in range(8):
                nc.tensor.matmul(y2_ps[:BROWS, :C + 1], lhsT=w2[t][:],
                                 rhs=kwS[:, t * CP4:t * CP4 + C + 1],
                                 start=(t == 0), stop=(t == 7))
            # ---- y1 = relu(q) @ Wc1 ----
            y1_ps = psmi.tile([128, CP4], f32, tag="mi")
            for dc in range(4):
                nc.tensor.matmul(y1_ps[:BROWS, :C], lhsT=qT[dc], rhs=Wc[dc],
                                 start=(dc == 0), stop=(dc == 3))

            osb = smallp.tile([2 * BROWS, CP4], f32, tag="osb")
            nc.scalar.copy(out=osb[BROWS:, :C + 1], in_=y2_ps[:BROWS, :C + 1])
            nc.scalar.copy(out=osb[:BROWS, :C], in_=y1_ps[:BROWS, :C])
            nc.sync.dma_start(out=out_d[:, :], in_=osb[:])
    nc.finalize()
    return nc


def _phase1_nc():
    global _PH1
    if _PH1 is None:
        _PH1 = _build_phase1()
    return _PH1


def _phase2_nc():
    global _PH2
    if _PH2 is None:
        _PH2 = _build_phase2()
    return _PH2


def kernel(query_feat, memory_keys, Wq, bq, Wm, bm, Ws, bs, Wc, bc):
    query_feat = np.asarray(query_feat, np.float32)
    memory_keys = np.asarray(memory_keys, np.float32)

    # ---- host prep: pad + normalize + transpose + shard keys (bf16) ----
    kn = np.sqrt((memory_keys ** 2).sum(axis=1))
    khat = memory_keys * (1.0 / kn)[:, None]
    pad = np.full((NPAD - N, D), -1.0 / np.sqrt(D), np.float32)
    khat_pad = np.concatenate([khat.astype(np.float32), pad], axis=0)
    q32 = np.maximum(query_feat, 0)
    qT_full = np.ascontiguousarray(q32.T.astype(BF))  # [512, 256] bf16

    ph1 = _phase1_nc()
    in_maps = []
    for c in range(NC_CORES):
        sh = khat_pad[c * SHARD:(c + 1) * SHARD]          # [12800, 512]
        arr = np.ascontiguousarray(
            sh.reshape(NCHUNK, CHUNK, 4, 128).transpose(0, 3, 2, 1).astype(BF)
        ).reshape(NCHUNK, 128, 4 * CHUNK)
        in_maps.append({"khatT": arr, "qT": qT_full})
    res1 = run_bass_kernel_spmd(ph1, in_maps, core_ids=list(range(NC_CORES)))

    # ---- host merge: recover indices, exact re-score of candidates ----
    all_gidx = np.zeros((B, NC_CORES, KLOC), np.int64)
    for c in range(NC_CORES):
        win = np.asarray(res1.results[c]["win"]).view(np.uint32)
        within = (win & np.uint32(0x3FFF)).astype(np.int64)  # in-shard index
        all_gidx[:, c, :] = within + c * SHARD
    gidx = all_gidx.reshape(B, CAND)
    safe = np.minimum(gidx, N - 1)
    cand_keys = memory_keys[safe]                       # [256, 256, 512]
    dots = np.einsum("bd,bcd->bc", q32, cand_keys, optimize=True)
    cos = dots / np.maximum(
        np.linalg.norm(q32, axis=1)[:, None] * kn[safe], np.float32(1e-8))
    cos[gidx >= N] = -np.inf                            # mask dummy-pad hits
    order = np.argsort(-cos, axis=1, kind="stable")[:, :K]
    top_idx = np.take_along_axis(safe, order, axis=1)   # [256, 32]

    # ---- phase 2 (batch sharded): pack one bf16 blob per core ----
    ph2 = _phase2_nc()
    bqm_f = (np.asarray(bq, np.float32) + np.asarray(bm, np.float32))
    Wm_b = np.asarray(Wm, np.float32).reshape(4, 128, A).transpose(1, 0, 2).reshape(128, 1024)
    Wq_b = np.asarray(Wq, np.float32).reshape(4, 128, A).transpose(1, 0, 2).reshape(128, 1024)
    Ws_b = np.asarray(Ws, np.float32)[:, 0].reshape(2, 128).T         # [128, 2]
    Wc_b = np.asarray(Wc, np.float32).reshape(8, 128, C).transpose(1, 0, 2).reshape(128, 800)
    bqm_u16 = np.ascontiguousarray(
        bqm_f.reshape(2, 128).T.astype(np.float32)).view(np.uint16)   # [128, 4]

    wpart = np.zeros((128, W_W), np.uint16)
    wpart[:, W_WM:W_WM + 1024] = Wm_b.astype(BF).view(np.uint16)
    wpart[:, W_WQ:W_WQ + 1024] = Wq_b.astype(BF).view(np.uint16)
    wpart[:, W_WS:W_WS + 2] = Ws_b.astype(BF).view(np.uint16)
    wpart[:, W_BQM:W_BQM + 4] = bqm_u16
    wpart[:, W_WC:W_WC + 800] = Wc_b.astype(BF).view(np.uint16)

    in_maps2 = []
    for c in range(NC_CORES):
        rows = slice(c * BROWS, (c + 1) * BROWS)
        knn_rows = memory_keys[top_idx[rows]].reshape(NCD, D)
        ktp = np.ascontiguousarray(
            knn_rows.reshape(2, 512, 4, 128).transpose(3, 0, 2, 1)
        ).reshape(128, KT_W).astype(BF).view(np.uint16)
        qTc = np.ascontiguousarray(
            q32[rows].T.reshape(4, 128, BROWS).transpose(1, 0, 2)
        ).reshape(128, 128).astype(BF).view(np.uint16)
        blob = np.zeros((128, BLOB_W), np.uint16)
        blob[:, :KT_W] = ktp
        blob[:, KT_W:] = wpart
        blob[:, KT_W + W_QT:KT_W + W_QT + 128] = qTc
        in_maps2.append({"blob": blob.view(BF)})
    res2 = run_bass_kernel_spmd(ph2, in_maps2, core_ids=list(range(NC_CORES)))

    out = np.zeros((B, C), np.float32)
    for c in range(NC_CORES):
        r = np.asarray(res2.results[c]["out"], np.float32)   # [64, 104]
        y1 = r[:BROWS, :C]
        y2 = r[BROWS:, :C]
        den = r[BROWS:, C]
        out[c * BROWS:(c + 1) * BROWS] = y1 + y2 / den[:, None]
    return (out + np.asarray(bc, np.float32)[None, :]).astype(np.float32)


# revision 9
# speedup vs baseline: 1.6666x; 1.2051x over previous
"""Trainium2 Bass kernel for retrieval-knn attention classifier (nn_MA_51866025067137).

Strategy (8 NeuronCores):
  Phase 1 — memory_keys sharded along N (12800 keys/core, padded 100000->102400
  with dummy rows), fed in bf16 (keys pre-normalized on host so the matmul
  directly yields cosine ranking values; host re-scores candidates in exact
  fp32 afterwards, so ranking precision only has to preserve the top-32 set).
  Per chunk of 512 keys: PE computes sims for all 256 queries (bf16 matmuls,
  fp32 PSUM), ACT evicts both query-halves in one [128,1024] copy, GPSIMD
  packs a 9-bit in-chunk index into the sim mantissa, DVE max8 keeps the
  top-8 per chunk.  Tail: 4 rounds of max8/max_index/match_replace extract
  the per-core top-32 (value, position) per query row.
  Host — merges the 8x32 candidates per row, re-scores them exactly in fp32,
  and gathers the global top-32 key vectors.
  Phase 2 — batch sharded (32 queries/core), all inputs packed into one bf16
  blob (2 logical DMAs): memory-attention module via bf16 matmuls; the
  softmax-score row is transposed with 8 tiny PE transposes (no DRAM bounce);
  attended@Wc is reassociated as sum_k w_k * (knn_k @ Wc2) so the weighted
  sum runs over a precomputed [1024,100] knnWc instead of [1024,512] knn
  (no knn tile, no attT transpose); normalization by sum(e) happens on host.
"""

import numpy as np
import ml_dtypes

import concourse.bacc as bacc
import concourse.mybir as mybir
from concourse.tile import TileContext
from concourse.bass_utils import run_bass_kernel_spmd

# problem dims (hardcoded per harness contract)
B, N, D = 256, 100000, 512
A, C, K = 256, 100, 32
NC_CORES = 8
NPAD = 102400             # 8 * 12800
SHARD = NPAD // NC_CORES  # 12800
CHUNK = 512               # keys per inner loop step
NCHUNK = SHARD // CHUNK   # 25
L1W = NCHUNK * 8          # 200
BROWS = B // NC_CORES     # 32 rows per core in phase 2
KLOC = 32                 # local candidates per core per row
NROUND = KLOC // 8        # 4 extraction rounds
CAND = NC_CORES * KLOC    # 256 merged candidates per row

f32 = mybir.dt.float32
f32r = mybir.dt.float32r
bf16 = mybir.dt.bfloat16
u32 = mybir.dt.uint32
BF = ml_dtypes.bfloat16

# ---- phase-2 blob layout (bf16 columns) ----
NCD = BROWS * K           # 1024
# ktile: [128, 4096], col = half*2048 + dc*512 + i  (bk = half*512 + i)
KT_W = 4096
# wtile: [128, 2984]
W_WM = 0                  # 4 dc x 256
W_WQ = 1024               # 4 dc x 256
W_WS = 2048               # 2 (col at)
W_BQM = 2050              # 4 bf16 cols = [128,2] f32 (byte offset 4100, 4-aligned)
W_QT = 2054               # 4 dc x 32
W_WC = 2182               # 8 m x 100
W_W = 2984                # padded (2982 used)
BLOB_W = KT_W + W_W       # 7080
CP4 = C + 4               # 104: knnWc stride (100 vals, col 100 = 1.0)

_PH1 = None
_PH2 = None


def _build_phase1():
    nc = bacc.Bacc("TRN2", target_bir_lowering=False)
    khatT = nc.dram_tensor("khatT", [NCHUNK, 128, 4 * CHUNK], bf16, kind="ExternalInput")
    qT = nc.dram_tensor("qT", [D, B], bf16, kind="ExternalInput")
    win_out = nc.dram_tensor("win", [B, KLOC], f32, kind="ExternalOutput")

    with TileContext(nc) as tc:
        with (
            tc.tile_pool(name="const", bufs=1) as constp,
            tc.tile_pool(name="qpool", bufs=1) as qpool,
            tc.tile_pool(name="keys", bufs=5) as keyp,
            tc.tile_pool(name="pk", bufs=4) as pkp,
            tc.tile_pool(name="l1", bufs=1) as l1p,
            tc.tile_pool(name="small", bufs=1) as smallp,
            tc.tile_pool(name="psum", bufs=3, space="PSUM") as psump,
            tc.tile_pool(name="pswarm", bufs=1, space="PSUM") as pswarm,
        ):
            # PE p-state warmup: keep the tensor engine queue non-empty from
            # ~1us until the first key chunk lands so the clock ramps to
            # 2.4GHz before real matmuls start.
            warm = constp.tile([128, CHUNK], bf16, tag="warm")
            nc.vector.memset(warm[:], 0.0)
            warm_ps = pswarm.tile([128, CHUNK], f32, tag="wps")
            for i in range(12):
                nc.tensor.matmul(warm_ps[:], lhsT=warm[:, :128], rhs=warm[:],
                                 start=True, stop=True)

            # qT already relu'd + bf16 on host; one DMA, dc-major [128, 4*256]
            qTall = qpool.tile([128, 4 * B], bf16, tag="qtall")
            nc.sync.dma_start(out=qTall[:].rearrange("p (dc b) -> p dc b", dc=4),
                              in_=qT[:, :].rearrange("(dc p) b -> p dc b", p=128))
            qTr = [qTall[:, dc * B:(dc + 1) * B] for dc in range(4)]

            L1 = [l1p.tile([128, L1W], f32, tag=f"l1_{qt}", name=f"l1_{qt}")
                  for qt in range(2)]

            for c in range(NCHUNK):
                kt = keyp.tile([128, 4 * CHUNK], bf16, tag="kt")
                nc.sync.dma_start(out=kt[:], in_=khatT[c, :, :])
                ps = psump.tile([128, 2 * CHUNK], f32, tag="sim")
                for qt in range(2):
                    for dc in range(4):
                        nc.tensor.matmul(
                            ps[:, qt * CHUNK:(qt + 1) * CHUNK],
                            lhsT=qTr[dc][:, qt * 128:(qt + 1) * 128],
                            rhs=kt[:, dc * CHUNK:(dc + 1) * CHUNK],
                            start=(dc == 0), stop=(dc == 3),
                        )
                # packed candidate word = bf16(sim) in the high u16 (ACT evict
                # with stride-2 bf16 output), in-shard index in the low u16
                # (GPSIMD iota into the even u16 lanes). Read as f32, words
                # rank by sim value with the index as tiebreaker.
                pk = pkp.tile([128, 2 * CHUNK], f32, tag="pk")
                pk16 = pk[:].bitcast(bf16)          # [128, 4096] 2-byte lanes
                nc.gpsimd.iota(
                    pk16.rearrange("p (b two) -> p b two", two=2)[:, :, 0]
                        .bitcast(mybir.dt.uint16)
                        .rearrange("p (a b) -> p a b", a=2),
                    pattern=[[0, 2], [1, CHUNK]], base=c * CHUNK,
                    channel_multiplier=0)
                nc.scalar.copy(
                    out=pk16.rearrange("p (b two) -> p b two", two=2)[:, :, 1],
                    in_=ps[:])
                for qt in range(2):
                    nc.vector.max(out=L1[qt][:, c * 8:(c + 1) * 8],
                                  in_=pk[:, qt * CHUNK:(qt + 1) * CHUNK])

            # extraction: NROUND rounds of top-8 from L1 (200 wide); winners
            # carry their in-shard index in the low 14 bits -> no max_index
            for qt in range(2):
                win = smallp.tile([128, KLOC], f32, tag=f"win{qt}")
                for r in range(NROUND):
                    w8 = win[:, r * 8:(r + 1) * 8]
                    nc.vector.max(out=w8, in_=L1[qt][:])
                    if r < NROUND - 1:
                        nc.vector.match_replace(out=L1[qt][:], in_to_replace=w8,
                                                in_values=L1[qt][:],
                                                imm_value=-3.0e38)
                nc.sync.dma_start(out=win_out[qt * 128:(qt + 1) * 128, :], in_=win[:])
    nc.finalize()
    return nc


def _build_phase2():
    nc = bacc.Bacc("TRN2", target_bir_lowering=False)
    blob = nc.dram_tensor("blob", [128, BLOB_W], bf16, kind="ExternalInput")
    out_d = nc.dram_tensor("out", [2 * BROWS, CP4], f32, kind="ExternalOutput")

    with TileContext(nc) as tc:
        with (
            tc.tile_pool(name="big", bufs=1) as bigp,
            tc.tile_pool(name="small", bufs=1) as smallp,
            tc.tile_pool(name="pskp", bufs=2, space="PSUM") as pskp,
            tc.tile_pool(name="pssc", bufs=1, space="PSUM") as pssc,
            tc.tile_pool(name="psmi", bufs=2, space="PSUM") as psmi,
        ):
            # ---- DMAs: weights first (small), then knnT halves ----
            wt = bigp.tile([128, W_W], bf16, tag="wt")
            nc.sync.dma_start(out=wt[:, :2982], in_=blob[:, KT_W:KT_W + 2982])
            kt = bigp.tile([128, KT_W], bf16, tag="ktile")
            for half in range(2):
                nc.sync.dma_start(out=kt[:, half * 2048:(half + 1) * 2048],
                                  in_=blob[:, half * 2048:(half + 1) * 2048])

            Wm = [wt[:, W_WM + dc * A:W_WM + (dc + 1) * A] for dc in range(4)]
            Wq = [wt[:, W_WQ + dc * A:W_WQ + (dc + 1) * A] for dc in range(4)]
            Ws = [wt[:, W_WS + at:W_WS + at + 1] for at in range(2)]
            bqm = wt[:, W_BQM:W_BQM + 4].bitcast(f32)            # [128, 2]
            qT = [wt[:, W_QT + dc * BROWS:W_QT + (dc + 1) * BROWS] for dc in range(4)]
            Wc = [wt[:, W_WC + m * C:W_WC + (m + 1) * C] for m in range(8)]

            # ---- small constants ----
            ident1 = smallp.tile([1, 1], f32, tag="id1")
            nc.vector.memset(ident1[:], 1.0)
            # mask4[p, j] = 1.0 iff j == p // 32
            mask4 = smallp.tile([128, 4], f32, tag="mask4")
            nc.vector.memset(mask4[:], 1.0)
            nc.gpsimd.affine_select(out=mask4[:], in_=mask4[:],
                                    compare_op=mybir.AluOpType.is_ge, fill=0.0,
                                    base=0, pattern=[[-32, 4]], channel_multiplier=1)
            nc.gpsimd.affine_select(out=mask4[:], in_=mask4[:],
                                    compare_op=mybir.AluOpType.is_ge, fill=0.0,
                                    base=31, pattern=[[32, 4]], channel_multiplier=-1)
            w2 = [smallp.tile([128, BROWS], bf16, tag=f"w2_{t}", name=f"w2t{t}")
                  for t in range(8)]
            for t in range(8):
                nc.vector.memset(w2[t][:], 0.0)
            kwS = smallp.tile([128, 8 * CP4], bf16, tag="kwS")   # knn@Wc2 (+ones col)
            for t in range(8):
                nc.vector.memset(kwS[:, t * CP4 + C:t * CP4 + C + 1], 1.0)

            # ---- qprojT [128(a), 2*32] ----
            qp_ps = psmi.tile([128, CP4], f32, tag="mi")
            for at in range(2):
                for dc in range(4):
                    nc.tensor.matmul(
                        qp_ps[:, at * BROWS:(at + 1) * BROWS],
                        lhsT=Wq[dc][:, at * 128:(at + 1) * 128], rhs=qT[dc],
                        start=(dc == 0), stop=(dc == 3))
            qprojT = smallp.tile([128, 2 * BROWS], f32, tag="qprojT")
            nc.scalar.copy(out=qprojT[:], in_=qp_ps[:, :2 * BROWS])

            # ---- kprojT + h = tanh(. + qproj + bqm) -> scores row ----
            sc_ps = pssc.tile([128, NCD], f32, tag="sc")
            hT = [bigp.tile([128, NCD], bf16, tag=f"hT{at}", name=f"hTt{at}")
                  for at in range(2)]
            for at in range(2):
                kp = pskp.tile([128, NCD], f32, tag="kp")
                for half in range(2):
                    for dc in range(4):
                        nc.tensor.matmul(
                            kp[:, half * 512:(half + 1) * 512],
                            lhsT=Wm[dc][:, at * 128:(at + 1) * 128],
                            rhs=kt[:, half * 2048 + dc * 512:half * 2048 + (dc + 1) * 512],
                            start=(dc == 0), stop=(dc == 3))
                for half in range(2):
                    cols = slice(half * 512, (half + 1) * 512)
                    qb = qprojT[:, at * BROWS + half * 16:at * BROWS + half * 16 + 16,
                                None].to_broadcast([128, 16, K])
                    nc.vector.tensor_tensor(
                        hT[at][:, cols].rearrange("p (b k) -> p b k", k=K),
                        kp[:, cols].rearrange("p (b k) -> p b k", k=K),
                        qb, mybir.AluOpType.add)
                    nc.scalar.activation(hT[at][:, cols], hT[at][:, cols],
                                         mybir.ActivationFunctionType.Tanh,
                                         bias=bqm[:, at:at + 1])
                for half in range(2):
                    nc.tensor.matmul(
                        sc_ps[:1, half * 512:(half + 1) * 512],
                        lhsT=Ws[at],
                        rhs=hT[at][:, half * 512:(half + 1) * 512],
                        start=(at == 0), stop=(at == 1))

            # ---- knnWc[t] = knn_block_t @ Wc2 (overlaps scores tail) ----
            for t in range(8):
                kw_ps = psmi.tile([128, CP4], f32, tag="mi")
                half, blk = t // 4, t % 4
                for dc in range(4):
                    nc.tensor.matmul(
                        kw_ps[:, :C],
                        lhsT=kt[:, half * 2048 + dc * 512 + blk * 128:
                                half * 2048 + dc * 512 + (blk + 1) * 128],
                        rhs=Wc[4 + dc],
                        start=(dc == 0), stop=(dc == 3))
                nc.vector.tensor_copy(kwS[:, t * CP4:t * CP4 + C], kw_ps[:, :C])

            # ---- e row -> e_col via PE transposes; softmax weights ----
            e_row = smallp.tile([1, NCD], f32, tag="e_row")
            nc.scalar.activation(e_row[:], sc_ps[:1, :],
                                 mybir.ActivationFunctionType.Exp)
            ecT_ps = psmi.tile([128, CP4], f32, tag="mi")
            for t in range(8):
                nc.tensor.transpose(ecT_ps[:, t:t + 1],
                                    e_row[:, t * 128:(t + 1) * 128], ident1[:])
            e_col = smallp.tile([128, 8], f32, tag="e_col")
            nc.scalar.copy(out=e_col[:], in_=ecT_ps[:, :8])
            for t in range(8):
                nc.vector.tensor_scalar_mul(w2[t][:, 4 * t:4 * t + 4], mask4[:],
                                            e_col[:, t:t + 1])

            # ---- y2[b,:] = sum_k e * knnWc ; col C = sum_k e (den) ----
            y2_ps = psmi.tile([128, CP4], f32, tag="mi")
            for t in range(8):
                nc.tensor.matmul(y2_ps[:BROWS, :C + 1], lhsT=w2[t][:],
                                 rhs=kwS[:, t * CP4:t * CP4 + C + 1],
                                 start=(t == 0), stop=(t == 7))
            # ---- y1 = relu(q) @ Wc1 ----
            y1_ps = psmi.tile([128, CP4], f32, tag="mi")
            for dc in range(4):
                nc.tensor.matmul(y1_ps[:BROWS, :C], lhsT=qT[dc], rhs=Wc[dc],
                                 start=(dc == 0), stop=(dc == 3))

            osb = smallp.tile([2 * BROWS, CP4], f32, tag="osb")
            nc.scalar.copy(out=osb[BROWS:, :C + 1], in_=y2_ps[:BROWS, :C + 1])
            nc.scalar.copy(out=osb[:BROWS, :C], in_=y1_ps[:BROWS, :C])
            nc.sync.dma_start(out=out_d[:, :], in_=osb[:])
    nc.finalize()
    return nc


def _phase1_nc():
    global _PH1
    if _PH1 is None:
        _PH1 = _build_phase1()
    return _PH1


def _phase2_nc():
    global _PH2
    if _PH2 is None:
        _PH2 = _build_phase2()
    return _PH2


def kernel(query_feat, memory_keys, Wq, bq, Wm, bm, Ws, bs, Wc, bc):
    query_feat = np.asarray(query_feat, np.float32)
    memory_keys = np.asarray(memory_keys, np.float32)

    # ---- host prep: pad + normalize + transpose + shard keys (bf16) ----
    kn = np.sqrt((memory_keys ** 2).sum(axis=1))
    khat = memory_keys * (1.0 / kn)[:, None]
    pad = np.full((NPAD - N, D), -1.0 / np.sqrt(D), np.float32)
    khat_pad = np.concatenate([khat.astype(np.float32), pad], axis=0)
    q32 = np.maximum(query_feat, 0)
    qT_full = np.ascontiguousarray(q32.T.astype(BF))  # [512, 256] bf16

    ph1 = _phase1_nc()
    in_maps = []
    for c in range(NC_CORES):
        sh = khat_pad[c * SHARD:(c + 1) * SHARD]          # [12800, 512]
        arr = np.ascontiguousarray(
            sh.reshape(NCHUNK, CHUNK, 4, 128).transpose(0, 3, 2, 1).astype(BF)
        ).reshape(NCHUNK, 128, 4 * CHUNK)
        in_maps.append({"khatT": arr, "qT": qT_full})
    res1 = run_bass_kernel_spmd(ph1, in_maps, core_ids=list(range(NC_CORES)))

    # ---- host merge: recover indices, exact re-score of candidates ----
    all_gidx = np.zeros((B, NC_CORES, KLOC), np.int64)
    for c in range(NC_CORES):
        win = np.asarray(res1.results[c]["win"]).view(np.uint32)
        within = (win & np.uint32(0xFFFF)).astype(np.int64)  # in-shard index
        all_gidx[:, c, :] = within + c * SHARD
    gidx = all_gidx.reshape(B, CAND)
    safe = np.minimum(gidx, N - 1)
    cand_keys = memory_keys[safe]                       # [256, 256, 512]
    dots = np.einsum("bd,bcd->bc", q32, cand_keys, optimize=True)
    cos = dots / np.maximum(
        np.linalg.norm(q32, axis=1)[:, None] * kn[safe], np.float32(1e-8))
    cos[gidx >= N] = -np.inf                            # mask dummy-pad hits
    order = np.argsort(-cos, axis=1, kind="stable")[:, :K]
    top_idx = np.take_along_axis(safe, order, axis=1)   # [256, 32]

    # ---- phase 2 (batch sharded): pack one bf16 blob per core ----
    ph2 = _phase2_nc()
    bqm_f = (np.asarray(bq, np.float32) + np.asarray(bm, np.float32))
    Wm_b = np.asarray(Wm, np.float32).reshape(4, 128, A).transpose(1, 0, 2).reshape(128, 1024)
    Wq_b = np.asarray(Wq, np.float32).reshape(4, 128, A).transpose(1, 0, 2).reshape(128, 1024)
    Ws_b = np.asarray(Ws, np.float32)[:, 0].reshape(2, 128).T         # [128, 2]
    Wc_b = np.asarray(Wc, np.float32).reshape(8, 128, C).transpose(1, 0, 2).reshape(128, 800)
    bqm_u16 = np.ascontiguousarray(
        bqm_f.reshape(2, 128).T.astype(np.float32)).view(np.uint16)   # [128, 4]

    wpart = np.zeros((128, W_W), np.uint16)
    wpart[:, W_WM:W_WM + 1024] = Wm_b.astype(BF).view(np.uint16)
    wpart[:, W_WQ:W_WQ + 1024] = Wq_b.astype(BF).view(np.uint16)
    wpart[:, W_WS:W_WS + 2] = Ws_b.astype(BF).view(np.uint16)
    wpart[:, W_BQM:W_BQM + 4] = bqm_u16
    wpart[:, W_WC:W_WC + 800] = Wc_b.astype(BF).view(np.uint16)

    in_maps2 = []
    for c in range(NC_CORES):
        rows = slice(c * BROWS, (c + 1) * BROWS)
        knn_rows = memory_keys[top_idx[rows]].reshape(NCD, D)
        ktp = np.ascontiguousarray(
            knn_rows.reshape(2, 512, 4, 128).transpose(3, 0, 2, 1)
        ).reshape(128, KT_W).astype(BF).view(np.uint16)
        qTc = np.ascontiguousarray(
            q32[rows].T.reshape(4, 128, BROWS).transpose(1, 0, 2)
        ).reshape(128, 128).astype(BF).view(np.uint16)
        blob = np.zeros((128, BLOB_W), np.uint16)
        blob[:, :KT_W] = ktp
        blob[:, KT_W:] = wpart
        blob[:, KT_W + W_QT:KT_W + W_QT + 128] = qTc
        in_maps2.append({"blob": blob.view(BF)})
    res2 = run_bass_kernel_spmd(ph2, in_maps2, core_ids=list(range(NC_CORES)))

    out = np.zeros((B, C), np.float32)
    for c in range(NC_CORES):
        r = np.asarray(res2.results[c]["out"], np.float32)   # [64, 104]
        y1 = r[:BROWS, :C]
        y2 = r[BROWS:, :C]
        den = r[BROWS:, C]
        out[c * BROWS:(c + 1) * BROWS] = y1 + y2 / den[:, None]
    return (out + np.asarray(bc, np.float32)[None, :]).astype(np.float32)


# revision 15
# speedup vs baseline: 1.7843x; 1.0706x over previous
"""Trainium2 Bass kernel for retrieval-knn attention classifier (nn_MA_51866025067137).

Strategy (8 NeuronCores):
  Phase 1 — memory_keys sharded along N (12800 keys/core, padded 100000->102400
  with dummy rows), fed in bf16 (keys pre-normalized on host so the matmul
  directly yields cosine ranking values; host re-scores candidates in exact
  fp32 afterwards, so ranking precision only has to preserve the top-32 set).
  Per chunk of 512 keys: PE computes sims for all 256 queries (bf16 matmuls,
  fp32 PSUM), ACT evicts both query-halves in one [128,1024] copy, GPSIMD
  packs a 9-bit in-chunk index into the sim mantissa, DVE max8 keeps the
  top-8 per chunk.  Tail: 4 rounds of max8/max_index/match_replace extract
  the per-core top-32 (value, position) per query row.
  Host — merges the 8x32 candidates per row, re-scores them exactly in fp32,
  and gathers the global top-32 key vectors.
  Phase 2 — batch sharded (32 queries/core), all inputs packed into one bf16
  blob (2 logical DMAs): memory-attention module via bf16 matmuls; the
  softmax-score row is transposed with 8 tiny PE transposes (no DRAM bounce);
  attended@Wc is reassociated as sum_k w_k * (knn_k @ Wc2) so the weighted
  sum runs over a precomputed [1024,100] knnWc instead of [1024,512] knn
  (no knn tile, no attT transpose); normalization by sum(e) happens on host.
"""

import numpy as np
import ml_dtypes

import concourse.bacc as bacc
import concourse.mybir as mybir
from concourse.tile import TileContext
from concourse.bass_utils import run_bass_kernel_spmd

# problem dims (hardcoded per harness contract)
B, N, D = 256, 100000, 512
A, C, K = 256, 100, 32
NC_CORES = 8
NPAD = 102400             # 8 * 12800
SHARD = NPAD // NC_CORES  # 12800
CHUNK = 512               # keys per inner loop step
NCHUNK = SHARD // CHUNK   # 25
L1W = NCHUNK * 8          # 200
BROWS = B // NC_CORES     # 32 rows per core in phase 2
KLOC = 32                 # local candidates per core per row
NROUND = KLOC // 8        # 4 extraction rounds
CAND = NC_CORES * KLOC    # 256 merged candidates per row

f32 = mybir.dt.float32
f32r = mybir.dt.float32r
bf16 = mybir.dt.bfloat16
u32 = mybir.dt.uint32
BF = ml_dtypes.bfloat16

# ---- phase-2 blob layout (bf16 columns) ----
NCD = BROWS * K           # 1024
# ktile: [128, 4096], col = half*2048 + dc*512 + i  (bk = half*512 + i)
KT_W = 4096
# wtile: [128, 2984]
W_WM = 0                  # 4 dc x 256
W_WQ = 1024               # 4 dc x 256
W_WS = 2048               # 2 (col at)
W_BQM = 2050              # 4 bf16 cols = [128,2] f32 (byte offset 4100, 4-aligned)
W_QT = 2054               # 4 dc x 32
W_WC = 2182               # 8 m x 100
W_W = 2984                # padded (2982 used)
BLOB_W = KT_W + W_W       # 7080
CP4 = C + 4               # 104: knnWc stride (100 vals, col 100 = 1.0)

_PH1 = None
_PH2 = None


def _build_phase1():
    nc = bacc.Bacc("TRN2", target_bir_lowering=False)
    khatT = nc.dram_tensor("khatT", [NCHUNK, 128, 4 * CHUNK], bf16, kind="ExternalInput")
    qT = nc.dram_tensor("qT", [D, B], bf16, kind="ExternalInput")
    win_out = nc.dram_tensor("win", [B, KLOC], f32, kind="ExternalOutput")

    with TileContext(nc) as tc:
        with (
            tc.tile_pool(name="const", bufs=1) as constp,
            tc.tile_pool(name="qpool", bufs=1) as qpool,
            tc.tile_pool(name="keys", bufs=5) as keyp,
            tc.tile_pool(name="pk", bufs=4) as pkp,
            tc.tile_pool(name="l1", bufs=1) as l1p,
            tc.tile_pool(name="small", bufs=1) as smallp,
            tc.tile_pool(name="psum", bufs=3, space="PSUM") as psump,
            tc.tile_pool(name="pswarm", bufs=1, space="PSUM") as pswarm,
        ):
            # PE p-state warmup: keep the tensor engine queue non-empty from
            # ~1us until the first key chunk lands so the clock ramps to
            # 2.4GHz before real matmuls start.
            warm = constp.tile([128, CHUNK], bf16, tag="warm")
            nc.vector.memset(warm[:], 0.0)
            warm_ps = pswarm.tile([128, CHUNK], f32, tag="wps")
            for i in range(12):
                nc.tensor.matmul(warm_ps[:], lhsT=warm[:, :128], rhs=warm[:],
                                 start=True, stop=True)

            # qT already relu'd + bf16 on host; one DMA, dc-major [128, 4*256]
            qTall = qpool.tile([128, 4 * B], bf16, tag="qtall")
            nc.sync.dma_start(out=qTall[:].rearrange("p (dc b) -> p dc b", dc=4),
                              in_=qT[:, :].rearrange("(dc p) b -> p dc b", p=128))
            qTr = [qTall[:, dc * B:(dc + 1) * B] for dc in range(4)]

            L1 = [l1p.tile([128, L1W], f32, tag=f"l1_{qt}", name=f"l1_{qt}")
                  for qt in range(2)]

            for c in range(NCHUNK):
                kt = keyp.tile([128, 4 * CHUNK], bf16, tag="kt")
                nc.sync.dma_start(out=kt[:], in_=khatT[c, :, :])
                ps = psump.tile([128, 2 * CHUNK], f32, tag="sim")
                for qt in range(2):
                    for dc in range(4):
                        nc.tensor.matmul(
                            ps[:, qt * CHUNK:(qt + 1) * CHUNK],
                            lhsT=qTr[dc][:, qt * 128:(qt + 1) * 128],
                            rhs=kt[:, dc * CHUNK:(dc + 1) * CHUNK],
                            start=(dc == 0), stop=(dc == 3),
                        )
                # packed candidate word = bf16(sim) in the high u16 (ACT evict
                # with stride-2 bf16 output), in-shard index in the low u16
                # (GPSIMD iota into the even u16 lanes). Read as f32, words
                # rank by sim value with the index as tiebreaker.
                pk = pkp.tile([128, 2 * CHUNK], f32, tag="pk")
                pk16 = pk[:].bitcast(bf16)          # [128, 4096] 2-byte lanes
                nc.gpsimd.iota(
                    pk16.rearrange("p (b two) -> p b two", two=2)[:, :, 0]
                        .bitcast(mybir.dt.uint16)
                        .rearrange("p (a b) -> p a b", a=2),
                    pattern=[[0, 2], [1, CHUNK]], base=c * CHUNK,
                    channel_multiplier=0)
                nc.scalar.copy(
                    out=pk16.rearrange("p (b two) -> p b two", two=2)[:, :, 1],
                    in_=ps[:])
                for qt in range(2):
                    nc.vector.max(out=L1[qt][:, c * 8:(c + 1) * 8],
                                  in_=pk[:, qt * CHUNK:(qt + 1) * CHUNK])

            # extraction: NROUND rounds of top-8 from L1 (200 wide); winners
            # carry their in-shard index in the low 14 bits -> no max_index
            for qt in range(2):
                win = smallp.tile([128, KLOC], f32, tag=f"win{qt}")
                for r in range(NROUND):
                    w8 = win[:, r * 8:(r + 1) * 8]
                    nc.vector.max(out=w8, in_=L1[qt][:])
                    if r < NROUND - 1:
                        nc.vector.match_replace(out=L1[qt][:], in_to_replace=w8,
                                                in_values=L1[qt][:],
                                                imm_value=-3.0e38)
                nc.sync.dma_start(out=win_out[qt * 128:(qt + 1) * 128, :], in_=win[:])
    nc.finalize()
    return nc


def _build_phase2():
    nc = bacc.Bacc("TRN2", target_bir_lowering=False)
    blob = nc.dram_tensor("blob", [128, BLOB_W], bf16, kind="ExternalInput")
    out_d = nc.dram_tensor("out", [2 * BROWS, CP4], f32, kind="ExternalOutput")

    with TileContext(nc) as tc:
        with (
            tc.tile_pool(name="big", bufs=1) as bigp,
            tc.tile_pool(name="small", bufs=1) as smallp,
            tc.tile_pool(name="pskp", bufs=3, space="PSUM") as pskp,
            tc.tile_pool(name="pssc", bufs=1, space="PSUM") as pssc,
            tc.tile_pool(name="psmi", bufs=2, space="PSUM") as psmi,
            tc.tile_pool(name="psy", bufs=1, space="PSUM") as psy,
        ):
            # ---- DMAs: critical weights, then knnT quarters, then Wc ----
            wt = bigp.tile([128, W_W], bf16, tag="wt")
            nc.sync.dma_start(out=wt[:, :W_WC], in_=blob[:, KT_W:KT_W + W_WC])
            kt = bigp.tile([128, KT_W], bf16, tag="ktile")
            for q in range(4):
                nc.sync.dma_start(out=kt[:, q * 1024:(q + 1) * 1024],
                                  in_=blob[:, q * 1024:(q + 1) * 1024])
            nc.sync.dma_start(out=wt[:, W_WC:W_WC + 800],
                              in_=blob[:, KT_W + W_WC:KT_W + W_WC + 800])

            Wm = [wt[:, W_WM + dc * A:W_WM + (dc + 1) * A] for dc in range(4)]
            Wq = [wt[:, W_WQ + dc * A:W_WQ + (dc + 1) * A] for dc in range(4)]
            Ws = [wt[:, W_WS + at:W_WS + at + 1] for at in range(2)]
            bqm = wt[:, W_BQM:W_BQM + 4].bitcast(f32)            # [128, 2]
            qT = [wt[:, W_QT + dc * BROWS:W_QT + (dc + 1) * BROWS] for dc in range(4)]
            Wc = [wt[:, W_WC + m * C:W_WC + (m + 1) * C] for m in range(8)]

            # ---- PE p-state warmup (bridges the DMA lead-in) ----
            warm = smallp.tile([128, 512], bf16, tag="warm")
            nc.vector.memset(warm[:], 0.0)
            warm_ps = psmi.tile([128, CP4], f32, tag="mi")
            for i in range(12):
                nc.tensor.matmul(warm_ps[:, :C], lhsT=warm[:, :128],
                                 rhs=warm[:, :C], start=True, stop=True)

            # ---- small constants ----
            ident1 = smallp.tile([1, 1], f32, tag="id1")
            nc.vector.memset(ident1[:], 1.0)
            # mask4[p, j] = 1.0 iff j == p // 32
            mask4 = smallp.tile([128, 4], f32, tag="mask4")
            nc.vector.memset(mask4[:], 1.0)
            nc.gpsimd.affine_select(out=mask4[:], in_=mask4[:],
                                    compare_op=mybir.AluOpType.is_ge, fill=0.0,
                                    base=0, pattern=[[-32, 4]], channel_multiplier=1)
            nc.gpsimd.affine_select(out=mask4[:], in_=mask4[:],
                                    compare_op=mybir.AluOpType.is_ge, fill=0.0,
                                    base=31, pattern=[[32, 4]], channel_multiplier=-1)
            w2 = [smallp.tile([128, BROWS], bf16, tag=f"w2_{t}", name=f"w2t{t}")
                  for t in range(8)]
            for t in range(8):
                nc.vector.memset(w2[t][:], 0.0)
            kwS = smallp.tile([128, 8 * CP4], bf16, tag="kwS")   # knn@Wc2 (+ones col)
            for t in range(8):
                nc.vector.memset(kwS[:, t * CP4 + C:t * CP4 + C + 1], 1.0)

            # ---- qprojT [128(a), 2*32] ----
            qp_ps = psmi.tile([128, CP4], f32, tag="mi")
            for at in range(2):
                for dc in range(4):
                    nc.tensor.matmul(
                        qp_ps[:, at * BROWS:(at + 1) * BROWS],
                        lhsT=Wq[dc][:, at * 128:(at + 1) * 128], rhs=qT[dc],
                        start=(dc == 0), stop=(dc == 3))
            qprojT = smallp.tile([128, 2 * BROWS], f32, tag="qprojT")
            nc.scalar.copy(out=qprojT[:], in_=qp_ps[:, :2 * BROWS])

            # ---- y1 = relu(q) @ Wc1, shipped early (partitions 32..63) ----
            yy_ps = psy.tile([128, CP4], f32, tag="yy")
            for dc in range(4):
                nc.tensor.matmul(yy_ps[BROWS:2 * BROWS, :C], lhsT=qT[dc],
                                 rhs=Wc[dc], start=(dc == 0), stop=(dc == 3))
            osb = smallp.tile([2 * BROWS, CP4], f32, tag="osb")
            nc.scalar.copy(out=osb[:BROWS, :C], in_=yy_ps[BROWS:2 * BROWS, :C])
            nc.sync.dma_start(out=out_d[:BROWS, :], in_=osb[:BROWS, :])

            # ---- kprojT + h = tanh(. + qproj + bqm) -> scores row ----
            sc_ps = pssc.tile([128, NCD], f32, tag="sc")
            hT = [bigp.tile([128, NCD], bf16, tag=f"hT{at}", name=f"hTt{at}")
                  for at in range(2)]
            kph = {}
            for at in range(2):
                for half in range(2):
                    kp = pskp.tile([128, 512], f32, tag="kp")
                    kph[at, half] = kp
                    for dc in range(4):
                        nc.tensor.matmul(
                            kp[:],
                            lhsT=Wm[dc][:, at * 128:(at + 1) * 128],
                            rhs=kt[:, half * 2048 + dc * 512:half * 2048 + (dc + 1) * 512],
                            start=(dc == 0), stop=(dc == 3))
            for at in range(2):
                for half in range(2):
                    cols = slice(half * 512, (half + 1) * 512)
                    qb = qprojT[:, at * BROWS + half * 16:at * BROWS + half * 16 + 16,
                                None].to_broadcast([128, 16, K])
                    nc.vector.tensor_tensor(
                        hT[at][:, cols].rearrange("p (b k) -> p b k", k=K),
                        kph[at, half][:].rearrange("p (b k) -> p b k", k=K),
                        qb, mybir.AluOpType.add)
                    nc.scalar.activation(hT[at][:, cols], hT[at][:, cols],
                                         mybir.ActivationFunctionType.Tanh,
                                         bias=bqm[:, at:at + 1])
            for at in range(2):
                for half in range(2):
                    nc.tensor.matmul(
                        sc_ps[:1, half * 512:(half + 1) * 512],
                        lhsT=Ws[at],
                        rhs=hT[at][:, half * 512:(half + 1) * 512],
                        start=(at == 0), stop=(at == 1))

            # ---- knnWc[t] = knn_block_t @ Wc2 (overlaps scores tail) ----
            for t in range(8):
                kw_ps = psmi.tile([128, CP4], f32, tag="mi")
                half, blk = t // 4, t % 4
                for dc in range(4):
                    nc.tensor.matmul(
                        kw_ps[:, :C],
                        lhsT=kt[:, half * 2048 + dc * 512 + blk * 128:
                                half * 2048 + dc * 512 + (blk + 1) * 128],
                        rhs=Wc[4 + dc],
                        start=(dc == 0), stop=(dc == 3))
                nc.vector.tensor_copy(kwS[:, t * CP4:t * CP4 + C], kw_ps[:, :C])

            # ---- e row (exp per half) -> e_col via PE transposes; weights ----
            e_row = smallp.tile([1, NCD], f32, tag="e_row")
            ecT_ps = psmi.tile([128, CP4], f32, tag="mi")
            for half in range(2):
                nc.scalar.activation(e_row[:, half * 512:(half + 1) * 512],
                                     sc_ps[:1, half * 512:(half + 1) * 512],
                                     mybir.ActivationFunctionType.Exp)
                for tt in range(4):
                    t = half * 4 + tt
                    nc.tensor.transpose(ecT_ps[:, t:t + 1],
                                        e_row[:, t * 128:(t + 1) * 128], ident1[:])
                for tt in range(4):
                    t = half * 4 + tt
                    nc.vector.tensor_scalar_mul(w2[t][:, 4 * t:4 * t + 4],
                                                mask4[:], ecT_ps[:, t:t + 1])

            # ---- y2[b,:] = sum_k e * knnWc ; col C = sum_k e (den) ----
            for t in range(8):
                nc.tensor.matmul(yy_ps[:BROWS, :C + 1], lhsT=w2[t][:],
                                 rhs=kwS[:, t * CP4:t * CP4 + C + 1],
                                 start=(t == 0), stop=(t == 7))
            nc.scalar.copy(out=osb[BROWS:, :C + 1], in_=yy_ps[:BROWS, :C + 1])
            nc.sync.dma_start(out=out_d[BROWS:, :], in_=osb[BROWS:, :])
    nc.finalize()
    return nc


def _phase1_nc():
    global _PH1
    if _PH1 is None:
        _PH1 = _build_phase1()
    return _PH1


def _phase2_nc():
    global _PH2
    if _PH2 is None:
        _PH2 = _build_phase2()
    return _PH2


def kernel(query_feat, memory_keys, Wq, bq, Wm, bm, Ws, bs, Wc, bc):
    query_feat = np.asarray(query_feat, np.float32)
    memory_keys = np.asarray(memory_keys, np.float32)

    # ---- host prep: pad + normalize + transpose + shard keys (bf16) ----
    kn = np.sqrt((memory_keys ** 2).sum(axis=1))
    khat = memory_keys * (1.0 / kn)[:, None]
    pad = np.full((NPAD - N, D), -1.0 / np.sqrt(D), np.float32)
    khat_pad = np.concatenate([khat.astype(np.float32), pad], axis=0)
    q32 = np.maximum(query_feat, 0)
    qT_full = np.ascontiguousarray(q32.T.astype(BF))  # [512, 256] bf16

    ph1 = _phase1_nc()
    in_maps = []
    for c in range(NC_CORES):
        sh = khat_pad[c * SHARD:(c + 1) * SHARD]          # [12800, 512]
        arr = np.ascontiguousarray(
            sh.reshape(NCHUNK, CHUNK, 4, 128).transpose(0, 3, 2, 1).astype(BF)
        ).reshape(NCHUNK, 128, 4 * CHUNK)
        in_maps.append({"khatT": arr, "qT": qT_full})
    res1 = run_bass_kernel_spmd(ph1, in_maps, core_ids=list(range(NC_CORES)))

    # ---- host merge: recover indices, exact re-score of candidates ----
    all_gidx = np.zeros((B, NC_CORES, KLOC), np.int64)
    for c in range(NC_CORES):
        win = np.asarray(res1.results[c]["win"]).view(np.uint32)
        within = (win & np.uint32(0xFFFF)).astype(np.int64)  # in-shard index
        all_gidx[:, c, :] = within + c * SHARD
    gidx = all_gidx.reshape(B, CAND)
    safe = np.minimum(gidx, N - 1)
    cand_keys = memory_keys[safe]                       # [256, 256, 512]
    dots = np.einsum("bd,bcd->bc", q32, cand_keys, optimize=True)
    cos = dots / np.maximum(
        np.linalg.norm(q32, axis=1)[:, None] * kn[safe], np.float32(1e-8))
    cos[gidx >= N] = -np.inf                            # mask dummy-pad hits
    order = np.argsort(-cos, axis=1, kind="stable")[:, :K]
    top_idx = np.take_along_axis(safe, order, axis=1)   # [256, 32]

    # ---- phase 2 (batch sharded): pack one bf16 blob per core ----
    ph2 = _phase2_nc()
    bqm_f = (np.asarray(bq, np.float32) + np.asarray(bm, np.float32))
    Wm_b = np.asarray(Wm, np.float32).reshape(4, 128, A).transpose(1, 0, 2).reshape(128, 1024)
    Wq_b = np.asarray(Wq, np.float32).reshape(4, 128, A).transpose(1, 0, 2).reshape(128, 1024)
    Ws_b = np.asarray(Ws, np.float32)[:, 0].reshape(2, 128).T         # [128, 2]
    Wc_b = np.asarray(Wc, np.float32).reshape(8, 128, C).transpose(1, 0, 2).reshape(128, 800)
    bqm_u16 = np.ascontiguousarray(
        bqm_f.reshape(2, 128).T.astype(np.float32)).view(np.uint16)   # [128, 4]

    wpart = np.zeros((128, W_W), np.uint16)
    wpart[:, W_WM:W_WM + 1024] = Wm_b.astype(BF).view(np.uint16)
    wpart[:, W_WQ:W_WQ + 1024] = Wq_b.astype(BF).view(np.uint16)
    wpart[:, W_WS:W_WS + 2] = Ws_b.astype(BF).view(np.uint16)
    wpart[:, W_BQM:W_BQM + 4] = bqm_u16
    wpart[:, W_WC:W_WC + 800] = Wc_b.astype(BF).view(np.uint16)

    in_maps2 = []
    for c in range(NC_CORES):
        rows = slice(c * BROWS, (c + 1) * BROWS)
        knn_rows = memory_keys[top_idx[rows]].reshape(NCD, D)
        ktp = np.ascontiguousarray(
            knn_rows.reshape(2, 512, 4, 128).transpose(3, 0, 2, 1)
        ).reshape(128, KT_W).astype(BF).view(np.uint16)
        qTc = np.ascontiguousarray(
            q32[rows].T.reshape(4, 128, BROWS).transpose(1, 0, 2)
        ).reshape(128, 128).astype(BF).view(np.uint16)
        blob = np.zeros((128, BLOB_W), np.uint16)
        blob[:, :KT_W] = ktp
        blob[:, KT_W:] = wpart
        blob[:, KT_W + W_QT:KT_W + W_QT + 128] = qTc
        in_maps2.append({"blob": blob.view(BF)})
    res2 = run_bass_kernel_spmd(ph2, in_maps2, core_ids=list(range(NC_CORES)))

    out = np.zeros((B, C), np.float32)
    for c in range(NC_CORES):
        r = np.asarray(res2.results[c]["out"], np.float32)   # [64, 104]
        y1 = r[:BROWS, :C]
        y2 = r[BROWS:, :C]
        den = r[BROWS:, C]
        out[c * BROWS:(c + 1) * BROWS] = y1 + y2 / den[:, None]
    return (out + np.asarray(bc, np.float32)[None, :]).astype(np.float32)


# revision 19
# speedup vs baseline: 2.0305x; 1.1380x over previous
"""Trainium2 Bass kernel for retrieval-knn attention classifier (nn_MA_51866025067137).

Strategy (8 NeuronCores):
  Phase 1 — memory_keys sharded along N (12800 keys/core, padded 100000->102400
  with dummy rows), fed in bf16 (keys pre-normalized on host so the matmul
  directly yields cosine ranking values; host re-scores candidates in exact
  fp32 afterwards, so ranking precision only has to preserve the top-32 set).
  Per chunk of 512 keys: PE computes sims for all 256 queries (bf16 matmuls,
  fp32 PSUM), ACT evicts both query-halves in one [128,1024] copy, GPSIMD
  packs a 9-bit in-chunk index into the sim mantissa, DVE max8 keeps the
  top-8 per chunk.  Tail: 4 rounds of max8/max_index/match_replace extract
  the per-core top-32 (value, position) per query row.
  Host — merges the 8x32 candidates per row, re-scores them exactly in fp32,
  and gathers the global top-32 key vectors.
  Phase 2 — batch sharded (32 queries/core), all inputs packed into one bf16
  blob (2 logical DMAs): memory-attention module via bf16 matmuls; the
  softmax-score row is transposed with 8 tiny PE transposes (no DRAM bounce);
  attended@Wc is reassociated as sum_k w_k * (knn_k @ Wc2) so the weighted
  sum runs over a precomputed [1024,100] knnWc instead of [1024,512] knn
  (no knn tile, no attT transpose); normalization by sum(e) happens on host.
"""

import numpy as np
import ml_dtypes

import concourse.bacc as bacc
import concourse.mybir as mybir
from concourse.tile import TileContext
from concourse.bass_utils import run_bass_kernel_spmd

# problem dims (hardcoded per harness contract)
B, N, D = 256, 100000, 512
A, C, K = 256, 100, 32
NC_CORES = 8
NPAD = 102400             # 8 * 12800
SHARD = NPAD // NC_CORES  # 12800
CHUNK = 512               # keys per inner loop step
NCHUNK = SHARD // CHUNK   # 25
L1W = NCHUNK * 8          # 200
BROWS = B // NC_CORES     # 32 rows per core in phase 2
KLOC = 40                 # local candidates per core per row
NROUND = KLOC // 8        # 5 extraction rounds
CAND = NC_CORES * KLOC    # 320 merged candidates per row
KSCALE = 16.0             # fp8 range scaling (ranking is scale-invariant)
QSCALE = 32.0

f32 = mybir.dt.float32
f32r = mybir.dt.float32r
bf16 = mybir.dt.bfloat16
fp8 = mybir.dt.float8e4
u32 = mybir.dt.uint32
u16 = mybir.dt.uint16
BF = ml_dtypes.bfloat16
E4 = ml_dtypes.float8_e4m3

# ---- phase-2 blob layout (bf16 columns) ----
NCD = BROWS * K           # 1024
# ktile: [128, 4096], col = half*2048 + dc*512 + i  (bk = half*512 + i)
KT_W = 4096
# wtile: [128, 2984]
W_WM = 0                  # 4 dc x 256
W_WQ = 1024               # 4 dc x 256
W_WS = 2048               # 2 (col at)
W_BQM = 2050              # 4 bf16 cols = [128,2] f32 (byte offset 4100, 4-aligned)
W_QT = 2054               # 4 dc x 32
W_WC = 2182               # 8 m x 100
W_W = 2984                # padded (2982 used)
BLOB_W = KT_W + W_W       # 7080
CP4 = C + 4               # 104: knnWc stride (100 vals, col 100 = 1.0)

_PH1 = None
_PH2 = None


def _build_phase1():
    nc = bacc.Bacc("TRN2", target_bir_lowering=False)
    khatT = nc.dram_tensor("khatT", [NCHUNK, 128, 4 * CHUNK], fp8, kind="ExternalInput")
    qT = nc.dram_tensor("qT", [D, B], fp8, kind="ExternalInput")
    win_out = nc.dram_tensor("win", [B, KLOC], f32, kind="ExternalOutput")
    pos_out = nc.dram_tensor("pos", [B, KLOC], u32, kind="ExternalOutput")

    with TileContext(nc) as tc:
        with (
            tc.tile_pool(name="const", bufs=1) as constp,
            tc.tile_pool(name="qpool", bufs=1) as qpool,
            tc.tile_pool(name="keys", bufs=5) as keyp,
            tc.tile_pool(name="pk", bufs=4) as pkp,
            tc.tile_pool(name="l1", bufs=1) as l1p,
            tc.tile_pool(name="small", bufs=1) as smallp,
            tc.tile_pool(name="psum", bufs=3, space="PSUM") as psump,
            tc.tile_pool(name="pswarm", bufs=1, space="PSUM") as pswarm,
        ):
            # PE p-state warmup: keep the tensor engine queue non-empty from
            # ~1us until the first key chunk lands so the clock ramps to
            # 2.4GHz before real matmuls start.
            warm = constp.tile([128, CHUNK], bf16, tag="warm")
            nc.vector.memset(warm[:], 0.0)
            warm_ps = pswarm.tile([128, CHUNK], f32, tag="wps")
            for i in range(12):
                nc.tensor.matmul(warm_ps[:], lhsT=warm[:, :128], rhs=warm[:],
                                 start=True, stop=True)

            # qT already relu'd, scaled, fp8 on host; one DMA, dc-major
            qTall = qpool.tile([128, 4 * B], fp8, tag="qtall")
            nc.sync.dma_start(out=qTall[:].rearrange("p (dc b) -> p dc b", dc=4),
                              in_=qT[:, :].rearrange("(dc p) b -> p dc b", p=128))
            qT3 = qTall[:].rearrange("p (dc b) -> p dc b", dc=4)  # [128,4,256]

            # static index lanes: each pk buffer's low u16 lanes hold the
            # in-chunk key index (0..511, repeated for both query halves);
            # written once by GPSIMD, reused as buffers rotate. The chunk id
            # is recovered from the winner's L1 position via max_index.
            pks = []
            for b in range(4):
                pk = pkp.tile([128, 2 * CHUNK], f32, tag="pk")
                pks.append(pk)
                nc.gpsimd.iota(
                    pk[:].bitcast(u16)
                        .rearrange("p (b two) -> p b two", two=2)[:, :, 0]
                        .rearrange("p (a b) -> p a b", a=2),
                    pattern=[[0, 2], [1, CHUNK]], base=0,
                    channel_multiplier=0)

            L1 = [l1p.tile([128, L1W], f32, tag=f"l1_{qt}", name=f"l1_{qt}")
                  for qt in range(2)]

            for c in range(NCHUNK):
                kt = keyp.tile([128, 4 * CHUNK], fp8, tag="kt")
                nc.sync.dma_start(out=kt[:], in_=khatT[c, :, :])
                kt3 = kt[:].rearrange("p (dc n) -> p dc n", dc=4)
                ps = psump.tile([128, 2 * CHUNK], f32, tag="sim")
                for qt in range(2):
                    for m in range(2):
                        nc.tensor.matmul(
                            ps[:, qt * CHUNK:(qt + 1) * CHUNK],
                            lhsT=qT3[:, 2 * m:2 * m + 2, qt * 128:(qt + 1) * 128],
                            rhs=kt3[:, 2 * m:2 * m + 2, :],
                            perf_mode=mybir.MatmulPerfMode.DoubleRow,
                            start=(m == 0), stop=(m == 1),
                        )
                # bf16(sim) into the high u16 lanes over the static index lanes
                pk = pks[c % 4]
                nc.scalar.copy(
                    out=pk[:].bitcast(bf16)
                        .rearrange("p (b two) -> p b two", two=2)[:, :, 1],
                    in_=ps[:])
                for qt in range(2):
                    nc.vector.max(out=L1[qt][:, c * 8:(c + 1) * 8],
                                  in_=pk[:, qt * CHUNK:(qt + 1) * CHUNK])

            # extraction: NROUND rounds of top-8 from L1 (200 wide)
            for qt in range(2):
                win = smallp.tile([128, KLOC], f32, tag=f"win{qt}")
                pos = smallp.tile([128, KLOC], u32, tag=f"pos{qt}")
                for r in range(NROUND):
                    w8 = win[:, r * 8:(r + 1) * 8]
                    nc.vector.max(out=w8, in_=L1[qt][:])
                    nc.vector.max_index(out=pos[:, r * 8:(r + 1) * 8],
                                        in_max=w8, in_values=L1[qt][:])
                    if r < NROUND - 1:
                        nc.vector.match_replace(out=L1[qt][:], in_to_replace=w8,
                                                in_values=L1[qt][:],
                                                imm_value=-3.0e38)
                nc.sync.dma_start(out=win_out[qt * 128:(qt + 1) * 128, :], in_=win[:])
                nc.sync.dma_start(out=pos_out[qt * 128:(qt + 1) * 128, :], in_=pos[:])
    nc.finalize()
    return nc


def _build_phase2():
    nc = bacc.Bacc("TRN2", target_bir_lowering=False)
    blob = nc.dram_tensor("blob", [128, BLOB_W], bf16, kind="ExternalInput")
    out_d = nc.dram_tensor("out", [2 * BROWS, CP4], f32, kind="ExternalOutput")

    with TileContext(nc) as tc:
        with (
            tc.tile_pool(name="big", bufs=1) as bigp,
            tc.tile_pool(name="small", bufs=1) as smallp,
            tc.tile_pool(name="pskp", bufs=3, space="PSUM") as pskp,
            tc.tile_pool(name="pssc", bufs=1, space="PSUM") as pssc,
            tc.tile_pool(name="psmi", bufs=2, space="PSUM") as psmi,
            tc.tile_pool(name="psy", bufs=1, space="PSUM") as psy,
        ):
            # ---- DMAs: critical weights, then knnT quarters, then Wc ----
            wt = bigp.tile([128, W_W], bf16, tag="wt")
            nc.sync.dma_start(out=wt[:, :W_WC], in_=blob[:, KT_W:KT_W + W_WC])
            kt = bigp.tile([128, KT_W], bf16, tag="ktile")
            for q in range(4):
                nc.sync.dma_start(out=kt[:, q * 1024:(q + 1) * 1024],
                                  in_=blob[:, q * 1024:(q + 1) * 1024])
            nc.sync.dma_start(out=wt[:, W_WC:W_WC + 800],
                              in_=blob[:, KT_W + W_WC:KT_W + W_WC + 800])

            Wm = [wt[:, W_WM + dc * A:W_WM + (dc + 1) * A] for dc in range(4)]
            Wq = [wt[:, W_WQ + dc * A:W_WQ + (dc + 1) * A] for dc in range(4)]
            Ws = [wt[:, W_WS + at:W_WS + at + 1] for at in range(2)]
            bqm = wt[:, W_BQM:W_BQM + 4].bitcast(f32)            # [128, 2]
            qT = [wt[:, W_QT + dc * BROWS:W_QT + (dc + 1) * BROWS] for dc in range(4)]
            Wc = [wt[:, W_WC + m * C:W_WC + (m + 1) * C] for m in range(8)]

            # ---- PE p-state warmup (bridges the DMA lead-in) ----
            warm = smallp.tile([128, 512], bf16, tag="warm")
            nc.vector.memset(warm[:], 0.0)
            warm_ps = psmi.tile([128, CP4], f32, tag="mi")
            for i in range(12):
                nc.tensor.matmul(warm_ps[:, :C], lhsT=warm[:, :128],
                                 rhs=warm[:, :C], start=True, stop=True)

            # ---- small constants ----
            ident1 = smallp.tile([1, 1], f32, tag="id1")
            nc.vector.memset(ident1[:], 1.0)
            # mask4[p, j] = 1.0 iff j == p // 32
            mask4 = smallp.tile([128, 4], f32, tag="mask4")
            nc.vector.memset(mask4[:], 1.0)
            nc.gpsimd.affine_select(out=mask4[:], in_=mask4[:],
                                    compare_op=mybir.AluOpType.is_ge, fill=0.0,
                                    base=0, pattern=[[-32, 4]], channel_multiplier=1)
            nc.gpsimd.affine_select(out=mask4[:], in_=mask4[:],
                                    compare_op=mybir.AluOpType.is_ge, fill=0.0,
                                    base=31, pattern=[[32, 4]], channel_multiplier=-1)
            w2 = [smallp.tile([128, BROWS], bf16, tag=f"w2_{t}", name=f"w2t{t}")
                  for t in range(8)]
            for t in range(8):
                nc.vector.memset(w2[t][:], 0.0)
            kwS = smallp.tile([128, 8 * CP4], bf16, tag="kwS")   # knn@Wc2 (+ones col)
            for t in range(8):
                nc.vector.memset(kwS[:, t * CP4 + C:t * CP4 + C + 1], 1.0)

            # ---- qprojT [128(a), 2*32] ----
            qp_ps = psmi.tile([128, CP4], f32, tag="mi")
            for at in range(2):
                for dc in range(4):
                    nc.tensor.matmul(
                        qp_ps[:, at * BROWS:(at + 1) * BROWS],
                        lhsT=Wq[dc][:, at * 128:(at + 1) * 128], rhs=qT[dc],
                        start=(dc == 0), stop=(dc == 3))
            qprojT = smallp.tile([128, 2 * BROWS], f32, tag="qprojT")
            nc.scalar.copy(out=qprojT[:], in_=qp_ps[:, :2 * BROWS])

            # ---- y1 = relu(q) @ Wc1, shipped early (partitions 32..63) ----
            yy_ps = psy.tile([128, CP4], f32, tag="yy")
            for dc in range(4):
                nc.tensor.matmul(yy_ps[BROWS:2 * BROWS, :C], lhsT=qT[dc],
                                 rhs=Wc[dc], start=(dc == 0), stop=(dc == 3))
            osb = smallp.tile([2 * BROWS, CP4], f32, tag="osb")
            nc.scalar.copy(out=osb[:BROWS, :C], in_=yy_ps[BROWS:2 * BROWS, :C])
            nc.sync.dma_start(out=out_d[:BROWS, :], in_=osb[:BROWS, :])

            # ---- kprojT + h = tanh(. + qproj + bqm) -> scores row ----
            sc_ps = pssc.tile([128, NCD], f32, tag="sc")
            hT = [bigp.tile([128, NCD], bf16, tag=f"hT{at}", name=f"hTt{at}")
                  for at in range(2)]
            kph = {}
            for at in range(2):
                for half in range(2):
                    kp = pskp.tile([128, 512], f32, tag="kp")
                    kph[at, half] = kp
                    for dc in range(4):
                        nc.tensor.matmul(
                            kp[:],
                            lhsT=Wm[dc][:, at * 128:(at + 1) * 128],
                            rhs=kt[:, half * 2048 + dc * 512:half * 2048 + (dc + 1) * 512],
                            start=(dc == 0), stop=(dc == 3))
            for at in range(2):
                for half in range(2):
                    cols = slice(half * 512, (half + 1) * 512)
                    qb = qprojT[:, at * BROWS + half * 16:at * BROWS + half * 16 + 16,
                                None].to_broadcast([128, 16, K])
                    nc.vector.tensor_tensor(
                        hT[at][:, cols].rearrange("p (b k) -> p b k", k=K),
                        kph[at, half][:].rearrange("p (b k) -> p b k", k=K),
                        qb, mybir.AluOpType.add)
                    nc.scalar.activation(hT[at][:, cols], hT[at][:, cols],
                                         mybir.ActivationFunctionType.Tanh,
                                         bias=bqm[:, at:at + 1])
            for at in range(2):
                for half in range(2):
                    nc.tensor.matmul(
                        sc_ps[:1, half * 512:(half + 1) * 512],
                        lhsT=Ws[at],
                        rhs=hT[at][:, half * 512:(half + 1) * 512],
                        start=(at == 0), stop=(at == 1))

            # ---- knnWc[t] = knn_block_t @ Wc2 (overlaps scores tail) ----
            for t in range(8):
                kw_ps = psmi.tile([128, CP4], f32, tag="mi")
                half, blk = t // 4, t % 4
                for dc in range(4):
                    nc.tensor.matmul(
                        kw_ps[:, :C],
                        lhsT=kt[:, half * 2048 + dc * 512 + blk * 128:
                                half * 2048 + dc * 512 + (blk + 1) * 128],
                        rhs=Wc[4 + dc],
                        start=(dc == 0), stop=(dc == 3))
                nc.vector.tensor_copy(kwS[:, t * CP4:t * CP4 + C], kw_ps[:, :C])

            # ---- e row (exp per half) -> e_col via PE transposes; weights ----
            e_row = smallp.tile([1, NCD], f32, tag="e_row")
            ecT_ps = psmi.tile([128, CP4], f32, tag="mi")
            for half in range(2):
                nc.scalar.activation(e_row[:, half * 512:(half + 1) * 512],
                                     sc_ps[:1, half * 512:(half + 1) * 512],
                                     mybir.ActivationFunctionType.Exp)
                for tt in range(4):
                    t = half * 4 + tt
                    nc.tensor.transpose(ecT_ps[:, t:t + 1],
                                        e_row[:, t * 128:(t + 1) * 128], ident1[:])
                for tt in range(4):
                    t = half * 4 + tt
                    nc.vector.tensor_scalar_mul(w2[t][:, 4 * t:4 * t + 4],
                                                mask4[:], ecT_ps[:, t:t + 1])

            # ---- y2[b,:] = sum_k e * knnWc ; col C = sum_k e (den) ----
            for t in range(8):
                nc.tensor.matmul(yy_ps[:BROWS, :C + 1], lhsT=w2[t][:],
                                 rhs=kwS[:, t * CP4:t * CP4 + C + 1],
                                 start=(t == 0), stop=(t == 7))
            nc.scalar.copy(out=osb[BROWS:, :C + 1], in_=yy_ps[:BROWS, :C + 1])
            nc.sync.dma_start(out=out_d[BROWS:, :], in_=osb[BROWS:, :])
    nc.finalize()
    return nc


def _phase1_nc():
    global _PH1
    if _PH1 is None:
        _PH1 = _build_phase1()
    return _PH1


def _phase2_nc():
    global _PH2
    if _PH2 is None:
        _PH2 = _build_phase2()
    return _PH2


def kernel(query_feat, memory_keys, Wq, bq, Wm, bm, Ws, bs, Wc, bc):
    query_feat = np.asarray(query_feat, np.float32)
    memory_keys = np.asarray(memory_keys, np.float32)

    # ---- host prep: pad + normalize + transpose + shard keys (bf16) ----
    kn = np.sqrt((memory_keys ** 2).sum(axis=1))
    khat = memory_keys * (KSCALE / kn)[:, None]
    pad = np.full((NPAD - N, D), -KSCALE / np.sqrt(D), np.float32)
    khat_pad = np.concatenate([khat.astype(np.float32), pad], axis=0)
    q32 = np.maximum(query_feat, 0)
    qT_full = np.ascontiguousarray((q32.T * QSCALE).astype(E4))  # [512, 256]

    ph1 = _phase1_nc()
    in_maps = []
    for c in range(NC_CORES):
        sh = khat_pad[c * SHARD:(c + 1) * SHARD]          # [12800, 512]
        arr = np.ascontiguousarray(
            sh.reshape(NCHUNK, CHUNK, 4, 128).transpose(0, 3, 2, 1).astype(E4)
        ).reshape(NCHUNK, 128, 4 * CHUNK)
        in_maps.append({"khatT": arr, "qT": qT_full})
    res1 = run_bass_kernel_spmd(ph1, in_maps, core_ids=list(range(NC_CORES)))

    # ---- host merge: recover indices, exact re-score of candidates ----
    all_gidx = np.zeros((B, NC_CORES, KLOC), np.int64)
    for c in range(NC_CORES):
        win = np.asarray(res1.results[c]["win"]).view(np.uint32)
        pos = np.asarray(res1.results[c]["pos"]).astype(np.int64)  # 0..199
        within = (win & np.uint32(0xFFFF)).astype(np.int64)  # in-chunk index
        all_gidx[:, c, :] = (pos // 8) * CHUNK + within + c * SHARD
    gidx = all_gidx.reshape(B, CAND)
    safe = np.minimum(gidx, N - 1)
    cand_keys = memory_keys[safe]                       # [256, 256, 512]
    dots = np.einsum("bd,bcd->bc", q32, cand_keys, optimize=True)
    cos = dots / np.maximum(
        np.linalg.norm(q32, axis=1)[:, None] * kn[safe], np.float32(1e-8))
    cos[gidx >= N] = -np.inf                            # mask dummy-pad hits
    order = np.argsort(-cos, axis=1, kind="stable")[:, :K]
    top_idx = np.take_along_axis(safe, order, axis=1)   # [256, 32]

    # ---- phase 2 (batch sharded): pack one bf16 blob per core ----
    ph2 = _phase2_nc()
    bqm_f = (np.asarray(bq, np.float32) + np.asarray(bm, np.float32))
    Wm_b = np.asarray(Wm, np.float32).reshape(4, 128, A).transpose(1, 0, 2).reshape(128, 1024)
    Wq_b = np.asarray(Wq, np.float32).reshape(4, 128, A).transpose(1, 0, 2).reshape(128, 1024)
    Ws_b = np.asarray(Ws, np.float32)[:, 0].reshape(2, 128).T         # [128, 2]
    Wc_b = np.asarray(Wc, np.float32).reshape(8, 128, C).transpose(1, 0, 2).reshape(128, 800)
    bqm_u16 = np.ascontiguousarray(
        bqm_f.reshape(2, 128).T.astype(np.float32)).view(np.uint16)   # [128, 4]

    wpart = np.zeros((128, W_W), np.uint16)
    wpart[:, W_WM:W_WM + 1024] = Wm_b.astype(BF).view(np.uint16)
    wpart[:, W_WQ:W_WQ + 1024] = Wq_b.astype(BF).view(np.uint16)
    wpart[:, W_WS:W_WS + 2] = Ws_b.astype(BF).view(np.uint16)
    wpart[:, W_BQM:W_BQM + 4] = bqm_u16
    wpart[:, W_WC:W_WC + 800] = Wc_b.astype(BF).view(np.uint16)

    in_maps2 = []
    for c in range(NC_CORES):
        rows = slice(c * BROWS, (c + 1) * BROWS)
        knn_rows = memory_keys[top_idx[rows]].reshape(NCD, D)
        ktp = np.ascontiguousarray(
            knn_rows.reshape(2, 512, 4, 128).transpose(3, 0, 2, 1)
        ).reshape(128, KT_W).astype(BF).view(np.uint16)
        qTc = np.ascontiguousarray(
            q32[rows].T.reshape(4, 128, BROWS).transpose(1, 0, 2)
        ).reshape(128, 128).astype(BF).view(np.uint16)
        blob = np.zeros((128, BLOB_W), np.uint16)
        blob[:, :KT_W] = ktp
        blob[:, KT_W:] = wpart
        blob[:, KT_W + W_QT:KT_W + W_QT + 128] = qTc
        in_maps2.append({"blob": blob.view(BF)})
    res2 = run_bass_kernel_spmd(ph2, in_maps2, core_ids=list(range(NC_CORES)))

    out = np.zeros((B, C), np.float32)
    for c in range(NC_CORES):
        r = np.asarray(res2.results[c]["out"], np.float32)   # [64, 104]
        y1 = r[:BROWS, :C]
        y2 = r[BROWS:, :C]
        den = r[BROWS:, C]
        out[c * BROWS:(c + 1) * BROWS] = y1 + y2 / den[:, None]
    return (out + np.asarray(bc, np.float32)[None, :]).astype(np.float32)


# revision 22
# speedup vs baseline: 2.1081x; 1.0383x over previous
"""Trainium2 Bass kernel for retrieval-knn attention classifier (nn_MA_51866025067137).

Strategy (8 NeuronCores):
  Phase 1 — memory_keys sharded along N (12800 keys/core, padded 100000->102400
  with dummy rows), fed in bf16 (keys pre-normalized on host so the matmul
  directly yields cosine ranking values; host re-scores candidates in exact
  fp32 afterwards, so ranking precision only has to preserve the top-32 set).
  Per chunk of 512 keys: PE computes sims for all 256 queries (bf16 matmuls,
  fp32 PSUM), ACT evicts both query-halves in one [128,1024] copy, GPSIMD
  packs a 9-bit in-chunk index into the sim mantissa, DVE max8 keeps the
  top-8 per chunk.  Tail: 4 rounds of max8/max_index/match_replace extract
  the per-core top-32 (value, position) per query row.
  Host — merges the 8x32 candidates per row, re-scores them exactly in fp32,
  and gathers the global top-32 key vectors.
  Phase 2 — batch sharded (32 queries/core), all inputs packed into one bf16
  blob (2 logical DMAs): memory-attention module via bf16 matmuls; the
  softmax-score row is transposed with 8 tiny PE transposes (no DRAM bounce);
  attended@Wc is reassociated as sum_k w_k * (knn_k @ Wc2) so the weighted
  sum runs over a precomputed [1024,100] knnWc instead of [1024,512] knn
  (no knn tile, no attT transpose); normalization by sum(e) happens on host.
"""

import numpy as np
import ml_dtypes

import concourse.bacc as bacc
import concourse.mybir as mybir
from concourse.tile import TileContext
from concourse.bass_utils import run_bass_kernel_spmd

# problem dims (hardcoded per harness contract)
B, N, D = 256, 100000, 512
A, C, K = 256, 100, 32
NC_CORES = 8
NPAD = 102400             # 8 * 12800
SHARD = NPAD // NC_CORES  # 12800
CHUNK = 512               # keys per inner loop step
NCHUNK = SHARD // CHUNK   # 25
L1W = NCHUNK * 8          # 200
BROWS = B // NC_CORES     # 32 rows per core in phase 2
KLOC = 40                 # local candidates per core per row
NROUND = KLOC // 8        # 5 extraction rounds
CAND = NC_CORES * KLOC    # 320 merged candidates per row
KSCALE = 16.0             # fp8 range scaling (ranking is scale-invariant)
QSCALE = 32.0

f32 = mybir.dt.float32
f32r = mybir.dt.float32r
bf16 = mybir.dt.bfloat16
fp8 = mybir.dt.float8e4
u32 = mybir.dt.uint32
u16 = mybir.dt.uint16
BF = ml_dtypes.bfloat16
E4 = ml_dtypes.float8_e4m3

# ---- phase-2 blob layout (bf16 columns) ----
NCD = BROWS * K           # 1024
# ktile: [128, 4096], col = half*2048 + dc*512 + i  (bk = half*512 + i)
KT_W = 4096
# wtile: [128, 2984]
W_WM = 0                  # 4 dc x 256
W_WQ = 1024               # 4 dc x 256
W_WS = 2048               # 2 (col at)
W_BQM = 2050              # 4 bf16 cols = [128,2] f32 (byte offset 4100, 4-aligned)
W_QT = 2054               # 4 dc x 32
W_WC = 2182               # 8 m x 100
W_W = 2984                # padded (2982 used)
BLOB_W = KT_W + W_W       # 7080
CP4 = C + 4               # 104: knnWc stride (100 vals, col 100 = 1.0)

_PH1 = None
_PH2 = None


def _build_phase1():
    nc = bacc.Bacc("TRN2", target_bir_lowering=False)
    khatT = nc.dram_tensor("khatT", [NCHUNK, 128, 4 * CHUNK], fp8, kind="ExternalInput")
    qT = nc.dram_tensor("qT", [D, B], fp8, kind="ExternalInput")
    wp_out = nc.dram_tensor("wp", [B, 2 * KLOC], u32, kind="ExternalOutput")

    with TileContext(nc) as tc:
        with (
            tc.tile_pool(name="qpool", bufs=1) as qpool,
            tc.tile_pool(name="keys", bufs=5) as keyp,
            tc.tile_pool(name="pk", bufs=4) as pkp,
            tc.tile_pool(name="l1", bufs=1) as l1p,
            tc.tile_pool(name="small", bufs=1) as smallp,
            tc.tile_pool(name="psum", bufs=3, space="PSUM") as psump,
        ):
            # qT already relu'd, scaled, fp8 on host; one DMA, dc-major
            qTall = qpool.tile([128, 4 * B], fp8, tag="qtall")
            nc.sync.dma_start(out=qTall[:].rearrange("p (dc b) -> p dc b", dc=4),
                              in_=qT[:, :].rearrange("(dc p) b -> p dc b", p=128))
            qT3 = qTall[:].rearrange("p (dc b) -> p dc b", dc=4)  # [128,4,256]

            # static index lanes: each pk buffer's low u16 lanes hold the
            # in-chunk key index (0..511, repeated for both query halves);
            # written once by GPSIMD, reused as buffers rotate. The chunk id
            # is recovered from the winner's L1 position via max_index.
            pks = []
            for b in range(4):
                pk = pkp.tile([128, 2 * CHUNK], f32, tag="pk")
                pks.append(pk)
                nc.gpsimd.iota(
                    pk[:].bitcast(u16)
                        .rearrange("p (b two) -> p b two", two=2)[:, :, 0]
                        .rearrange("p (a b) -> p a b", a=2),
                    pattern=[[0, 2], [1, CHUNK]], base=0,
                    channel_multiplier=0)

            L1 = [l1p.tile([128, L1W], f32, tag=f"l1_{qt}", name=f"l1_{qt}")
                  for qt in range(2)]

            for c in range(NCHUNK):
                kt = keyp.tile([128, 4 * CHUNK], fp8, tag="kt")
                nc.sync.dma_start(out=kt[:], in_=khatT[c, :, :])
                kt3 = kt[:].rearrange("p (dc n) -> p dc n", dc=4)
                ps = psump.tile([128, 2 * CHUNK], f32, tag="sim")
                for qt in range(2):
                    for m in range(2):
                        nc.tensor.matmul(
                            ps[:, qt * CHUNK:(qt + 1) * CHUNK],
                            lhsT=qT3[:, 2 * m:2 * m + 2, qt * 128:(qt + 1) * 128],
                            rhs=kt3[:, 2 * m:2 * m + 2, :],
                            perf_mode=mybir.MatmulPerfMode.DoubleRow,
                            start=(m == 0), stop=(m == 1),
                        )
                # bf16(sim) into the high u16 lanes over the static index lanes
                pk = pks[c % 4]
                nc.scalar.copy(
                    out=pk[:].bitcast(bf16)
                        .rearrange("p (b two) -> p b two", two=2)[:, :, 1],
                    in_=ps[:])
                for qt in range(2):
                    nc.vector.max(out=L1[qt][:, c * 8:(c + 1) * 8],
                                  in_=pk[:, qt * CHUNK:(qt + 1) * CHUNK])

            # extraction: NROUND rounds of top-8 from L1 (200 wide);
            # win values (cols 0..KLOC) and L1 positions (cols KLOC..2K)
            for qt in range(2):
                wp = smallp.tile([128, 2 * KLOC], u32, tag=f"wp{qt}")
                for r in range(NROUND):
                    w8 = wp[:, r * 8:(r + 1) * 8].bitcast(f32)
                    nc.vector.max(out=w8, in_=L1[qt][:])
                    nc.vector.max_index(out=wp[:, KLOC + r * 8:KLOC + (r + 1) * 8],
                                        in_max=w8, in_values=L1[qt][:])
                    if r < NROUND - 1:
                        nc.vector.match_replace(out=L1[qt][:], in_to_replace=w8,
                                                in_values=L1[qt][:],
                                                imm_value=-3.0e38)
                nc.sync.dma_start(out=wp_out[qt * 128:(qt + 1) * 128, :], in_=wp[:])
    nc.finalize()
    return nc


def _build_phase2():
    nc = bacc.Bacc("TRN2", target_bir_lowering=False)
    blob = nc.dram_tensor("blob", [128, BLOB_W], bf16, kind="ExternalInput")
    out_d = nc.dram_tensor("out", [2 * BROWS, CP4], f32, kind="ExternalOutput")

    with TileContext(nc) as tc:
        with (
            tc.tile_pool(name="big", bufs=1) as bigp,
            tc.tile_pool(name="small", bufs=1) as smallp,
            tc.tile_pool(name="pskp", bufs=3, space="PSUM") as pskp,
            tc.tile_pool(name="pssc", bufs=1, space="PSUM") as pssc,
            tc.tile_pool(name="psmi", bufs=2, space="PSUM") as psmi,
            tc.tile_pool(name="psy", bufs=1, space="PSUM") as psy,
        ):
            # ---- DMAs: critical weights, then knnT quarters, then Wc ----
            wt = bigp.tile([128, W_W], bf16, tag="wt")
            nc.sync.dma_start(out=wt[:, :W_WC], in_=blob[:, KT_W:KT_W + W_WC])
            kt = bigp.tile([128, KT_W], bf16, tag="ktile")
            for q in range(4):
                nc.sync.dma_start(out=kt[:, q * 1024:(q + 1) * 1024],
                                  in_=blob[:, q * 1024:(q + 1) * 1024])
            nc.sync.dma_start(out=wt[:, W_WC:W_WC + 800],
                              in_=blob[:, KT_W + W_WC:KT_W + W_WC + 800])

            Wm = [wt[:, W_WM + dc * A:W_WM + (dc + 1) * A] for dc in range(4)]
            Wq = [wt[:, W_WQ + dc * A:W_WQ + (dc + 1) * A] for dc in range(4)]
            Ws = [wt[:, W_WS + at:W_WS + at + 1] for at in range(2)]
            bqm = wt[:, W_BQM:W_BQM + 4].bitcast(f32)            # [128, 2]
            qT = [wt[:, W_QT + dc * BROWS:W_QT + (dc + 1) * BROWS] for dc in range(4)]
            Wc = [wt[:, W_WC + m * C:W_WC + (m + 1) * C] for m in range(8)]

            # ---- PE p-state warmup (bridges the DMA lead-in) ----
            warm = smallp.tile([128, 512], bf16, tag="warm")
            nc.vector.memset(warm[:], 0.0)
            warm_ps = psmi.tile([128, CP4], f32, tag="mi")
            for i in range(12):
                nc.tensor.matmul(warm_ps[:, :C], lhsT=warm[:, :128],
                                 rhs=warm[:, :C], start=True, stop=True)

            # ---- small constants ----
            ident1 = smallp.tile([1, 1], f32, tag="id1")
            nc.vector.memset(ident1[:], 1.0)
            # mask4[p, j] = 1.0 iff j == p // 32
            mask4 = smallp.tile([128, 4], f32, tag="mask4")
            nc.vector.memset(mask4[:], 1.0)
            nc.gpsimd.affine_select(out=mask4[:], in_=mask4[:],
                                    compare_op=mybir.AluOpType.is_ge, fill=0.0,
                                    base=0, pattern=[[-32, 4]], channel_multiplier=1)
            nc.gpsimd.affine_select(out=mask4[:], in_=mask4[:],
                                    compare_op=mybir.AluOpType.is_ge, fill=0.0,
                                    base=31, pattern=[[32, 4]], channel_multiplier=-1)
            w2 = [smallp.tile([128, BROWS], bf16, tag=f"w2_{t}", name=f"w2t{t}")
                  for t in range(8)]
            for t in range(8):
                nc.vector.memset(w2[t][:], 0.0)
            kwS = smallp.tile([128, 8 * CP4], bf16, tag="kwS")   # knn@Wc2 (+ones col)
            for t in range(8):
                nc.vector.memset(kwS[:, t * CP4 + C:t * CP4 + C + 1], 1.0)

            # ---- qprojT [128(a), 2*32] ----
            qp_ps = psmi.tile([128, CP4], f32, tag="mi")
            for at in range(2):
                for dc in range(4):
                    nc.tensor.matmul(
                        qp_ps[:, at * BROWS:(at + 1) * BROWS],
                        lhsT=Wq[dc][:, at * 128:(at + 1) * 128], rhs=qT[dc],
                        start=(dc == 0), stop=(dc == 3))
            qprojT = smallp.tile([128, 2 * BROWS], f32, tag="qprojT")
            nc.scalar.copy(out=qprojT[:], in_=qp_ps[:, :2 * BROWS])

            # ---- y1 = relu(q) @ Wc1, shipped early (partitions 32..63) ----
            yy_ps = psy.tile([128, CP4], f32, tag="yy")
            for dc in range(4):
                nc.tensor.matmul(yy_ps[BROWS:2 * BROWS, :C], lhsT=qT[dc],
                                 rhs=Wc[dc], start=(dc == 0), stop=(dc == 3))
            osb = smallp.tile([2 * BROWS, CP4], f32, tag="osb")
            nc.scalar.copy(out=osb[:BROWS, :C], in_=yy_ps[BROWS:2 * BROWS, :C])
            nc.sync.dma_start(out=out_d[:BROWS, :], in_=osb[:BROWS, :])

            # ---- kprojT + h = tanh(. + qproj + bqm) -> scores row ----
            sc_ps = pssc.tile([128, NCD], f32, tag="sc")
            hT = [bigp.tile([128, NCD], bf16, tag=f"hT{at}", name=f"hTt{at}")
                  for at in range(2)]
            kph = {}
            for at in range(2):
                for half in range(2):
                    kp = pskp.tile([128, 512], f32, tag="kp")
                    kph[at, half] = kp
                    for dc in range(4):
                        nc.tensor.matmul(
                            kp[:],
                            lhsT=Wm[dc][:, at * 128:(at + 1) * 128],
                            rhs=kt[:, half * 2048 + dc * 512:half * 2048 + (dc + 1) * 512],
                            start=(dc == 0), stop=(dc == 3))
            for at in range(2):
                for half in range(2):
                    cols = slice(half * 512, (half + 1) * 512)
                    qb = qprojT[:, at * BROWS + half * 16:at * BROWS + half * 16 + 16,
                                None].to_broadcast([128, 16, K])
                    nc.vector.tensor_tensor(
                        hT[at][:, cols].rearrange("p (b k) -> p b k", k=K),
                        kph[at, half][:].rearrange("p (b k) -> p b k", k=K),
                        qb, mybir.AluOpType.add)
                    nc.scalar.activation(hT[at][:, cols], hT[at][:, cols],
                                         mybir.ActivationFunctionType.Tanh,
                                         bias=bqm[:, at:at + 1])
            for at in range(2):
                for half in range(2):
                    nc.tensor.matmul(
                        sc_ps[:1, half * 512:(half + 1) * 512],
                        lhsT=Ws[at],
                        rhs=hT[at][:, half * 512:(half + 1) * 512],
                        start=(at == 0), stop=(at == 1))

            # ---- knnWc[t] = knn_block_t @ Wc2 (overlaps scores tail) ----
            for t in range(8):
                kw_ps = psmi.tile([128, CP4], f32, tag="mi")
                half, blk = t // 4, t % 4
                for dc in range(4):
                    nc.tensor.matmul(
                        kw_ps[:, :C],
                        lhsT=kt[:, half * 2048 + dc * 512 + blk * 128:
                                half * 2048 + dc * 512 + (blk + 1) * 128],
                        rhs=Wc[4 + dc],
                        start=(dc == 0), stop=(dc == 3))
                nc.vector.tensor_copy(kwS[:, t * CP4:t * CP4 + C], kw_ps[:, :C])

            # ---- e row (exp per half) -> e_col via PE transposes; weights ----
            e_row = smallp.tile([1, NCD], f32, tag="e_row")
            ecT_ps = psmi.tile([128, CP4], f32, tag="mi")
            for half in range(2):
                nc.scalar.activation(e_row[:, half * 512:(half + 1) * 512],
                                     sc_ps[:1, half * 512:(half + 1) * 512],
                                     mybir.ActivationFunctionType.Exp)
                for tt in range(4):
                    t = half * 4 + tt
                    nc.tensor.transpose(ecT_ps[:, t:t + 1],
                                        e_row[:, t * 128:(t + 1) * 128], ident1[:])
                for tt in range(4):
                    t = half * 4 + tt
                    nc.vector.tensor_scalar_mul(w2[t][:, 4 * t:4 * t + 4],
                                                mask4[:], ecT_ps[:, t:t + 1])

            # ---- y2[b,:] = sum_k e * knnWc ; col C = sum_k e (den) ----
            for t in range(8):
                nc.tensor.matmul(yy_ps[:BROWS, :C + 1], lhsT=w2[t][:],
                                 rhs=kwS[:, t * CP4:t * CP4 + C + 1],
                                 start=(t == 0), stop=(t == 7))
            nc.scalar.copy(out=osb[BROWS:, :C + 1], in_=yy_ps[:BROWS, :C + 1])
            nc.sync.dma_start(out=out_d[BROWS:, :], in_=osb[BROWS:, :])
    nc.finalize()
    return nc


def _phase1_nc():
    global _PH1
    if _PH1 is None:
        _PH1 = _build_phase1()
    return _PH1


def _phase2_nc():
    global _PH2
    if _PH2 is None:
        _PH2 = _build_phase2()
    return _PH2


def kernel(query_feat, memory_keys, Wq, bq, Wm, bm, Ws, bs, Wc, bc):
    query_feat = np.asarray(query_feat, np.float32)
    memory_keys = np.asarray(memory_keys, np.float32)

    # ---- host prep: pad + normalize + transpose + shard keys (bf16) ----
    kn = np.sqrt((memory_keys ** 2).sum(axis=1))
    khat = memory_keys * (KSCALE / kn)[:, None]
    pad = np.full((NPAD - N, D), -KSCALE / np.sqrt(D), np.float32)
    khat_pad = np.concatenate([khat.astype(np.float32), pad], axis=0)
    q32 = np.maximum(query_feat, 0)
    qT_full = np.ascontiguousarray((q32.T * QSCALE).astype(E4))  # [512, 256]

    ph1 = _phase1_nc()
    in_maps = []
    for c in range(NC_CORES):
        sh = khat_pad[c * SHARD:(c + 1) * SHARD]          # [12800, 512]
        arr = np.ascontiguousarray(
            sh.reshape(NCHUNK, CHUNK, 4, 128).transpose(0, 3, 2, 1).astype(E4)
        ).reshape(NCHUNK, 128, 4 * CHUNK)
        in_maps.append({"khatT": arr, "qT": qT_full})
    res1 = run_bass_kernel_spmd(ph1, in_maps, core_ids=list(range(NC_CORES)))

    # ---- host merge: recover indices, exact re-score of candidates ----
    all_gidx = np.zeros((B, NC_CORES, KLOC), np.int64)
    for c in range(NC_CORES):
        wp = np.asarray(res1.results[c]["wp"]).view(np.uint32)
        win, pos = wp[:, :KLOC], wp[:, KLOC:].astype(np.int64)
        within = (win & np.uint32(0xFFFF)).astype(np.int64)  # in-chunk index
        all_gidx[:, c, :] = (pos // 8) * CHUNK + within + c * SHARD
    gidx = all_gidx.reshape(B, CAND)
    safe = np.minimum(gidx, N - 1)
    cand_keys = memory_keys[safe]                       # [256, 256, 512]
    dots = np.einsum("bd,bcd->bc", q32, cand_keys, optimize=True)
    cos = dots / np.maximum(
        np.linalg.norm(q32, axis=1)[:, None] * kn[safe], np.float32(1e-8))
    cos[gidx >= N] = -np.inf                            # mask dummy-pad hits
    order = np.argsort(-cos, axis=1, kind="stable")[:, :K]
    top_idx = np.take_along_axis(safe, order, axis=1)   # [256, 32]

    # ---- phase 2 (batch sharded): pack one bf16 blob per core ----
    ph2 = _phase2_nc()
    bqm_f = (np.asarray(bq, np.float32) + np.asarray(bm, np.float32))
    Wm_b = np.asarray(Wm, np.float32).reshape(4, 128, A).transpose(1, 0, 2).reshape(128, 1024)
    Wq_b = np.asarray(Wq, np.float32).reshape(4, 128, A).transpose(1, 0, 2).reshape(128, 1024)
    Ws_b = np.asarray(Ws, np.float32)[:, 0].reshape(2, 128).T         # [128, 2]
    Wc_b = np.asarray(Wc, np.float32).reshape(8, 128, C).transpose(1, 0, 2).reshape(128, 800)
    bqm_u16 = np.ascontiguousarray(
        bqm_f.reshape(2, 128).T.astype(np.float32)).view(np.uint16)   # [128, 4]

    wpart = np.zeros((128, W_W), np.uint16)
    wpart[:, W_WM:W_WM + 1024] = Wm_b.astype(BF).view(np.uint16)
    wpart[:, W_WQ:W_WQ + 1024] = Wq_b.astype(BF).view(np.uint16)
    wpart[:, W_WS:W_WS + 2] = Ws_b.astype(BF).view(np.uint16)
    wpart[:, W_BQM:W_BQM + 4] = bqm_u16
    wpart[:, W_WC:W_WC + 800] = Wc_b.astype(BF).view(np.uint16)

    in_maps2 = []
    for c in range(NC_CORES):
        rows = slice(c * BROWS, (c + 1) * BROWS)
        knn_rows = memory_keys[top_idx[rows]].reshape(NCD, D)
        ktp = np.ascontiguousarray(
            knn_rows.reshape(2, 512, 4, 128).transpose(3, 0, 2, 1)
        ).reshape(128, KT_W).astype(BF).view(np.uint16)
        qTc = np.ascontiguousarray(
            q32[rows].T.reshape(4, 128, BROWS).transpose(1, 0, 2)
        ).reshape(128, 128).astype(BF).view(np.uint16)
        blob = np.zeros((128, BLOB_W), np.uint16)
        blob[:, :KT_W] = ktp
        blob[:, KT_W:] = wpart
        blob[:, KT_W + W_QT:KT_W + W_QT + 128] = qTc
        in_maps2.append({"blob": blob.view(BF)})
    res2 = run_bass_kernel_spmd(ph2, in_maps2, core_ids=list(range(NC_CORES)))

    out = np.zeros((B, C), np.float32)
    for c in range(NC_CORES):
        r = np.asarray(res2.results[c]["out"], np.float32)   # [64, 104]
        y1 = r[:BROWS, :C]
        y2 = r[BROWS:, :C]
        den = r[BROWS:, C]
        out[c * BROWS:(c + 1) * BROWS] = y1 + y2 / den[:, None]
    return (out + np.asarray(bc, np.float32)[None, :]).astype(np.float32)


# revision 23
# speedup vs baseline: 2.1626x; 1.0258x over previous
"""Trainium2 Bass kernel for retrieval-knn attention classifier (nn_MA_51866025067137).

Strategy (8 NeuronCores):
  Phase 1 — memory_keys sharded along N (12800 keys/core, padded 100000->102400
  with dummy rows), fed in bf16 (keys pre-normalized on host so the matmul
  directly yields cosine ranking values; host re-scores candidates in exact
  fp32 afterwards, so ranking precision only has to preserve the top-32 set).
  Per chunk of 512 keys: PE computes sims for all 256 queries (bf16 matmuls,
  fp32 PSUM), ACT evicts both query-halves in one [128,1024] copy, GPSIMD
  packs a 9-bit in-chunk index into the sim mantissa, DVE max8 keeps the
  top-8 per chunk.  Tail: 4 rounds of max8/max_index/match_replace extract
  the per-core top-32 (value, position) per query row.
  Host — merges the 8x32 candidates per row, re-scores them exactly in fp32,
  and gathers the global top-32 key vectors.
  Phase 2 — batch sharded (32 queries/core), all inputs packed into one bf16
  blob (2 logical DMAs): memory-attention module via bf16 matmuls; the
  softmax-score row is transposed with 8 tiny PE transposes (no DRAM bounce);
  attended@Wc is reassociated as sum_k w_k * (knn_k @ Wc2) so the weighted
  sum runs over a precomputed [1024,100] knnWc instead of [1024,512] knn
  (no knn tile, no attT transpose); normalization by sum(e) happens on host.
"""

import numpy as np
import ml_dtypes

import concourse.bacc as bacc
import concourse.mybir as mybir
from concourse.tile import TileContext
from concourse.bass_utils import run_bass_kernel_spmd

# problem dims (hardcoded per harness contract)
B, N, D = 256, 100000, 512
A, C, K = 256, 100, 32
NC_CORES = 8
NPAD = 102400             # 8 * 12800
SHARD = NPAD // NC_CORES  # 12800
CHUNK = 512               # keys per inner loop step
NCHUNK = SHARD // CHUNK   # 25
L1W = NCHUNK * 8          # 200
BROWS = B // NC_CORES     # 32 rows per core in phase 2
KLOC = 32                 # local candidates per core per row
NROUND = KLOC // 8        # 5 extraction rounds
CAND = NC_CORES * KLOC    # 320 merged candidates per row
KSCALE = 16.0             # fp8 range scaling (ranking is scale-invariant)
QSCALE = 32.0

f32 = mybir.dt.float32
f32r = mybir.dt.float32r
bf16 = mybir.dt.bfloat16
fp8 = mybir.dt.float8e4
u32 = mybir.dt.uint32
u16 = mybir.dt.uint16
BF = ml_dtypes.bfloat16
E4 = ml_dtypes.float8_e4m3

# ---- phase-2 blob layout (bf16 columns) ----
NCD = BROWS * K           # 1024
# ktile: [128, 4096], col = half*2048 + dc*512 + i  (bk = half*512 + i)
KT_W = 4096
# wtile: [128, 2984]
W_WM = 0                  # 4 dc x 256
W_WQ = 1024               # 4 dc x 256
W_WS = 2048               # 2 (col at)
W_BQM = 2050              # 4 bf16 cols = [128,2] f32 (byte offset 4100, 4-aligned)
W_QT = 2054               # 4 dc x 32
W_WC = 2182               # 8 m x 100
W_W = 2984                # padded (2982 used)
BLOB_W = KT_W + W_W       # 7080
CP4 = C + 4               # 104: knnWc stride (100 vals, col 100 = 1.0)

_PH1 = None
_PH2 = None


def _build_phase1():
    nc = bacc.Bacc("TRN2", target_bir_lowering=False)
    khatT = nc.dram_tensor("khatT", [NCHUNK, 128, 4 * CHUNK], fp8, kind="ExternalInput")
    qT = nc.dram_tensor("qT", [D, B], fp8, kind="ExternalInput")
    wp_out = nc.dram_tensor("wp", [B, 2 * KLOC], u32, kind="ExternalOutput")

    with TileContext(nc) as tc:
        with (
            tc.tile_pool(name="qpool", bufs=1) as qpool,
            tc.tile_pool(name="keys", bufs=5) as keyp,
            tc.tile_pool(name="pk", bufs=4) as pkp,
            tc.tile_pool(name="l1", bufs=1) as l1p,
            tc.tile_pool(name="small", bufs=1) as smallp,
            tc.tile_pool(name="psum", bufs=3, space="PSUM") as psump,
        ):
            # qT already relu'd, scaled, fp8 on host; one DMA, dc-major
            qTall = qpool.tile([128, 4 * B], fp8, tag="qtall")
            nc.sync.dma_start(out=qTall[:].rearrange("p (dc b) -> p dc b", dc=4),
                              in_=qT[:, :].rearrange("(dc p) b -> p dc b", p=128))
            qT3 = qTall[:].rearrange("p (dc b) -> p dc b", dc=4)  # [128,4,256]

            # static index lanes: each pk buffer's low u16 lanes hold the
            # in-chunk key index (0..511, repeated for both query halves);
            # written once by GPSIMD, reused as buffers rotate. The chunk id
            # is recovered from the winner's L1 position via max_index.
            pks = []
            for b in range(4):
                pk = pkp.tile([128, 2 * CHUNK], f32, tag="pk")
                pks.append(pk)
                nc.gpsimd.iota(
                    pk[:].bitcast(u16)
                        .rearrange("p (b two) -> p b two", two=2)[:, :, 0]
                        .rearrange("p (a b) -> p a b", a=2),
                    pattern=[[0, 2], [1, CHUNK]], base=0,
                    channel_multiplier=0)

            L1 = [l1p.tile([128, L1W], f32, tag=f"l1_{qt}", name=f"l1_{qt}")
                  for qt in range(2)]

            for c in range(NCHUNK):
                kt = keyp.tile([128, 4 * CHUNK], fp8, tag="kt")
                nc.sync.dma_start(out=kt[:], in_=khatT[c, :, :])
                kt3 = kt[:].rearrange("p (dc n) -> p dc n", dc=4)
                ps = psump.tile([128, 2 * CHUNK], f32, tag="sim")
                for qt in range(2):
                    for m in range(2):
                        nc.tensor.matmul(
                            ps[:, qt * CHUNK:(qt + 1) * CHUNK],
                            lhsT=qT3[:, 2 * m:2 * m + 2, qt * 128:(qt + 1) * 128],
                            rhs=kt3[:, 2 * m:2 * m + 2, :],
                            perf_mode=mybir.MatmulPerfMode.DoubleRow,
                            start=(m == 0), stop=(m == 1),
                        )
                # bf16(sim) into the high u16 lanes over the static index lanes
                pk = pks[c % 4]
                nc.scalar.copy(
                    out=pk[:].bitcast(bf16)
                        .rearrange("p (b two) -> p b two", two=2)[:, :, 1],
                    in_=ps[:])
                for qt in range(2):
                    nc.vector.max(out=L1[qt][:, c * 8:(c + 1) * 8],
                                  in_=pk[:, qt * CHUNK:(qt + 1) * CHUNK])

            # extraction: NROUND rounds of top-8 from L1 (200 wide);
            # win values (cols 0..KLOC) and L1 positions (cols KLOC..2K)
            for qt in range(2):
                wp = smallp.tile([128, 2 * KLOC], u32, tag=f"wp{qt}")
                for r in range(NROUND):
                    w8 = wp[:, r * 8:(r + 1) * 8].bitcast(f32)
                    nc.vector.max(out=w8, in_=L1[qt][:])
                    nc.vector.max_index(out=wp[:, KLOC + r * 8:KLOC + (r + 1) * 8],
                                        in_max=w8, in_values=L1[qt][:])
                    if r < NROUND - 1:
                        nc.vector.match_replace(out=L1[qt][:], in_to_replace=w8,
                                                in_values=L1[qt][:],
                                                imm_value=-3.0e38)
                nc.sync.dma_start(out=wp_out[qt * 128:(qt + 1) * 128, :], in_=wp[:])
    nc.finalize()
    return nc


def _build_phase2():
    nc = bacc.Bacc("TRN2", target_bir_lowering=False)
    blob = nc.dram_tensor("blob", [128, BLOB_W], bf16, kind="ExternalInput")
    out_d = nc.dram_tensor("out", [2 * BROWS, CP4], f32, kind="ExternalOutput")

    with TileContext(nc) as tc:
        with (
            tc.tile_pool(name="big", bufs=1) as bigp,
            tc.tile_pool(name="small", bufs=1) as smallp,
            tc.tile_pool(name="pskp", bufs=3, space="PSUM") as pskp,
            tc.tile_pool(name="pssc", bufs=1, space="PSUM") as pssc,
            tc.tile_pool(name="psmi", bufs=2, space="PSUM") as psmi,
            tc.tile_pool(name="psy", bufs=1, space="PSUM") as psy,
        ):
            # ---- DMAs: critical weights, then knnT quarters, then Wc ----
            wt = bigp.tile([128, W_W], bf16, tag="wt")
            nc.sync.dma_start(out=wt[:, :W_WC], in_=blob[:, KT_W:KT_W + W_WC])
            kt = bigp.tile([128, KT_W], bf16, tag="ktile")
            for q in range(4):
                nc.sync.dma_start(out=kt[:, q * 1024:(q + 1) * 1024],
                                  in_=blob[:, q * 1024:(q + 1) * 1024])
            nc.sync.dma_start(out=wt[:, W_WC:W_WC + 800],
                              in_=blob[:, KT_W + W_WC:KT_W + W_WC + 800])

            Wm = [wt[:, W_WM + dc * A:W_WM + (dc + 1) * A] for dc in range(4)]
            Wq = [wt[:, W_WQ + dc * A:W_WQ + (dc + 1) * A] for dc in range(4)]
            Ws = [wt[:, W_WS + at:W_WS + at + 1] for at in range(2)]
            bqm = wt[:, W_BQM:W_BQM + 4].bitcast(f32)            # [128, 2]
            qT = [wt[:, W_QT + dc * BROWS:W_QT + (dc + 1) * BROWS] for dc in range(4)]
            Wc = [wt[:, W_WC + m * C:W_WC + (m + 1) * C] for m in range(8)]

            # ---- PE p-state warmup (bridges the DMA lead-in) ----
            warm = smallp.tile([128, 512], bf16, tag="warm")
            nc.vector.memset(warm[:], 0.0)
            warm_ps = psmi.tile([128, CP4], f32, tag="mi")
            for i in range(12):
                nc.tensor.matmul(warm_ps[:, :C], lhsT=warm[:, :128],
                                 rhs=warm[:, :C], start=True, stop=True)

            # ---- small constants ----
            ident1 = smallp.tile([1, 1], f32, tag="id1")
            nc.vector.memset(ident1[:], 1.0)
            # mask4[p, j] = 1.0 iff j == p // 32
            mask4 = smallp.tile([128, 4], f32, tag="mask4")
            nc.vector.memset(mask4[:], 1.0)
            nc.gpsimd.affine_select(out=mask4[:], in_=mask4[:],
                                    compare_op=mybir.AluOpType.is_ge, fill=0.0,
                                    base=0, pattern=[[-32, 4]], channel_multiplier=1)
            nc.gpsimd.affine_select(out=mask4[:], in_=mask4[:],
                                    compare_op=mybir.AluOpType.is_ge, fill=0.0,
                                    base=31, pattern=[[32, 4]], channel_multiplier=-1)
            w2 = [smallp.tile([128, BROWS], bf16, tag=f"w2_{t}", name=f"w2t{t}")
                  for t in range(8)]
            for t in range(8):
                nc.vector.memset(w2[t][:], 0.0)
            kwS = smallp.tile([128, 8 * CP4], bf16, tag="kwS")   # knn@Wc2 (+ones col)
            for t in range(8):
                nc.vector.memset(kwS[:, t * CP4 + C:t * CP4 + C + 1], 1.0)

            # ---- qprojT [128(a), 2*32] ----
            qp_ps = psmi.tile([128, CP4], f32, tag="mi")
            for at in range(2):
                for dc in range(4):
                    nc.tensor.matmul(
                        qp_ps[:, at * BROWS:(at + 1) * BROWS],
                        lhsT=Wq[dc][:, at * 128:(at + 1) * 128], rhs=qT[dc],
                        start=(dc == 0), stop=(dc == 3))
            qprojT = smallp.tile([128, 2 * BROWS], f32, tag="qprojT")
            nc.scalar.copy(out=qprojT[:], in_=qp_ps[:, :2 * BROWS])

            # ---- y1 = relu(q) @ Wc1, shipped early (partitions 32..63) ----
            yy_ps = psy.tile([128, CP4], f32, tag="yy")
            for dc in range(4):
                nc.tensor.matmul(yy_ps[BROWS:2 * BROWS, :C], lhsT=qT[dc],
                                 rhs=Wc[dc], start=(dc == 0), stop=(dc == 3))
            osb = smallp.tile([2 * BROWS, CP4], f32, tag="osb")
            nc.scalar.copy(out=osb[:BROWS, :C], in_=yy_ps[BROWS:2 * BROWS, :C])
            nc.sync.dma_start(out=out_d[:BROWS, :], in_=osb[:BROWS, :])

            # ---- kprojT + h = tanh(. + qproj + bqm) -> scores row ----
            sc_ps = pssc.tile([128, NCD], f32, tag="sc")
            hT = [bigp.tile([128, NCD], bf16, tag=f"hT{at}", name=f"hTt{at}")
                  for at in range(2)]
            kph = {}
            for at in range(2):
                for half in range(2):
                    kp = pskp.tile([128, 512], f32, tag="kp")
                    kph[at, half] = kp
                    for dc in range(4):
                        nc.tensor.matmul(
                            kp[:],
                            lhsT=Wm[dc][:, at * 128:(at + 1) * 128],
                            rhs=kt[:, half * 2048 + dc * 512:half * 2048 + (dc + 1) * 512],
                            start=(dc == 0), stop=(dc == 3))
            for at in range(2):
                for half in range(2):
                    cols = slice(half * 512, (half + 1) * 512)
                    qb = qprojT[:, at * BROWS + half * 16:at * BROWS + half * 16 + 16,
                                None].to_broadcast([128, 16, K])
                    nc.vector.tensor_tensor(
                        hT[at][:, cols].rearrange("p (b k) -> p b k", k=K),
                        kph[at, half][:].rearrange("p (b k) -> p b k", k=K),
                        qb, mybir.AluOpType.add)
                    nc.scalar.activation(hT[at][:, cols], hT[at][:, cols],
                                         mybir.ActivationFunctionType.Tanh,
                                         bias=bqm[:, at:at + 1])
            for at in range(2):
                for half in range(2):
                    nc.tensor.matmul(
                        sc_ps[:1, half * 512:(half + 1) * 512],
                        lhsT=Ws[at],
                        rhs=hT[at][:, half * 512:(half + 1) * 512],
                        start=(at == 0), stop=(at == 1))

            # ---- knnWc[t] = knn_block_t @ Wc2 (overlaps scores tail) ----
            for t in range(8):
                kw_ps = psmi.tile([128, CP4], f32, tag="mi")
                half, blk = t // 4, t % 4
                for dc in range(4):
                    nc.tensor.matmul(
                        kw_ps[:, :C],
                        lhsT=kt[:, half * 2048 + dc * 512 + blk * 128:
                                half * 2048 + dc * 512 + (blk + 1) * 128],
                        rhs=Wc[4 + dc],
                        start=(dc == 0), stop=(dc == 3))
                nc.vector.tensor_copy(kwS[:, t * CP4:t * CP4 + C], kw_ps[:, :C])

            # ---- e row (exp per half) -> e_col via PE transposes; weights ----
            e_row = smallp.tile([1, NCD], f32, tag="e_row")
            ecT_ps = psmi.tile([128, CP4], f32, tag="mi")
            for half in range(2):
                nc.scalar.activation(e_row[:, half * 512:(half + 1) * 512],
                                     sc_ps[:1, half * 512:(half + 1) * 512],
                                     mybir.ActivationFunctionType.Exp)
                for tt in range(4):
                    t = half * 4 + tt
                    nc.tensor.transpose(ecT_ps[:, t:t + 1],
                                        e_row[:, t * 128:(t + 1) * 128], ident1[:])
                for tt in range(4):
                    t = half * 4 + tt
                    nc.vector.tensor_scalar_mul(w2[t][:, 4 * t:4 * t + 4],
                                                mask4[:], ecT_ps[:, t:t + 1])

            # ---- y2[b,:] = sum_k e * knnWc ; col C = sum_k e (den) ----
            for t in range(8):
                nc.tensor.matmul(yy_ps[:BROWS, :C + 1], lhsT=w2[t][:],
                                 rhs=kwS[:, t * CP4:t * CP4 + C + 1],
                                 start=(t == 0), stop=(t == 7))
            nc.scalar.copy(out=osb[BROWS:, :C + 1], in_=yy_ps[:BROWS, :C + 1])
            nc.sync.dma_start(out=out_d[BROWS:, :], in_=osb[BROWS:, :])
    nc.finalize()
    return nc


def _phase1_nc():
    global _PH1
    if _PH1 is None:
        _PH1 = _build_phase1()
    return _PH1


def _phase2_nc():
    global _PH2
    if _PH2 is None:
        _PH2 = _build_phase2()
    return _PH2


def kernel(query_feat, memory_keys, Wq, bq, Wm, bm, Ws, bs, Wc, bc):
    query_feat = np.asarray(query_feat, np.float32)
    memory_keys = np.asarray(memory_keys, np.float32)

    # ---- host prep: pad + normalize + transpose + shard keys (bf16) ----
    kn = np.sqrt((memory_keys ** 2).sum(axis=1))
    khat = memory_keys * (KSCALE / kn)[:, None]
    pad = np.full((NPAD - N, D), -KSCALE / np.sqrt(D), np.float32)
    khat_pad = np.concatenate([khat.astype(np.float32), pad], axis=0)
    q32 = np.maximum(query_feat, 0)
    qT_full = np.ascontiguousarray((q32.T * QSCALE).astype(E4))  # [512, 256]

    ph1 = _phase1_nc()
    in_maps = []
    for c in range(NC_CORES):
        sh = khat_pad[c * SHARD:(c + 1) * SHARD]          # [12800, 512]
        arr = np.ascontiguousarray(
            sh.reshape(NCHUNK, CHUNK, 4, 128).transpose(0, 3, 2, 1).astype(E4)
        ).reshape(NCHUNK, 128, 4 * CHUNK)
        in_maps.append({"khatT": arr, "qT": qT_full})
    res1 = run_bass_kernel_spmd(ph1, in_maps, core_ids=list(range(NC_CORES)))

    # ---- host merge: recover indices, exact re-score of candidates ----
    all_gidx = np.zeros((B, NC_CORES, KLOC), np.int64)
    for c in range(NC_CORES):
        wp = np.asarray(res1.results[c]["wp"]).view(np.uint32)
        win, pos = wp[:, :KLOC], wp[:, KLOC:].astype(np.int64)
        within = (win & np.uint32(0xFFFF)).astype(np.int64)  # in-chunk index
        all_gidx[:, c, :] = (pos // 8) * CHUNK + within + c * SHARD
    gidx = all_gidx.reshape(B, CAND)
    safe = np.minimum(gidx, N - 1)
    cand_keys = memory_keys[safe]                       # [256, 256, 512]
    dots = np.einsum("bd,bcd->bc", q32, cand_keys, optimize=True)
    cos = dots / np.maximum(
        np.linalg.norm(q32, axis=1)[:, None] * kn[safe], np.float32(1e-8))
    cos[gidx >= N] = -np.inf                            # mask dummy-pad hits
    order = np.argsort(-cos, axis=1, kind="stable")[:, :K]
    top_idx = np.take_along_axis(safe, order, axis=1)   # [256, 32]

    # ---- phase 2 (batch sharded): pack one bf16 blob per core ----
    ph2 = _phase2_nc()
    bqm_f = (np.asarray(bq, np.float32) + np.asarray(bm, np.float32))
    Wm_b = np.asarray(Wm, np.float32).reshape(4, 128, A).transpose(1, 0, 2).reshape(128, 1024)
    Wq_b = np.asarray(Wq, np.float32).reshape(4, 128, A).transpose(1, 0, 2).reshape(128, 1024)
    Ws_b = np.asarray(Ws, np.float32)[:, 0].reshape(2, 128).T         # [128, 2]
    Wc_b = np.asarray(Wc, np.float32).reshape(8, 128, C).transpose(1, 0, 2).reshape(128, 800)
    bqm_u16 = np.ascontiguousarray(
        bqm_f.reshape(2, 128).T.astype(np.float32)).view(np.uint16)   # [128, 4]

    wpart = np.zeros((128, W_W), np.uint16)
    wpart[:, W_WM:W_WM + 1024] = Wm_b.astype(BF).view(np.uint16)
    wpart[:, W_WQ:W_WQ + 1024] = Wq_b.astype(BF).view(np.uint16)
    wpart[:, W_WS:W_WS + 2] = Ws_b.astype(BF).view(np.uint16)
    wpart[:, W_BQM:W_BQM + 4] = bqm_u16
    wpart[:, W_WC:W_WC + 800] = Wc_b.astype(BF).view(np.uint16)

    in_maps2 = []
    for c in range(NC_CORES):
        rows = slice(c * BROWS, (c + 1) * BROWS)
        knn_rows = memory_keys[top_idx[rows]].reshape(NCD, D)
        ktp = np.ascontiguousarray(
            knn_rows.reshape(2, 512, 4, 128).transpose(3, 0, 2, 1)
        ).reshape(128, KT_W).astype(BF).view(np.uint16)
        qTc = np.ascontiguousarray(
            q32[rows].T.reshape(4, 128, BROWS).transpose(1, 0, 2)
        ).reshape(128, 128).astype(BF).view(np.uint16)
        blob = np.zeros((128, BLOB_W), np.uint16)
        blob[:, :KT_W] = ktp
        blob[:, KT_W:] = wpart
        blob[:, KT_W + W_QT:KT_W + W_QT + 128] = qTc
        in_maps2.append({"blob": blob.view(BF)})
    res2 = run_bass_kernel_spmd(ph2, in_maps2, core_ids=list(range(NC_CORES)))

    out = np.zeros((B, C), np.float32)
    for c in range(NC_CORES):
        r = np.asarray(res2.results[c]["out"], np.float32)   # [64, 104]
        y1 = r[:BROWS, :C]
        y2 = r[BROWS:, :C]
        den = r[BROWS:, C]
        out[c * BROWS:(c + 1) * BROWS] = y1 + y2 / den[:, None]
    return (out + np.asarray(bc, np.float32)[None, :]).astype(np.float32)


# revision 24
# speedup vs baseline: 2.2199x; 1.0265x over previous
"""Trainium2 Bass kernel for retrieval-knn attention classifier (nn_MA_51866025067137).

Strategy (8 NeuronCores):
  Phase 1 — memory_keys sharded along N (12800 keys/core, padded 100000->102400
  with dummy rows), fed in bf16 (keys pre-normalized on host so the matmul
  directly yields cosine ranking values; host re-scores candidates in exact
  fp32 afterwards, so ranking precision only has to preserve the top-32 set).
  Per chunk of 512 keys: PE computes sims for all 256 queries (bf16 matmuls,
  fp32 PSUM), ACT evicts both query-halves in one [128,1024] copy, GPSIMD
  packs a 9-bit in-chunk index into the sim mantissa, DVE max8 keeps the
  top-8 per chunk.  Tail: 4 rounds of max8/max_index/match_replace extract
  the per-core top-32 (value, position) per query row.
  Host — merges the 8x32 candidates per row, re-scores them exactly in fp32,
  and gathers the global top-32 key vectors.
  Phase 2 — batch sharded (32 queries/core), all inputs packed into one bf16
  blob (2 logical DMAs): memory-attention module via bf16 matmuls; the
  softmax-score row is transposed with 8 tiny PE transposes (no DRAM bounce);
  attended@Wc is reassociated as sum_k w_k * (knn_k @ Wc2) so the weighted
  sum runs over a precomputed [1024,100] knnWc instead of [1024,512] knn
  (no knn tile, no attT transpose); normalization by sum(e) happens on host.
"""

import numpy as np
import ml_dtypes

import concourse.bacc as bacc
import concourse.mybir as mybir
from concourse.tile import TileContext
from concourse.bass_utils import run_bass_kernel_spmd

# problem dims (hardcoded per harness contract)
B, N, D = 256, 100000, 512
A, C, K = 256, 100, 32
NC_CORES = 8
NPAD = 102400             # 8 * 12800
SHARD = NPAD // NC_CORES  # 12800
CHUNK = 512               # keys per inner loop step
NCHUNK = SHARD // CHUNK   # 25
L1W = NCHUNK * 8          # 200
BROWS = B // NC_CORES     # 32 rows per core in phase 2
KLOC = 24                 # local candidates per core per row
NROUND = KLOC // 8        # 5 extraction rounds
CAND = NC_CORES * KLOC    # 320 merged candidates per row
KSCALE = 16.0             # fp8 range scaling (ranking is scale-invariant)
QSCALE = 32.0

f32 = mybir.dt.float32
f32r = mybir.dt.float32r
bf16 = mybir.dt.bfloat16
fp8 = mybir.dt.float8e4
u32 = mybir.dt.uint32
u16 = mybir.dt.uint16
BF = ml_dtypes.bfloat16
E4 = ml_dtypes.float8_e4m3

# ---- phase-2 blob layout (bf16 columns) ----
NCD = BROWS * K           # 1024
# ktile: [128, 4096], col = half*2048 + dc*512 + i  (bk = half*512 + i)
KT_W = 4096
# wtile: [128, 2984]
W_WM = 0                  # 4 dc x 256
W_WQ = 1024               # 4 dc x 256
W_WS = 2048               # 2 (col at)
W_BQM = 2050              # 4 bf16 cols = [128,2] f32 (byte offset 4100, 4-aligned)
W_QT = 2054               # 4 dc x 32
W_WC = 2182               # 8 m x 100
W_W = 2984                # padded (2982 used)
BLOB_W = KT_W + W_W       # 7080
CP4 = C + 4               # 104: knnWc stride (100 vals, col 100 = 1.0)

_PH1 = None
_PH2 = None


def _build_phase1():
    nc = bacc.Bacc("TRN2", target_bir_lowering=False)
    khatT = nc.dram_tensor("khatT", [NCHUNK, 128, 4 * CHUNK], fp8, kind="ExternalInput")
    qT = nc.dram_tensor("qT", [D, B], fp8, kind="ExternalInput")
    wp_out = nc.dram_tensor("wp", [B, 2 * KLOC], u32, kind="ExternalOutput")

    with TileContext(nc) as tc:
        with (
            tc.tile_pool(name="qpool", bufs=1) as qpool,
            tc.tile_pool(name="keys", bufs=5) as keyp,
            tc.tile_pool(name="pk", bufs=4) as pkp,
            tc.tile_pool(name="l1", bufs=1) as l1p,
            tc.tile_pool(name="small", bufs=1) as smallp,
            tc.tile_pool(name="psum", bufs=3, space="PSUM") as psump,
        ):
            # qT already relu'd, scaled, fp8 on host; one DMA, dc-major
            qTall = qpool.tile([128, 4 * B], fp8, tag="qtall")
            nc.sync.dma_start(out=qTall[:].rearrange("p (dc b) -> p dc b", dc=4),
                              in_=qT[:, :].rearrange("(dc p) b -> p dc b", p=128))
            qT3 = qTall[:].rearrange("p (dc b) -> p dc b", dc=4)  # [128,4,256]

            # static index lanes: each pk buffer's low u16 lanes hold the
            # in-chunk key index (0..511, repeated for both query halves);
            # written once by GPSIMD, reused as buffers rotate. The chunk id
            # is recovered from the winner's L1 position via max_index.
            pks = []
            for b in range(4):
                pk = pkp.tile([128, 2 * CHUNK], f32, tag="pk")
                pks.append(pk)
                nc.gpsimd.iota(
                    pk[:].bitcast(u16)
                        .rearrange("p (b two) -> p b two", two=2)[:, :, 0]
                        .rearrange("p (a b) -> p a b", a=2),
                    pattern=[[0, 2], [1, CHUNK]], base=0,
                    channel_multiplier=0)

            L1 = [l1p.tile([128, L1W], f32, tag=f"l1_{qt}", name=f"l1_{qt}")
                  for qt in range(2)]

            for c in range(NCHUNK):
                kt = keyp.tile([128, 4 * CHUNK], fp8, tag="kt")
                nc.sync.dma_start(out=kt[:], in_=khatT[c, :, :])
                kt3 = kt[:].rearrange("p (dc n) -> p dc n", dc=4)
                ps = psump.tile([128, 2 * CHUNK], f32, tag="sim")
                for qt in range(2):
                    for m in range(2):
                        nc.tensor.matmul(
                            ps[:, qt * CHUNK:(qt + 1) * CHUNK],
                            lhsT=qT3[:, 2 * m:2 * m + 2, qt * 128:(qt + 1) * 128],
                            rhs=kt3[:, 2 * m:2 * m + 2, :],
                            perf_mode=mybir.MatmulPerfMode.DoubleRow,
                            start=(m == 0), stop=(m == 1),
                        )
                # bf16(sim) into the high u16 lanes over the static index lanes
                pk = pks[c % 4]
                nc.scalar.copy(
                    out=pk[:].bitcast(bf16)
                        .rearrange("p (b two) -> p b two", two=2)[:, :, 1],
                    in_=ps[:])
                for qt in range(2):
                    nc.vector.max(out=L1[qt][:, c * 8:(c + 1) * 8],
                                  in_=pk[:, qt * CHUNK:(qt + 1) * CHUNK])

            # extraction: NROUND rounds of top-8 from L1 (200 wide);
            # win values (cols 0..KLOC) and L1 positions (cols KLOC..2K)
            for qt in range(2):
                wp = smallp.tile([128, 2 * KLOC], u32, tag=f"wp{qt}")
                for r in range(NROUND):
                    w8 = wp[:, r * 8:(r + 1) * 8].bitcast(f32)
                    nc.vector.max(out=w8, in_=L1[qt][:])
                    nc.vector.max_index(out=wp[:, KLOC + r * 8:KLOC + (r + 1) * 8],
                                        in_max=w8, in_values=L1[qt][:])
                    if r < NROUND - 1:
                        nc.vector.match_replace(out=L1[qt][:], in_to_replace=w8,
                                                in_values=L1[qt][:],
                                                imm_value=-3.0e38)
                nc.sync.dma_start(out=wp_out[qt * 128:(qt + 1) * 128, :], in_=wp[:])
    nc.finalize()
    return nc


def _build_phase2():
    nc = bacc.Bacc("TRN2", target_bir_lowering=False)
    blob = nc.dram_tensor("blob", [128, BLOB_W], bf16, kind="ExternalInput")
    out_d = nc.dram_tensor("out", [2 * BROWS, CP4], f32, kind="ExternalOutput")

    with TileContext(nc) as tc:
        with (
            tc.tile_pool(name="big", bufs=1) as bigp,
            tc.tile_pool(name="small", bufs=1) as smallp,
            tc.tile_pool(name="pskp", bufs=3, space="PSUM") as pskp,
            tc.tile_pool(name="pssc", bufs=1, space="PSUM") as pssc,
            tc.tile_pool(name="psmi", bufs=2, space="PSUM") as psmi,
            tc.tile_pool(name="psy", bufs=1, space="PSUM") as psy,
        ):
            # ---- DMAs: critical weights, then knnT quarters, then Wc ----
            wt = bigp.tile([128, W_W], bf16, tag="wt")
            nc.sync.dma_start(out=wt[:, :W_WC], in_=blob[:, KT_W:KT_W + W_WC])
            kt = bigp.tile([128, KT_W], bf16, tag="ktile")
            for q in range(4):
                nc.sync.dma_start(out=kt[:, q * 1024:(q + 1) * 1024],
                                  in_=blob[:, q * 1024:(q + 1) * 1024])
            nc.sync.dma_start(out=wt[:, W_WC:W_WC + 800],
                              in_=blob[:, KT_W + W_WC:KT_W + W_WC + 800])

            Wm = [wt[:, W_WM + dc * A:W_WM + (dc + 1) * A] for dc in range(4)]
            Wq = [wt[:, W_WQ + dc * A:W_WQ + (dc + 1) * A] for dc in range(4)]
            Ws = [wt[:, W_WS + at:W_WS + at + 1] for at in range(2)]
            bqm = wt[:, W_BQM:W_BQM + 4].bitcast(f32)            # [128, 2]
            qT = [wt[:, W_QT + dc * BROWS:W_QT + (dc + 1) * BROWS] for dc in range(4)]
            Wc = [wt[:, W_WC + m * C:W_WC + (m + 1) * C] for m in range(8)]

            # ---- PE p-state warmup (bridges the DMA lead-in) ----
            warm = smallp.tile([128, 512], bf16, tag="warm")
            nc.vector.memset(warm[:], 0.0)
            warm_ps = psmi.tile([128, CP4], f32, tag="mi")
            for i in range(12):
                nc.tensor.matmul(warm_ps[:, :C], lhsT=warm[:, :128],
                                 rhs=warm[:, :C], start=True, stop=True)

            # ---- small constants ----
            ident1 = smallp.tile([1, 1], f32, tag="id1")
            nc.vector.memset(ident1[:], 1.0)
            # mask4[p, j] = 1.0 iff j == p // 32
            mask4 = smallp.tile([128, 4], f32, tag="mask4")
            nc.vector.memset(mask4[:], 1.0)
            nc.gpsimd.affine_select(out=mask4[:], in_=mask4[:],
                                    compare_op=mybir.AluOpType.is_ge, fill=0.0,
                                    base=0, pattern=[[-32, 4]], channel_multiplier=1)
            nc.gpsimd.affine_select(out=mask4[:], in_=mask4[:],
                                    compare_op=mybir.AluOpType.is_ge, fill=0.0,
                                    base=31, pattern=[[32, 4]], channel_multiplier=-1)
            w2 = [smallp.tile([128, BROWS], bf16, tag=f"w2_{t}", name=f"w2t{t}")
                  for t in range(8)]
            for t in range(8):
                nc.vector.memset(w2[t][:], 0.0)
            kwS = smallp.tile([128, 8 * CP4], bf16, tag="kwS")   # knn@Wc2 (+ones col)
            for t in range(8):
                nc.vector.memset(kwS[:, t * CP4 + C:t * CP4 + C + 1], 1.0)

            # ---- qprojT [128(a), 2*32] ----
            qp_ps = psmi.tile([128, CP4], f32, tag="mi")
            for at in range(2):
                for dc in range(4):
                    nc.tensor.matmul(
                        qp_ps[:, at * BROWS:(at + 1) * BROWS],
                        lhsT=Wq[dc][:, at * 128:(at + 1) * 128], rhs=qT[dc],
                        start=(dc == 0), stop=(dc == 3))
            qprojT = smallp.tile([128, 2 * BROWS], f32, tag="qprojT")
            nc.scalar.copy(out=qprojT[:], in_=qp_ps[:, :2 * BROWS])

            # ---- y1 = relu(q) @ Wc1, shipped early (partitions 32..63) ----
            yy_ps = psy.tile([128, CP4], f32, tag="yy")
            for dc in range(4):
                nc.tensor.matmul(yy_ps[BROWS:2 * BROWS, :C], lhsT=qT[dc],
                                 rhs=Wc[dc], start=(dc == 0), stop=(dc == 3))
            osb = smallp.tile([2 * BROWS, CP4], f32, tag="osb")
            nc.scalar.copy(out=osb[:BROWS, :C], in_=yy_ps[BROWS:2 * BROWS, :C])
            nc.sync.dma_start(out=out_d[:BROWS, :], in_=osb[:BROWS, :])

            # ---- kprojT + h = tanh(. + qproj + bqm) -> scores row ----
            sc_ps = pssc.tile([128, NCD], f32, tag="sc")
            hT = [bigp.tile([128, NCD], bf16, tag=f"hT{at}", name=f"hTt{at}")
                  for at in range(2)]
            kph = {}
            for at in range(2):
                for half in range(2):
                    kp = pskp.tile([128, 512], f32, tag="kp")
                    kph[at, half] = kp
                    for dc in range(4):
                        nc.tensor.matmul(
                            kp[:],
                            lhsT=Wm[dc][:, at * 128:(at + 1) * 128],
                            rhs=kt[:, half * 2048 + dc * 512:half * 2048 + (dc + 1) * 512],
                            start=(dc == 0), stop=(dc == 3))
            for at in range(2):
                for half in range(2):
                    cols = slice(half * 512, (half + 1) * 512)
                    qb = qprojT[:, at * BROWS + half * 16:at * BROWS + half * 16 + 16,
                                None].to_broadcast([128, 16, K])
                    nc.vector.tensor_tensor(
                        hT[at][:, cols].rearrange("p (b k) -> p b k", k=K),
                        kph[at, half][:].rearrange("p (b k) -> p b k", k=K),
                        qb, mybir.AluOpType.add)
                    nc.scalar.activation(hT[at][:, cols], hT[at][:, cols],
                                         mybir.ActivationFunctionType.Tanh,
                                         bias=bqm[:, at:at + 1])
            for at in range(2):
                for half in range(2):
                    nc.tensor.matmul(
                        sc_ps[:1, half * 512:(half + 1) * 512],
                        lhsT=Ws[at],
                        rhs=hT[at][:, half * 512:(half + 1) * 512],
                        start=(at == 0), stop=(at == 1))

            # ---- knnWc[t] = knn_block_t @ Wc2 (overlaps scores tail) ----
            for t in range(8):
                kw_ps = psmi.tile([128, CP4], f32, tag="mi")
                half, blk = t // 4, t % 4
                for dc in range(4):
                    nc.tensor.matmul(
                        kw_ps[:, :C],
                        lhsT=kt[:, half * 2048 + dc * 512 + blk * 128:
                                half * 2048 + dc * 512 + (blk + 1) * 128],
                        rhs=Wc[4 + dc],
                        start=(dc == 0), stop=(dc == 3))
                nc.vector.tensor_copy(kwS[:, t * CP4:t * CP4 + C], kw_ps[:, :C])

            # ---- e row (exp per half) -> e_col via PE transposes; weights ----
            e_row = smallp.tile([1, NCD], f32, tag="e_row")
            ecT_ps = psmi.tile([128, CP4], f32, tag="mi")
            for half in range(2):
                nc.scalar.activation(e_row[:, half * 512:(half + 1) * 512],
                                     sc_ps[:1, half * 512:(half + 1) * 512],
                                     mybir.ActivationFunctionType.Exp)
                for tt in range(4):
                    t = half * 4 + tt
                    nc.tensor.transpose(ecT_ps[:, t:t + 1],
                                        e_row[:, t * 128:(t + 1) * 128], ident1[:])
                for tt in range(4):
                    t = half * 4 + tt
                    nc.vector.tensor_scalar_mul(w2[t][:, 4 * t:4 * t + 4],
                                                mask4[:], ecT_ps[:, t:t + 1])

            # ---- y2[b,:] = sum_k e * knnWc ; col C = sum_k e (den) ----
            for t in range(8):
                nc.tensor.matmul(yy_ps[:BROWS, :C + 1], lhsT=w2[t][:],
                                 rhs=kwS[:, t * CP4:t * CP4 + C + 1],
                                 start=(t == 0), stop=(t == 7))
            nc.scalar.copy(out=osb[BROWS:, :C + 1], in_=yy_ps[:BROWS, :C + 1])
            nc.sync.dma_start(out=out_d[BROWS:, :], in_=osb[BROWS:, :])
    nc.finalize()
    return nc


def _phase1_nc():
    global _PH1
    if _PH1 is None:
        _PH1 = _build_phase1()
    return _PH1


def _phase2_nc():
    global _PH2
    if _PH2 is None:
        _PH2 = _build_phase2()
    return _PH2


def kernel(query_feat, memory_keys, Wq, bq, Wm, bm, Ws, bs, Wc, bc):
    query_feat = np.asarray(query_feat, np.float32)
    memory_keys = np.asarray(memory_keys, np.float32)

    # ---- host prep: pad + normalize + transpose + shard keys (bf16) ----
    kn = np.sqrt((memory_keys ** 2).sum(axis=1))
    khat = memory_keys * (KSCALE / kn)[:, None]
    pad = np.full((NPAD - N, D), -KSCALE / np.sqrt(D), np.float32)
    khat_pad = np.concatenate([khat.astype(np.float32), pad], axis=0)
    q32 = np.maximum(query_feat, 0)
    qT_full = np.ascontiguousarray((q32.T * QSCALE).astype(E4))  # [512, 256]

    ph1 = _phase1_nc()
    in_maps = []
    for c in range(NC_CORES):
        sh = khat_pad[c * SHARD:(c + 1) * SHARD]          # [12800, 512]
        arr = np.ascontiguousarray(
            sh.reshape(NCHUNK, CHUNK, 4, 128).transpose(0, 3, 2, 1).astype(E4)
        ).reshape(NCHUNK, 128, 4 * CHUNK)
        in_maps.append({"khatT": arr, "qT": qT_full})
    res1 = run_bass_kernel_spmd(ph1, in_maps, core_ids=list(range(NC_CORES)))

    # ---- host merge: recover indices, exact re-score of candidates ----
    all_gidx = np.zeros((B, NC_CORES, KLOC), np.int64)
    for c in range(NC_CORES):
        wp = np.asarray(res1.results[c]["wp"]).view(np.uint32)
        win, pos = wp[:, :KLOC], wp[:, KLOC:].astype(np.int64)
        within = (win & np.uint32(0xFFFF)).astype(np.int64)  # in-chunk index
        all_gidx[:, c, :] = (pos // 8) * CHUNK + within + c * SHARD
    gidx = all_gidx.reshape(B, CAND)
    safe = np.minimum(gidx, N - 1)
    cand_keys = memory_keys[safe]                       # [256, 256, 512]
    dots = np.einsum("bd,bcd->bc", q32, cand_keys, optimize=True)
    cos = dots / np.maximum(
        np.linalg.norm(q32, axis=1)[:, None] * kn[safe], np.float32(1e-8))
    cos[gidx >= N] = -np.inf                            # mask dummy-pad hits
    order = np.argsort(-cos, axis=1, kind="stable")[:, :K]
    top_idx = np.take_along_axis(safe, order, axis=1)   # [256, 32]

    # ---- phase 2 (batch sharded): pack one bf16 blob per core ----
    ph2 = _phase2_nc()
    bqm_f = (np.asarray(bq, np.float32) + np.asarray(bm, np.float32))
    Wm_b = np.asarray(Wm, np.float32).reshape(4, 128, A).transpose(1, 0, 2).reshape(128, 1024)
    Wq_b = np.asarray(Wq, np.float32).reshape(4, 128, A).transpose(1, 0, 2).reshape(128, 1024)
    Ws_b = np.asarray(Ws, np.float32)[:, 0].reshape(2, 128).T         # [128, 2]
    Wc_b = np.asarray(Wc, np.float32).reshape(8, 128, C).transpose(1, 0, 2).reshape(128, 800)
    bqm_u16 = np.ascontiguousarray(
        bqm_f.reshape(2, 128).T.astype(np.float32)).view(np.uint16)   # [128, 4]

    wpart = np.zeros((128, W_W), np.uint16)
    wpart[:, W_WM:W_WM + 1024] = Wm_b.astype(BF).view(np.uint16)
    wpart[:, W_WQ:W_WQ + 1024] = Wq_b.astype(BF).view(np.uint16)
    wpart[:, W_WS:W_WS + 2] = Ws_b.astype(BF).view(np.uint16)
    wpart[:, W_BQM:W_BQM + 4] = bqm_u16
    wpart[:, W_WC:W_WC + 800] = Wc_b.astype(BF).view(np.uint16)

    in_maps2 = []
    for c in range(NC_CORES):
        rows = slice(c * BROWS, (c + 1) * BROWS)
        knn_rows = memory_keys[top_idx[rows]].reshape(NCD, D)
        ktp = np.ascontiguousarray(
            knn_rows.reshape(2, 512, 4, 128).transpose(3, 0, 2, 1)
        ).reshape(128, KT_W).astype(BF).view(np.uint16)
        qTc = np.ascontiguousarray(
            q32[rows].T.reshape(4, 128, BROWS).transpose(1, 0, 2)
        ).reshape(128, 128).astype(BF).view(np.uint16)
        blob = np.zeros((128, BLOB_W), np.uint16)
        blob[:, :KT_W] = ktp
        blob[:, KT_W:] = wpart
        blob[:, KT_W + W_QT:KT_W + W_QT + 128] = qTc
        in_maps2.append({"blob": blob.view(BF)})
    res2 = run_bass_kernel_spmd(ph2, in_maps2, core_ids=list(range(NC_CORES)))

    out = np.zeros((B, C), np.float32)
    for c in range(NC_CORES):
        r = np.asarray(res2.results[c]["out"], np.float32)   # [64, 104]
        y1 = r[:BROWS, :C]
        y2 = r[BROWS:, :C]
        den = r[BROWS:, C]
        out[c * BROWS:(c + 1) * BROWS] = y1 + y2 / den[:, None]
    return (out + np.asarray(bc, np.float32)[None, :]).astype(np.float32)


# revision 25
# speedup vs baseline: 2.2803x; 1.0272x over previous
"""Trainium2 Bass kernel for retrieval-knn attention classifier (nn_MA_51866025067137).

Strategy (8 NeuronCores):
  Phase 1 — memory_keys sharded along N (12800 keys/core, padded 100000->102400
  with dummy rows), fed in bf16 (keys pre-normalized on host so the matmul
  directly yields cosine ranking values; host re-scores candidates in exact
  fp32 afterwards, so ranking precision only has to preserve the top-32 set).
  Per chunk of 512 keys: PE computes sims for all 256 queries (bf16 matmuls,
  fp32 PSUM), ACT evicts both query-halves in one [128,1024] copy, GPSIMD
  packs a 9-bit in-chunk index into the sim mantissa, DVE max8 keeps the
  top-8 per chunk.  Tail: 4 rounds of max8/max_index/match_replace extract
  the per-core top-32 (value, position) per query row.
  Host — merges the 8x32 candidates per row, re-scores them exactly in fp32,
  and gathers the global top-32 key vectors.
  Phase 2 — batch sharded (32 queries/core), all inputs packed into one bf16
  blob (2 logical DMAs): memory-attention module via bf16 matmuls; the
  softmax-score row is transposed with 8 tiny PE transposes (no DRAM bounce);
  attended@Wc is reassociated as sum_k w_k * (knn_k @ Wc2) so the weighted
  sum runs over a precomputed [1024,100] knnWc instead of [1024,512] knn
  (no knn tile, no attT transpose); normalization by sum(e) happens on host.
"""

import numpy as np
import ml_dtypes

import concourse.bacc as bacc
import concourse.mybir as mybir
from concourse.tile import TileContext
from concourse.bass_utils import run_bass_kernel_spmd

# problem dims (hardcoded per harness contract)
B, N, D = 256, 100000, 512
A, C, K = 256, 100, 32
NC_CORES = 8
NPAD = 102400             # 8 * 12800
SHARD = NPAD // NC_CORES  # 12800
CHUNK = 512               # keys per inner loop step
NCHUNK = SHARD // CHUNK   # 25
L1W = NCHUNK * 8          # 200
BROWS = B // NC_CORES     # 32 rows per core in phase 2
KLOC = 16                 # local candidates per core per row
NROUND = KLOC // 8        # 5 extraction rounds
CAND = NC_CORES * KLOC    # 320 merged candidates per row
KSCALE = 16.0             # fp8 range scaling (ranking is scale-invariant)
QSCALE = 32.0

f32 = mybir.dt.float32
f32r = mybir.dt.float32r
bf16 = mybir.dt.bfloat16
fp8 = mybir.dt.float8e4
u32 = mybir.dt.uint32
u16 = mybir.dt.uint16
BF = ml_dtypes.bfloat16
E4 = ml_dtypes.float8_e4m3

# ---- phase-2 blob layout (bf16 columns) ----
NCD = BROWS * K           # 1024
# ktile: [128, 4096], col = half*2048 + dc*512 + i  (bk = half*512 + i)
KT_W = 4096
# wtile: [128, 2984]
W_WM = 0                  # 4 dc x 256
W_WQ = 1024               # 4 dc x 256
W_WS = 2048               # 2 (col at)
W_BQM = 2050              # 4 bf16 cols = [128,2] f32 (byte offset 4100, 4-aligned)
W_QT = 2054               # 4 dc x 32
W_WC = 2182               # 8 m x 100
W_W = 2984                # padded (2982 used)
BLOB_W = KT_W + W_W       # 7080
CP4 = C + 4               # 104: knnWc stride (100 vals, col 100 = 1.0)

_PH1 = None
_PH2 = None


def _build_phase1():
    nc = bacc.Bacc("TRN2", target_bir_lowering=False)
    khatT = nc.dram_tensor("khatT", [NCHUNK, 128, 4 * CHUNK], fp8, kind="ExternalInput")
    qT = nc.dram_tensor("qT", [D, B], fp8, kind="ExternalInput")
    wp_out = nc.dram_tensor("wp", [B, 2 * KLOC], u32, kind="ExternalOutput")

    with TileContext(nc) as tc:
        with (
            tc.tile_pool(name="qpool", bufs=1) as qpool,
            tc.tile_pool(name="keys", bufs=5) as keyp,
            tc.tile_pool(name="pk", bufs=4) as pkp,
            tc.tile_pool(name="l1", bufs=1) as l1p,
            tc.tile_pool(name="small", bufs=1) as smallp,
            tc.tile_pool(name="psum", bufs=3, space="PSUM") as psump,
        ):
            # qT already relu'd, scaled, fp8 on host; one DMA, dc-major
            qTall = qpool.tile([128, 4 * B], fp8, tag="qtall")
            nc.sync.dma_start(out=qTall[:].rearrange("p (dc b) -> p dc b", dc=4),
                              in_=qT[:, :].rearrange("(dc p) b -> p dc b", p=128))
            qT3 = qTall[:].rearrange("p (dc b) -> p dc b", dc=4)  # [128,4,256]

            # static index lanes: each pk buffer's low u16 lanes hold the
            # in-chunk key index (0..511, repeated for both query halves);
            # written once by GPSIMD, reused as buffers rotate. The chunk id
            # is recovered from the winner's L1 position via max_index.
            pks = []
            for b in range(4):
                pk = pkp.tile([128, 2 * CHUNK], f32, tag="pk")
                pks.append(pk)
                nc.gpsimd.iota(
                    pk[:].bitcast(u16)
                        .rearrange("p (b two) -> p b two", two=2)[:, :, 0]
                        .rearrange("p (a b) -> p a b", a=2),
                    pattern=[[0, 2], [1, CHUNK]], base=0,
                    channel_multiplier=0)

            L1 = [l1p.tile([128, L1W], f32, tag=f"l1_{qt}", name=f"l1_{qt}")
                  for qt in range(2)]

            for c in range(NCHUNK):
                kt = keyp.tile([128, 4 * CHUNK], fp8, tag="kt")
                nc.sync.dma_start(out=kt[:], in_=khatT[c, :, :])
                kt3 = kt[:].rearrange("p (dc n) -> p dc n", dc=4)
                ps = psump.tile([128, 2 * CHUNK], f32, tag="sim")
                for qt in range(2):
                    for m in range(2):
                        nc.tensor.matmul(
                            ps[:, qt * CHUNK:(qt + 1) * CHUNK],
                            lhsT=qT3[:, 2 * m:2 * m + 2, qt * 128:(qt + 1) * 128],
                            rhs=kt3[:, 2 * m:2 * m + 2, :],
                            perf_mode=mybir.MatmulPerfMode.DoubleRow,
                            start=(m == 0), stop=(m == 1),
                        )
                # bf16(sim) into the high u16 lanes over the static index lanes
                pk = pks[c % 4]
                nc.scalar.copy(
                    out=pk[:].bitcast(bf16)
                        .rearrange("p (b two) -> p b two", two=2)[:, :, 1],
                    in_=ps[:])
                for qt in range(2):
                    nc.vector.max(out=L1[qt][:, c * 8:(c + 1) * 8],
                                  in_=pk[:, qt * CHUNK:(qt + 1) * CHUNK])

            # extraction: NROUND rounds of top-8 from L1 (200 wide);
            # win values (cols 0..KLOC) and L1 positions (cols KLOC..2K)
            for qt in range(2):
                wp = smallp.tile([128, 2 * KLOC], u32, tag=f"wp{qt}")
                for r in range(NROUND):
                    w8 = wp[:, r * 8:(r + 1) * 8].bitcast(f32)
                    nc.vector.max(out=w8, in_=L1[qt][:])
                    nc.vector.max_index(out=wp[:, KLOC + r * 8:KLOC + (r + 1) * 8],
                                        in_max=w8, in_values=L1[qt][:])
                    if r < NROUND - 1:
                        nc.vector.match_replace(out=L1[qt][:], in_to_replace=w8,
                                                in_values=L1[qt][:],
                                                imm_value=-3.0e38)
                nc.sync.dma_start(out=wp_out[qt * 128:(qt + 1) * 128, :], in_=wp[:])
    nc.finalize()
    return nc


def _build_phase2():
    nc = bacc.Bacc("TRN2", target_bir_lowering=False)
    blob = nc.dram_tensor("blob", [128, BLOB_W], bf16, kind="ExternalInput")
    out_d = nc.dram_tensor("out", [2 * BROWS, CP4], f32, kind="ExternalOutput")

    with TileContext(nc) as tc:
        with (
            tc.tile_pool(name="big", bufs=1) as bigp,
            tc.tile_pool(name="small", bufs=1) as smallp,
            tc.tile_pool(name="pskp", bufs=3, space="PSUM") as pskp,
            tc.tile_pool(name="pssc", bufs=1, space="PSUM") as pssc,
            tc.tile_pool(name="psmi", bufs=2, space="PSUM") as psmi,
            tc.tile_pool(name="psy", bufs=1, space="PSUM") as psy,
        ):
            # ---- DMAs: critical weights, then knnT quarters, then Wc ----
            wt = bigp.tile([128, W_W], bf16, tag="wt")
            nc.sync.dma_start(out=wt[:, :W_WC], in_=blob[:, KT_W:KT_W + W_WC])
            kt = bigp.tile([128, KT_W], bf16, tag="ktile")
            for q in range(4):
                nc.sync.dma_start(out=kt[:, q * 1024:(q + 1) * 1024],
                                  in_=blob[:, q * 1024:(q + 1) * 1024])
            nc.sync.dma_start(out=wt[:, W_WC:W_WC + 800],
                              in_=blob[:, KT_W + W_WC:KT_W + W_WC + 800])

            Wm = [wt[:, W_WM + dc * A:W_WM + (dc + 1) * A] for dc in range(4)]
            Wq = [wt[:, W_WQ + dc * A:W_WQ + (dc + 1) * A] for dc in range(4)]
            Ws = [wt[:, W_WS + at:W_WS + at + 1] for at in range(2)]
            bqm = wt[:, W_BQM:W_BQM + 4].bitcast(f32)            # [128, 2]
            qT = [wt[:, W_QT + dc * BROWS:W_QT + (dc + 1) * BROWS] for dc in range(4)]
            Wc = [wt[:, W_WC + m * C:W_WC + (m + 1) * C] for m in range(8)]

            # ---- PE p-state warmup (bridges the DMA lead-in) ----
            warm = smallp.tile([128, 512], bf16, tag="warm")
            nc.vector.memset(warm[:], 0.0)
            warm_ps = psmi.tile([128, CP4], f32, tag="mi")
            for i in range(12):
                nc.tensor.matmul(warm_ps[:, :C], lhsT=warm[:, :128],
                                 rhs=warm[:, :C], start=True, stop=True)

            # ---- small constants ----
            ident1 = smallp.tile([1, 1], f32, tag="id1")
            nc.vector.memset(ident1[:], 1.0)
            # mask4[p, j] = 1.0 iff j == p // 32
            mask4 = smallp.tile([128, 4], f32, tag="mask4")
            nc.vector.memset(mask4[:], 1.0)
            nc.gpsimd.affine_select(out=mask4[:], in_=mask4[:],
                                    compare_op=mybir.AluOpType.is_ge, fill=0.0,
                                    base=0, pattern=[[-32, 4]], channel_multiplier=1)
            nc.gpsimd.affine_select(out=mask4[:], in_=mask4[:],
                                    compare_op=mybir.AluOpType.is_ge, fill=0.0,
                                    base=31, pattern=[[32, 4]], channel_multiplier=-1)
            w2 = [smallp.tile([128, BROWS], bf16, tag=f"w2_{t}", name=f"w2t{t}")
                  for t in range(8)]
            for t in range(8):
                nc.vector.memset(w2[t][:], 0.0)
            kwS = smallp.tile([128, 8 * CP4], bf16, tag="kwS")   # knn@Wc2 (+ones col)
            for t in range(8):
                nc.vector.memset(kwS[:, t * CP4 + C:t * CP4 + C + 1], 1.0)

            # ---- qprojT [128(a), 2*32] ----
            qp_ps = psmi.tile([128, CP4], f32, tag="mi")
            for at in range(2):
                for dc in range(4):
                    nc.tensor.matmul(
                        qp_ps[:, at * BROWS:(at + 1) * BROWS],
                        lhsT=Wq[dc][:, at * 128:(at + 1) * 128], rhs=qT[dc],
                        start=(dc == 0), stop=(dc == 3))
            qprojT = smallp.tile([128, 2 * BROWS], f32, tag="qprojT")
            nc.scalar.copy(out=qprojT[:], in_=qp_ps[:, :2 * BROWS])

            # ---- y1 = relu(q) @ Wc1, shipped early (partitions 32..63) ----
            yy_ps = psy.tile([128, CP4], f32, tag="yy")
            for dc in range(4):
                nc.tensor.matmul(yy_ps[BROWS:2 * BROWS, :C], lhsT=qT[dc],
                                 rhs=Wc[dc], start=(dc == 0), stop=(dc == 3))
            osb = smallp.tile([2 * BROWS, CP4], f32, tag="osb")
            nc.scalar.copy(out=osb[:BROWS, :C], in_=yy_ps[BROWS:2 * BROWS, :C])
            nc.sync.dma_start(out=out_d[:BROWS, :], in_=osb[:BROWS, :])

            # ---- kprojT + h = tanh(. + qproj + bqm) -> scores row ----
            sc_ps = pssc.tile([128, NCD], f32, tag="sc")
            hT = [bigp.tile([128, NCD], bf16, tag=f"hT{at}", name=f"hTt{at}")
                  for at in range(2)]
            kph = {}
            for at in range(2):
                for half in range(2):
                    kp = pskp.tile([128, 512], f32, tag="kp")
                    kph[at, half] = kp
                    for dc in range(4):
                        nc.tensor.matmul(
                            kp[:],
                            lhsT=Wm[dc][:, at * 128:(at + 1) * 128],
                            rhs=kt[:, half * 2048 + dc * 512:half * 2048 + (dc + 1) * 512],
                            start=(dc == 0), stop=(dc == 3))
            for at in range(2):
                for half in range(2):
                    cols = slice(half * 512, (half + 1) * 512)
                    qb = qprojT[:, at * BROWS + half * 16:at * BROWS + half * 16 + 16,
                                None].to_broadcast([128, 16, K])
                    nc.vector.tensor_tensor(
                        hT[at][:, cols].rearrange("p (b k) -> p b k", k=K),
                        kph[at, half][:].rearrange("p (b k) -> p b k", k=K),
                        qb, mybir.AluOpType.add)
                    nc.scalar.activation(hT[at][:, cols], hT[at][:, cols],
                                         mybir.ActivationFunctionType.Tanh,
                                         bias=bqm[:, at:at + 1])
            for at in range(2):
                for half in range(2):
                    nc.tensor.matmul(
                        sc_ps[:1, half * 512:(half + 1) * 512],
                        lhsT=Ws[at],
                        rhs=hT[at][:, half * 512:(half + 1) * 512],
                        start=(at == 0), stop=(at == 1))

            # ---- knnWc[t] = knn_block_t @ Wc2 (overlaps scores tail) ----
            for t in range(8):
                kw_ps = psmi.tile([128, CP4], f32, tag="mi")
                half, blk = t // 4, t % 4
                for dc in range(4):
                    nc.tensor.matmul(
                        kw_ps[:, :C],
                        lhsT=kt[:, half * 2048 + dc * 512 + blk * 128:
                                half * 2048 + dc * 512 + (blk + 1) * 128],
                        rhs=Wc[4 + dc],
                        start=(dc == 0), stop=(dc == 3))
                nc.vector.tensor_copy(kwS[:, t * CP4:t * CP4 + C], kw_ps[:, :C])

            # ---- e row (exp per half) -> e_col via PE transposes; weights ----
            e_row = smallp.tile([1, NCD], f32, tag="e_row")
            ecT_ps = psmi.tile([128, CP4], f32, tag="mi")
            for half in range(2):
                nc.scalar.activation(e_row[:, half * 512:(half + 1) * 512],
                                     sc_ps[:1, half * 512:(half + 1) * 512],
                                     mybir.ActivationFunctionType.Exp)
                for tt in range(4):
                    t = half * 4 + tt
                    nc.tensor.transpose(ecT_ps[:, t:t + 1],
                                        e_row[:, t * 128:(t + 1) * 128], ident1[:])
                for tt in range(4):
                    t = half * 4 + tt
                    nc.vector.tensor_scalar_mul(w2[t][:, 4 * t:4 * t + 4],
                                                mask4[:], ecT_ps[:, t:t + 1])

            # ---- y2[b,:] = sum_k e * knnWc ; col C = sum_k e (den) ----
            for t in range(8):
                nc.tensor.matmul(yy_ps[:BROWS, :C + 1], lhsT=w2[t][:],
                                 rhs=kwS[:, t * CP4:t * CP4 + C + 1],
                                 start=(t == 0), stop=(t == 7))
            nc.scalar.copy(out=osb[BROWS:, :C + 1], in_=yy_ps[:BROWS, :C + 1])
            nc.sync.dma_start(out=out_d[BROWS:, :], in_=osb[BROWS:, :])
    nc.finalize()
    return nc


def _phase1_nc():
    global _PH1
    if _PH1 is None:
        _PH1 = _build_phase1()
    return _PH1


def _phase2_nc():
    global _PH2
    if _PH2 is None:
        _PH2 = _build_phase2()
    return _PH2


def kernel(query_feat, memory_keys, Wq, bq, Wm, bm, Ws, bs, Wc, bc):
    query_feat = np.asarray(query_feat, np.float32)
    memory_keys = np.asarray(memory_keys, np.float32)

    # ---- host prep: pad + normalize + transpose + shard keys (bf16) ----
    kn = np.sqrt((memory_keys ** 2).sum(axis=1))
    khat = memory_keys * (KSCALE / kn)[:, None]
    pad = np.full((NPAD - N, D), -KSCALE / np.sqrt(D), np.float32)
    khat_pad = np.concatenate([khat.astype(np.float32), pad], axis=0)
    q32 = np.maximum(query_feat, 0)
    qT_full = np.ascontiguousarray((q32.T * QSCALE).astype(E4))  # [512, 256]

    ph1 = _phase1_nc()
    in_maps = []
    for c in range(NC_CORES):
        sh = khat_pad[c * SHARD:(c + 1) * SHARD]          # [12800, 512]
        arr = np.ascontiguousarray(
            sh.reshape(NCHUNK, CHUNK, 4, 128).transpose(0, 3, 2, 1).astype(E4)
        ).reshape(NCHUNK, 128, 4 * CHUNK)
        in_maps.append({"khatT": arr, "qT": qT_full})
    res1 = run_bass_kernel_spmd(ph1, in_maps, core_ids=list(range(NC_CORES)))

    # ---- host merge: recover indices, exact re-score of candidates ----
    all_gidx = np.zeros((B, NC_CORES, KLOC), np.int64)
    for c in range(NC_CORES):
        wp = np.asarray(res1.results[c]["wp"]).view(np.uint32)
        win, pos = wp[:, :KLOC], wp[:, KLOC:].astype(np.int64)
        within = (win & np.uint32(0xFFFF)).astype(np.int64)  # in-chunk index
        all_gidx[:, c, :] = (pos // 8) * CHUNK + within + c * SHARD
    gidx = all_gidx.reshape(B, CAND)
    safe = np.minimum(gidx, N - 1)
    cand_keys = memory_keys[safe]                       # [256, 256, 512]
    dots = np.einsum("bd,bcd->bc", q32, cand_keys, optimize=True)
    cos = dots / np.maximum(
        np.linalg.norm(q32, axis=1)[:, None] * kn[safe], np.float32(1e-8))
    cos[gidx >= N] = -np.inf                            # mask dummy-pad hits
    order = np.argsort(-cos, axis=1, kind="stable")[:, :K]
    top_idx = np.take_along_axis(safe, order, axis=1)   # [256, 32]

    # ---- phase 2 (batch sharded): pack one bf16 blob per core ----
    ph2 = _phase2_nc()
    bqm_f = (np.asarray(bq, np.float32) + np.asarray(bm, np.float32))
    Wm_b = np.asarray(Wm, np.float32).reshape(4, 128, A).transpose(1, 0, 2).reshape(128, 1024)
    Wq_b = np.asarray(Wq, np.float32).reshape(4, 128, A).transpose(1, 0, 2).reshape(128, 1024)
    Ws_b = np.asarray(Ws, np.float32)[:, 0].reshape(2, 128).T         # [128, 2]
    Wc_b = np.asarray(Wc, np.float32).reshape(8, 128, C).transpose(1, 0, 2).reshape(128, 800)
    bqm_u16 = np.ascontiguousarray(
        bqm_f.reshape(2, 128).T.astype(np.float32)).view(np.uint16)   # [128, 4]

    wpart = np.zeros((128, W_W), np.uint16)
    wpart[:, W_WM:W_WM + 1024] = Wm_b.astype(BF).view(np.uint16)
    wpart[:, W_WQ:W_WQ + 1024] = Wq_b.astype(BF).view(np.uint16)
    wpart[:, W_WS:W_WS + 2] = Ws_b.astype(BF).view(np.uint16)
    wpart[:, W_BQM:W_BQM + 4] = bqm_u16
    wpart[:, W_WC:W_WC + 800] = Wc_b.astype(BF).view(np.uint16)

    in_maps2 = []
    for c in range(NC_CORES):
        rows = slice(c * BROWS, (c + 1) * BROWS)
        knn_rows = memory_keys[top_idx[rows]].reshape(NCD, D)
        ktp = np.ascontiguousarray(
            knn_rows.reshape(2, 512, 4, 128).transpose(3, 0, 2, 1)
        ).reshape(128, KT_W).astype(BF).view(np.uint16)
        qTc = np.ascontiguousarray(
            q32[rows].T.reshape(4, 128, BROWS).transpose(1, 0, 2)
        ).reshape(128, 128).astype(BF).view(np.uint16)
        blob = np.zeros((128, BLOB_W), np.uint16)
        blob[:, :KT_W] = ktp
        blob[:, KT_W:] = wpart
        blob[:, KT_W + W_QT:KT_W + W_QT + 128] = qTc
        in_maps2.append({"blob": blob.view(BF)})
    res2 = run_bass_kernel_spmd(ph2, in_maps2, core_ids=list(range(NC_CORES)))

    out = np.zeros((B, C), np.float32)
    for c in range(NC_CORES):
        r = np.asarray(res2.results[c]["out"], np.float32)   # [64, 104]
        y1 = r[:BROWS, :C]
        y2 = r[BROWS:, :C]
        den = r[BROWS:, C]
        out[c * BROWS:(c + 1) * BROWS] = y1 + y2 / den[:, None]
    return (out + np.asarray(bc, np.float32)[None, :]).astype(np.float32)


# revision 32
# speedup vs baseline: 2.3134x; 1.0145x over previous
"""Trainium2 Bass kernel for retrieval-knn attention classifier (nn_MA_51866025067137).

Strategy (8 NeuronCores):
  Phase 1 — memory_keys sharded along N (12800 keys/core, padded 100000->102400
  with dummy rows), fed in bf16 (keys pre-normalized on host so the matmul
  directly yields cosine ranking values; host re-scores candidates in exact
  fp32 afterwards, so ranking precision only has to preserve the top-32 set).
  Per chunk of 512 keys: PE computes sims for all 256 queries (bf16 matmuls,
  fp32 PSUM), ACT evicts both query-halves in one [128,1024] copy, GPSIMD
  packs a 9-bit in-chunk index into the sim mantissa, DVE max8 keeps the
  top-8 per chunk.  Tail: 4 rounds of max8/max_index/match_replace extract
  the per-core top-32 (value, position) per query row.
  Host — merges the 8x32 candidates per row, re-scores them exactly in fp32,
  and gathers the global top-32 key vectors.
  Phase 2 — batch sharded (32 queries/core), all inputs packed into one bf16
  blob (2 logical DMAs): memory-attention module via bf16 matmuls; the
  softmax-score row is transposed with 8 tiny PE transposes (no DRAM bounce);
  attended@Wc is reassociated as sum_k w_k * (knn_k @ Wc2) so the weighted
  sum runs over a precomputed [1024,100] knnWc instead of [1024,512] knn
  (no knn tile, no attT transpose); normalization by sum(e) happens on host.
"""

import numpy as np
import ml_dtypes

import concourse.bacc as bacc
import concourse.mybir as mybir
from concourse.tile import TileContext
from concourse.bass_utils import run_bass_kernel_spmd

# problem dims (hardcoded per harness contract)
B, N, D = 256, 100000, 512
A, C, K = 256, 100, 32
NC_CORES = 8
NPAD = 102400             # 8 * 12800
SHARD = NPAD // NC_CORES  # 12800
CHUNK = 512               # keys per inner loop step
NCHUNK = SHARD // CHUNK   # 25
L1W = NCHUNK * 8          # 200
BROWS = B // NC_CORES     # 32 rows per core in phase 2
KLOC = 16                 # local candidates per core per row
NROUND = KLOC // 8        # 5 extraction rounds
CAND = NC_CORES * KLOC    # 320 merged candidates per row
KSCALE = 16.0             # fp8 range scaling (ranking is scale-invariant)
QSCALE = 32.0

f32 = mybir.dt.float32
f32r = mybir.dt.float32r
bf16 = mybir.dt.bfloat16
fp8 = mybir.dt.float8e4
u32 = mybir.dt.uint32
u16 = mybir.dt.uint16
BF = ml_dtypes.bfloat16
E4 = ml_dtypes.float8_e4m3

# ---- phase-2 blob layout (bf16 columns) ----
NCD = BROWS * K           # 1024
# ktile: [128, 4096], col = half*2048 + dc*512 + i  (bk = half*512 + i)
KT_W = 4096
# wtile: [128, 2984]
W_WM = 0                  # 4 dc x 256
W_WQ = 1024               # 4 dc x 256
W_WS = 2048               # 2 (col at)
W_BQM = 2050              # 4 bf16 cols = [128,2] f32 (byte offset 4100, 4-aligned)
W_QT = 2054               # 4 dc x 32
W_WC = 2182               # 8 m x 100
W_W = 2984                # padded (2982 used)
BLOB_W = KT_W + W_W       # 7080
CP4 = C + 4               # 104: knnWc stride (100 vals, col 100 = 1.0)

_PH1 = None
_PH2 = None


def _build_phase1():
    nc = bacc.Bacc("TRN2", target_bir_lowering=False)
    khatT = nc.dram_tensor("khatT", [NCHUNK, 128, 4 * CHUNK], fp8, kind="ExternalInput")
    qT = nc.dram_tensor("qT", [D, B], fp8, kind="ExternalInput")
    wp_out = nc.dram_tensor("wp", [B, 2 * KLOC], u32, kind="ExternalOutput")

    with TileContext(nc) as tc:
        with (
            tc.tile_pool(name="qpool", bufs=1) as qpool,
            tc.tile_pool(name="keys", bufs=5) as keyp,
            tc.tile_pool(name="pk", bufs=4) as pkp,
            tc.tile_pool(name="l1", bufs=1) as l1p,
            tc.tile_pool(name="small", bufs=1) as smallp,
            tc.tile_pool(name="psum", bufs=3, space="PSUM") as psump,
        ):
            # qT already relu'd, scaled, fp8 on host; one DMA, dc-major
            qTall = qpool.tile([128, 4 * B], fp8, tag="qtall")
            nc.sync.dma_start(out=qTall[:].rearrange("p (dc b) -> p dc b", dc=4),
                              in_=qT[:, :].rearrange("(dc p) b -> p dc b", p=128))
            qT3 = qTall[:].rearrange("p (dc b) -> p dc b", dc=4)  # [128,4,256]

            # static index lanes: each pk buffer's low u16 lanes hold the
            # in-chunk key index (0..511, repeated for both query halves);
            # written once by GPSIMD, reused as buffers rotate. The chunk id
            # is recovered from the winner's L1 position via max_index.
            pks = []
            for b in range(4):
                pk = pkp.tile([128, 2 * CHUNK], f32, tag="pk")
                pks.append(pk)
                nc.gpsimd.iota(
                    pk[:].bitcast(u16)
                        .rearrange("p (b two) -> p b two", two=2)[:, :, 0]
                        .rearrange("p (a b) -> p a b", a=2),
                    pattern=[[0, 2], [1, CHUNK]], base=0,
                    channel_multiplier=0)

            L1 = [l1p.tile([128, L1W], f32, tag=f"l1_{qt}", name=f"l1_{qt}")
                  for qt in range(2)]

            for c in range(NCHUNK):
                kt = keyp.tile([128, 4 * CHUNK], fp8, tag="kt")
                nc.sync.dma_start(out=kt[:], in_=khatT[c, :, :])
                kt3 = kt[:].rearrange("p (dc n) -> p dc n", dc=4)
                ps = psump.tile([128, 2 * CHUNK], f32, tag="sim")
                for qt in range(2):
                    for m in range(2):
                        nc.tensor.matmul(
                            ps[:, qt * CHUNK:(qt + 1) * CHUNK],
                            lhsT=qT3[:, 2 * m:2 * m + 2, qt * 128:(qt + 1) * 128],
                            rhs=kt3[:, 2 * m:2 * m + 2, :],
                            perf_mode=mybir.MatmulPerfMode.DoubleRow,
                            start=(m == 0), stop=(m == 1),
                        )
                # bf16(sim) into the high u16 lanes over the static index lanes
                pk = pks[c % 4]
                nc.scalar.copy(
                    out=pk[:].bitcast(bf16)
                        .rearrange("p (b two) -> p b two", two=2)[:, :, 1],
                    in_=ps[:])
                for qt in range(2):
                    nc.vector.max(out=L1[qt][:, c * 8:(c + 1) * 8],
                                  in_=pk[:, qt * CHUNK:(qt + 1) * CHUNK])

            # extraction: NROUND rounds of top-8 from L1 (200 wide);
            # win values (cols 0..KLOC) and L1 positions (cols KLOC..2K)
            for qt in range(2):
                wp = smallp.tile([128, 2 * KLOC], u32, tag=f"wp{qt}")
                for r in range(NROUND):
                    w8 = wp[:, r * 8:(r + 1) * 8].bitcast(f32)
                    nc.vector.max(out=w8, in_=L1[qt][:])
                    nc.vector.max_index(out=wp[:, KLOC + r * 8:KLOC + (r + 1) * 8],
                                        in_max=w8, in_values=L1[qt][:])
                    if r < NROUND - 1:
                        nc.vector.match_replace(out=L1[qt][:], in_to_replace=w8,
                                                in_values=L1[qt][:],
                                                imm_value=-3.0e38)
                nc.sync.dma_start(out=wp_out[qt * 128:(qt + 1) * 128, :], in_=wp[:])
    nc.finalize()
    return nc


def _build_phase2():
    nc = bacc.Bacc("TRN2", target_bir_lowering=False)
    blob = nc.dram_tensor("blob", [128, BLOB_W], bf16, kind="ExternalInput")
    out_d = nc.dram_tensor("out", [2 * BROWS, CP4], f32, kind="ExternalOutput")

    with TileContext(nc) as tc:
        with (
            tc.tile_pool(name="big", bufs=1) as bigp,
            tc.tile_pool(name="small", bufs=1) as smallp,
            tc.tile_pool(name="pskp", bufs=3, space="PSUM") as pskp,
            tc.tile_pool(name="pssc", bufs=1, space="PSUM") as pssc,
            tc.tile_pool(name="psmi", bufs=2, space="PSUM") as psmi,
            tc.tile_pool(name="psy", bufs=1, space="PSUM") as psy,
        ):
            # ---- DMAs: Wm+Wq, knnT h0 quarters, small consts, h1, Wc ----
            wt = bigp.tile([128, W_W], bf16, tag="wt")
            kt = bigp.tile([128, KT_W], bf16, tag="ktile")
            nc.sync.dma_start(out=wt[:, W_WS:W_WC], in_=blob[:, KT_W + W_WS:KT_W + W_WC])
            nc.sync.dma_start(out=wt[:, :W_WS], in_=blob[:, KT_W:KT_W + W_WS])
            for q in range(4):
                nc.sync.dma_start(out=kt[:, q * 1024:(q + 1) * 1024],
                                  in_=blob[:, q * 1024:(q + 1) * 1024])
            nc.sync.dma_start(out=wt[:, W_WC:W_WC + 800],
                              in_=blob[:, KT_W + W_WC:KT_W + W_WC + 800])

            Wm = [wt[:, W_WM + dc * A:W_WM + (dc + 1) * A] for dc in range(4)]
            Wq = [wt[:, W_WQ + dc * A:W_WQ + (dc + 1) * A] for dc in range(4)]
            Ws = [wt[:, W_WS + at:W_WS + at + 1] for at in range(2)]
            bqm = wt[:, W_BQM:W_BQM + 4].bitcast(f32)            # [128, 2]
            qT = [wt[:, W_QT + dc * BROWS:W_QT + (dc + 1) * BROWS] for dc in range(4)]
            Wc = [wt[:, W_WC + m * C:W_WC + (m + 1) * C] for m in range(8)]

            # ---- PE p-state warmup (bridges the DMA lead-in) ----
            warm = smallp.tile([128, 512], bf16, tag="warm")
            nc.vector.memset(warm[:], 0.0)
            warm_ps = psmi.tile([128, 2 * CP4], f32, tag="mi")
            for i in range(12):
                nc.tensor.matmul(warm_ps[:, :C], lhsT=warm[:, :128],
                                 rhs=warm[:, :C], start=True, stop=True)

            # ---- small constants ----
            ident1 = smallp.tile([1, 1], f32, tag="id1")
            nc.vector.memset(ident1[:], 1.0)
            # mask4[p, j] = 1.0 iff j == p // 32
            mask4 = smallp.tile([128, 4], f32, tag="mask4")
            nc.vector.memset(mask4[:], 1.0)
            nc.gpsimd.affine_select(out=mask4[:], in_=mask4[:],
                                    compare_op=mybir.AluOpType.is_ge, fill=0.0,
                                    base=0, pattern=[[-32, 4]], channel_multiplier=1)
            nc.gpsimd.affine_select(out=mask4[:], in_=mask4[:],
                                    compare_op=mybir.AluOpType.is_ge, fill=0.0,
                                    base=31, pattern=[[32, 4]], channel_multiplier=-1)
            w2 = [smallp.tile([128, BROWS], bf16, tag=f"w2_{t}", name=f"w2t{t}")
                  for t in range(8)]
            for t in range(8):
                nc.vector.memset(w2[t][:], 0.0)
            kwS = smallp.tile([128, 8 * CP4], bf16, tag="kwS")   # knn@Wc2 (+ones col)
            for t in range(8):
                nc.vector.memset(kwS[:, t * CP4 + C:t * CP4 + C + 1], 1.0)

            # ---- qprojT [128(a), 2*32] ----
            qp_ps = psmi.tile([128, 2 * CP4], f32, tag="mi")
            for at in range(2):
                for dc in range(4):
                    nc.tensor.matmul(
                        qp_ps[:, at * BROWS:(at + 1) * BROWS],
                        lhsT=Wq[dc][:, at * 128:(at + 1) * 128], rhs=qT[dc],
                        start=(dc == 0), stop=(dc == 3))
            qprojT = smallp.tile([128, 2 * BROWS], f32, tag="qprojT")
            nc.scalar.copy(out=qprojT[:], in_=qp_ps[:, :2 * BROWS])

            # ---- y1 = relu(q) @ Wc1, shipped early (partitions 32..63) ----
            yy_ps = psy.tile([128, CP4], f32, tag="yy")
            for dc in range(4):
                nc.tensor.matmul(yy_ps[BROWS:2 * BROWS, :C], lhsT=qT[dc],
                                 rhs=Wc[dc], start=(dc == 0), stop=(dc == 3))
            osb = smallp.tile([2 * BROWS, CP4], f32, tag="osb")
            nc.scalar.copy(out=osb[:BROWS, :C], in_=yy_ps[BROWS:2 * BROWS, :C])
            nc.sync.dma_start(out=out_d[:BROWS, :], in_=osb[:BROWS, :])

            # ---- kprojT + h = tanh(. + qproj + bqm) -> scores row ----
            kwcopies = []
            sc_ps = pssc.tile([128, NCD], f32, tag="sc")
            hT = [bigp.tile([128, NCD], bf16, tag=f"hT{at}", name=f"hTt{at}")
                  for at in range(2)]
            kph = {}
            for half in range(2):
                for at in range(2):
                    kp = pskp.tile([128, 512], f32, tag="kp")
                    kph[at, half] = kp
                    for dc in range(4):
                        nc.tensor.matmul(
                            kp[:],
                            lhsT=Wm[dc][:, at * 128:(at + 1) * 128],
                            rhs=kt[:, half * 2048 + dc * 512:half * 2048 + (dc + 1) * 512],
                            start=(dc == 0), stop=(dc == 3))

            # ---- knnWc[2t,2t+1] pairs (PE work ahead of the scores chain) ----
            for tp in range(4):
                kw_ps = psmi.tile([128, 2 * CP4], f32, tag="mi")
                for tt in range(2):
                    t = 2 * tp + tt
                    half, blk = t // 4, t % 4
                    for dc in range(4):
                        nc.tensor.matmul(
                            kw_ps[:, tt * CP4:tt * CP4 + C],
                            lhsT=kt[:, half * 2048 + dc * 512 + blk * 128:
                                    half * 2048 + dc * 512 + (blk + 1) * 128],
                            rhs=Wc[4 + dc],
                            start=(dc == 0), stop=(dc == 3))
                kwcopies.append((tp, kw_ps))

            for half in range(2):
                for at in range(2):
                    cols = slice(half * 512, (half + 1) * 512)
                    qb = qprojT[:, at * BROWS + half * 16:at * BROWS + half * 16 + 16,
                                None].to_broadcast([128, 16, K])
                    nc.vector.tensor_tensor(
                        hT[at][:, cols].rearrange("p (b k) -> p b k", k=K),
                        kph[at, half][:].rearrange("p (b k) -> p b k", k=K),
                        qb, mybir.AluOpType.add)
                    nc.scalar.activation(hT[at][:, cols], hT[at][:, cols],
                                         mybir.ActivationFunctionType.Tanh,
                                         bias=bqm[:, at:at + 1])
                for at in range(2):
                    nc.tensor.matmul(
                        sc_ps[:1, half * 512:(half + 1) * 512],
                        lhsT=Ws[at],
                        rhs=hT[at][:, half * 512:(half + 1) * 512],
                        start=(at == 0), stop=(at == 1))

            # ---- evict knnWc pairs to SBUF (DVE), between adds and muls ----
            for tp, kw_ps in kwcopies:
                nc.vector.tensor_copy(
                    kwS[:].rearrange("p (t c) -> p t c", t=8)[:, 2 * tp:2 * tp + 2, :C],
                    kw_ps[:].rearrange("p (two c) -> p two c", two=2)[:, :, :C])

            # ---- e row (exp per half) -> e_col via PE transposes; weights ----
            e_row = smallp.tile([1, NCD], f32, tag="e_row")
            ecT_ps = psmi.tile([128, 2 * CP4], f32, tag="mi")
            for half in range(2):
                nc.scalar.activation(e_row[:, half * 512:(half + 1) * 512],
                                     sc_ps[:1, half * 512:(half + 1) * 512],
                                     mybir.ActivationFunctionType.Exp)
                for tt in range(4):
                    t = half * 4 + tt
                    nc.tensor.transpose(ecT_ps[:, t:t + 1],
                                        e_row[:, t * 128:(t + 1) * 128], ident1[:])
                for tt in range(4):
                    t = half * 4 + tt
                    nc.vector.tensor_scalar_mul(w2[t][:, 4 * t:4 * t + 4],
                                                mask4[:], ecT_ps[:, t:t + 1])

            # ---- y2[b,:] = sum_k e * knnWc ; col C = sum_k e (den) ----
            for t in range(8):
                nc.tensor.matmul(yy_ps[:BROWS, :C + 1], lhsT=w2[t][:],
                                 rhs=kwS[:, t * CP4:t * CP4 + C + 1],
                                 start=(t == 0), stop=(t == 7))
            nc.scalar.copy(out=osb[BROWS:, :C + 1], in_=yy_ps[:BROWS, :C + 1])
            nc.sync.dma_start(out=out_d[BROWS:, :], in_=osb[BROWS:, :])
    nc.finalize()
    return nc


def _phase1_nc():
    global _PH1
    if _PH1 is None:
        _PH1 = _build_phase1()
    return _PH1


def _phase2_nc():
    global _PH2
    if _PH2 is None:
        _PH2 = _build_phase2()
    return _PH2


def kernel(query_feat, memory_keys, Wq, bq, Wm, bm, Ws, bs, Wc, bc):
    query_feat = np.asarray(query_feat, np.float32)
    memory_keys = np.asarray(memory_keys, np.float32)

    # ---- host prep: pad + normalize + transpose + shard keys (bf16) ----
    kn = np.sqrt((memory_keys ** 2).sum(axis=1))
    khat = memory_keys * (KSCALE / kn)[:, None]
    pad = np.full((NPAD - N, D), -KSCALE / np.sqrt(D), np.float32)
    khat_pad = np.concatenate([khat.astype(np.float32), pad], axis=0)
    q32 = np.maximum(query_feat, 0)
    qT_full = np.ascontiguousarray((q32.T * QSCALE).astype(E4))  # [512, 256]

    ph1 = _phase1_nc()
    in_maps = []
    for c in range(NC_CORES):
        sh = khat_pad[c * SHARD:(c + 1) * SHARD]          # [12800, 512]
        arr = np.ascontiguousarray(
            sh.reshape(NCHUNK, CHUNK, 4, 128).transpose(0, 3, 2, 1).astype(E4)
        ).reshape(NCHUNK, 128, 4 * CHUNK)
        in_maps.append({"khatT": arr, "qT": qT_full})
    res1 = run_bass_kernel_spmd(ph1, in_maps, core_ids=list(range(NC_CORES)))

    # ---- host merge: recover indices, exact re-score of candidates ----
    all_gidx = np.zeros((B, NC_CORES, KLOC), np.int64)
    for c in range(NC_CORES):
        wp = np.asarray(res1.results[c]["wp"]).view(np.uint32)
        win, pos = wp[:, :KLOC], wp[:, KLOC:].astype(np.int64)
        within = (win & np.uint32(0xFFFF)).astype(np.int64)  # in-chunk index
        all_gidx[:, c, :] = (pos // 8) * CHUNK + within + c * SHARD
    gidx = all_gidx.reshape(B, CAND)
    safe = np.minimum(gidx, N - 1)
    cand_keys = memory_keys[safe]                       # [256, 256, 512]
    dots = np.einsum("bd,bcd->bc", q32, cand_keys, optimize=True)
    cos = dots / np.maximum(
        np.linalg.norm(q32, axis=1)[:, None] * kn[safe], np.float32(1e-8))
    cos[gidx >= N] = -np.inf                            # mask dummy-pad hits
    order = np.argsort(-cos, axis=1, kind="stable")[:, :K]
    top_idx = np.take_along_axis(safe, order, axis=1)   # [256, 32]

    # ---- phase 2 (batch sharded): pack one bf16 blob per core ----
    ph2 = _phase2_nc()
    bqm_f = (np.asarray(bq, np.float32) + np.asarray(bm, np.float32))
    Wm_b = np.asarray(Wm, np.float32).reshape(4, 128, A).transpose(1, 0, 2).reshape(128, 1024)
    Wq_b = np.asarray(Wq, np.float32).reshape(4, 128, A).transpose(1, 0, 2).reshape(128, 1024)
    Ws_b = np.asarray(Ws, np.float32)[:, 0].reshape(2, 128).T         # [128, 2]
    Wc_b = np.asarray(Wc, np.float32).reshape(8, 128, C).transpose(1, 0, 2).reshape(128, 800)
    bqm_u16 = np.ascontiguousarray(
        bqm_f.reshape(2, 128).T.astype(np.float32)).view(np.uint16)   # [128, 4]

    wpart = np.zeros((128, W_W), np.uint16)
    wpart[:, W_WM:W_WM + 1024] = Wm_b.astype(BF).view(np.uint16)
    wpart[:, W_WQ:W_WQ + 1024] = Wq_b.astype(BF).view(np.uint16)
    wpart[:, W_WS:W_WS + 2] = Ws_b.astype(BF).view(np.uint16)
    wpart[:, W_BQM:W_BQM + 4] = bqm_u16
    wpart[:, W_WC:W_WC + 800] = Wc_b.astype(BF).view(np.uint16)

    in_maps2 = []
    for c in range(NC_CORES):
        rows = slice(c * BROWS, (c + 1) * BROWS)
        knn_rows = memory_keys[top_idx[rows]].reshape(NCD, D)
        ktp = np.ascontiguousarray(
            knn_rows.reshape(2, 512, 4, 128).transpose(3, 0, 2, 1)
        ).reshape(128, KT_W).astype(BF).view(np.uint16)
        qTc = np.ascontiguousarray(
            q32[rows].T.reshape(4, 128, BROWS).transpose(1, 0, 2)
        ).reshape(128, 128).astype(BF).view(np.uint16)
        blob = np.zeros((128, BLOB_W), np.uint16)
        blob[:, :KT_W] = ktp
        blob[:, KT_W:] = wpart
        blob[:, KT_W + W_QT:KT_W + W_QT + 128] = qTc
        in_maps2.append({"blob": blob.view(BF)})
    res2 = run_bass_kernel_spmd(ph2, in_maps2, core_ids=list(range(NC_CORES)))

    out = np.zeros((B, C), np.float32)
    for c in range(NC_CORES):
        r = np.asarray(res2.results[c]["out"], np.float32)   # [64, 104]
        y1 = r[:BROWS, :C]
        y2 = r[BROWS:, :C]
        den = r[BROWS:, C]
        out[c * BROWS:(c + 1) * BROWS] = y1 + y2 / den[:, None]
    return (out + np.asarray(bc, np.float32)[None, :]).astype(np.float32)


# revision 36
# speedup vs baseline: 2.3545x; 1.0178x over previous
"""Trainium2 Bass kernel for retrieval-knn attention classifier (nn_MA_51866025067137).

Strategy (8 NeuronCores):
  Phase 1 — memory_keys sharded along N (12800 keys/core, padded 100000->102400
  with dummy rows), fed in bf16 (keys pre-normalized on host so the matmul
  directly yields cosine ranking values; host re-scores candidates in exact
  fp32 afterwards, so ranking precision only has to preserve the top-32 set).
  Per chunk of 512 keys: PE computes sims for all 256 queries (bf16 matmuls,
  fp32 PSUM), ACT evicts both query-halves in one [128,1024] copy, GPSIMD
  packs a 9-bit in-chunk index into the sim mantissa, DVE max8 keeps the
  top-8 per chunk.  Tail: 4 rounds of max8/max_index/match_replace extract
  the per-core top-32 (value, position) per query row.
  Host — merges the 8x32 candidates per row, re-scores them exactly in fp32,
  and gathers the global top-32 key vectors.
  Phase 2 — batch sharded (32 queries/core), all inputs packed into one bf16
  blob (2 logical DMAs): memory-attention module via bf16 matmuls; the
  softmax-score row is transposed with 8 tiny PE transposes (no DRAM bounce);
  attended@Wc is reassociated as sum_k w_k * (knn_k @ Wc2) so the weighted
  sum runs over a precomputed [1024,100] knnWc instead of [1024,512] knn
  (no knn tile, no attT transpose); normalization by sum(e) happens on host.
"""

import numpy as np
import ml_dtypes

import concourse.bacc as bacc
import concourse.mybir as mybir
from concourse.tile import TileContext
from concourse.bass_utils import run_bass_kernel_spmd

# problem dims (hardcoded per harness contract)
B, N, D = 256, 100000, 512
A, C, K = 256, 100, 32
NC_CORES = 8
NPAD = 102400             # 8 * 12800
SHARD = NPAD // NC_CORES  # 12800
CHUNK = 512               # keys per inner loop step
NCHUNK = SHARD // CHUNK   # 25
L1W = NCHUNK * 8          # 200
BROWS = B // NC_CORES     # 32 rows per core in phase 2
KLOC = 16                 # local candidates per core per row
NROUND = KLOC // 8        # 5 extraction rounds
CAND = NC_CORES * KLOC    # 320 merged candidates per row
KSCALE = 16.0             # fp8 range scaling (ranking is scale-invariant)
QSCALE = 32.0

f32 = mybir.dt.float32
f32r = mybir.dt.float32r
bf16 = mybir.dt.bfloat16
fp8 = mybir.dt.float8e4
u32 = mybir.dt.uint32
u16 = mybir.dt.uint16
BF = ml_dtypes.bfloat16
E4 = ml_dtypes.float8_e4m3

# ---- phase-2 blob layout (bf16 columns; fp8 regions bitcast) ----
NCD = BROWS * K           # 1024
P2_KT = 0                 # knnT fp8: 2048 bf16 cols (fp8 col = half*2048+dc*512+i)
P2_CONST = 2048           # Ws 2 | bqm 4 (f32) | qT 4dc x 32
P2_W8 = 2182              # Wm fp8 512 + Wq fp8 512 (scaled x64)
P2_MASK = 3206            # block-diag softmax mask, 256
P2_WC = 3462              # Wc bf16, 8 m x 100
BLOB_W = 4262
WSCALE = 64.0             # fp8 range scaling for Wm/Wq
CP4 = C + 4               # 104: knnWc stride (100 vals, col 100 = 1.0)

_PH1 = None
_PH2 = None


def _build_phase1():
    nc = bacc.Bacc("TRN2", target_bir_lowering=False)
    khatT = nc.dram_tensor("khatT", [NCHUNK, 128, 4 * CHUNK], fp8, kind="ExternalInput")
    qT = nc.dram_tensor("qT", [D, B], fp8, kind="ExternalInput")
    wp_out = nc.dram_tensor("wp", [B, 2 * KLOC], u32, kind="ExternalOutput")

    with TileContext(nc) as tc:
        with (
            tc.tile_pool(name="qpool", bufs=1) as qpool,
            tc.tile_pool(name="keys", bufs=5) as keyp,
            tc.tile_pool(name="pk", bufs=4) as pkp,
            tc.tile_pool(name="l1", bufs=1) as l1p,
            tc.tile_pool(name="small", bufs=1) as smallp,
            tc.tile_pool(name="psum", bufs=3, space="PSUM") as psump,
        ):
            # qT already relu'd, scaled, fp8 on host; one DMA, dc-major
            qTall = qpool.tile([128, 4 * B], fp8, tag="qtall")
            nc.sync.dma_start(out=qTall[:].rearrange("p (dc b) -> p dc b", dc=4),
                              in_=qT[:, :].rearrange("(dc p) b -> p dc b", p=128))
            qT3 = qTall[:].rearrange("p (dc b) -> p dc b", dc=4)  # [128,4,256]

            # static index lanes: each pk buffer's low u16 lanes hold the
            # in-chunk key index (0..511, repeated for both query halves);
            # written once by GPSIMD, reused as buffers rotate. The chunk id
            # is recovered from the winner's L1 position via max_index.
            pks = []
            for b in range(4):
                pk = pkp.tile([128, 2 * CHUNK], f32, tag="pk")
                pks.append(pk)
                nc.gpsimd.iota(
                    pk[:].bitcast(u16)
                        .rearrange("p (b two) -> p b two", two=2)[:, :, 0]
                        .rearrange("p (a b) -> p a b", a=2),
                    pattern=[[0, 2], [1, CHUNK]], base=0,
                    channel_multiplier=0)

            L1 = [l1p.tile([128, L1W], f32, tag=f"l1_{qt}", name=f"l1_{qt}")
                  for qt in range(2)]

            for c in range(NCHUNK):
                kt = keyp.tile([128, 4 * CHUNK], fp8, tag="kt")
                nc.sync.dma_start(out=kt[:], in_=khatT[c, :, :])
                kt3 = kt[:].rearrange("p (dc n) -> p dc n", dc=4)
                ps = psump.tile([128, 2 * CHUNK], f32, tag="sim")
                for qt in range(2):
                    for m in range(2):
                        nc.tensor.matmul(
                            ps[:, qt * CHUNK:(qt + 1) * CHUNK],
                            lhsT=qT3[:, 2 * m:2 * m + 2, qt * 128:(qt + 1) * 128],
                            rhs=kt3[:, 2 * m:2 * m + 2, :],
                            perf_mode=mybir.MatmulPerfMode.DoubleRow,
                            start=(m == 0), stop=(m == 1),
                        )
                # bf16(sim) into the high u16 lanes over the static index lanes
                pk = pks[c % 4]
                nc.scalar.copy(
                    out=pk[:].bitcast(bf16)
                        .rearrange("p (b two) -> p b two", two=2)[:, :, 1],
                    in_=ps[:])
                for qt in range(2):
                    nc.vector.max(out=L1[qt][:, c * 8:(c + 1) * 8],
                                  in_=pk[:, qt * CHUNK:(qt + 1) * CHUNK])

            # extraction: NROUND rounds of top-8 from L1 (200 wide);
            # win values (cols 0..KLOC) and L1 positions (cols KLOC..2K)
            for qt in range(2):
                wp = smallp.tile([128, 2 * KLOC], u32, tag=f"wp{qt}")
                for r in range(NROUND):
                    w8 = wp[:, r * 8:(r + 1) * 8].bitcast(f32)
                    nc.vector.max(out=w8, in_=L1[qt][:])
                    nc.vector.max_index(out=wp[:, KLOC + r * 8:KLOC + (r + 1) * 8],
                                        in_max=w8, in_values=L1[qt][:])
                    if r < NROUND - 1:
                        nc.vector.match_replace(out=L1[qt][:], in_to_replace=w8,
                                                in_values=L1[qt][:],
                                                imm_value=-3.0e38)
                nc.sync.dma_start(out=wp_out[qt * 128:(qt + 1) * 128, :], in_=wp[:])
    nc.finalize()
    return nc


def _build_phase2():
    nc = bacc.Bacc("TRN2", target_bir_lowering=False)
    blob = nc.dram_tensor("blob", [128, BLOB_W], bf16, kind="ExternalInput")
    out_d = nc.dram_tensor("out", [2 * BROWS, CP4], f32, kind="ExternalOutput")

    with TileContext(nc) as tc:
        with (
            tc.tile_pool(name="big", bufs=1) as bigp,
            tc.tile_pool(name="small", bufs=1) as smallp,
            tc.tile_pool(name="pskp", bufs=3, space="PSUM") as pskp,
            tc.tile_pool(name="pssc", bufs=1, space="PSUM") as pssc,
            tc.tile_pool(name="psmi", bufs=2, space="PSUM") as psmi,
            tc.tile_pool(name="psy", bufs=1, space="PSUM") as psy,
        ):
            # ---- DMAs: consts, Wm/Wq fp8, knnT fp8 quarters, mask+Wc ----
            cst = smallp.tile([128, 134], bf16, tag="cst")
            nc.sync.dma_start(out=cst[:], in_=blob[:, P2_CONST:P2_CONST + 134])
            wf = bigp.tile([128, 2048], fp8, tag="wf")
            nc.sync.dma_start(out=wf[:], in_=blob[:, P2_W8:P2_W8 + 1024].bitcast(fp8))
            kt = bigp.tile([128, 4096], fp8, tag="ktile")
            for q in range(4):
                nc.sync.dma_start(out=kt[:, q * 1024:(q + 1) * 1024],
                                  in_=blob[:, q * 512:(q + 1) * 512].bitcast(fp8))
            mw = bigp.tile([128, 1056], bf16, tag="mw")
            nc.sync.dma_start(out=mw[:], in_=blob[:, P2_MASK:P2_MASK + 1056])

            Wm8 = [wf[:, dc * A:(dc + 1) * A] for dc in range(4)]
            Wq8 = [wf[:, 1024 + dc * A:1024 + (dc + 1) * A] for dc in range(4)]
            Ws = [cst[:, at:at + 1] for at in range(2)]
            bqm = cst[:, 2:6].bitcast(f32)                       # [128, 2]
            qT = [cst[:, 6 + dc * BROWS:6 + (dc + 1) * BROWS] for dc in range(4)]
            mask256 = mw[:, :256]
            Wc = [mw[:, 256 + m * C:256 + (m + 1) * C] for m in range(8)]

            # ---- PE p-state warmup (bridges the DMA lead-in) ----
            warm = smallp.tile([128, 512], bf16, tag="warm")
            nc.vector.memset(warm[:], 0.0)
            warm_ps = psmi.tile([128, 2 * CP4], f32, tag="mi")
            for i in range(8):
                nc.tensor.matmul(warm_ps[:, :C], lhsT=warm[:, :128],
                                 rhs=warm[:, :C], start=True, stop=True)

            # ---- small constants ----
            ident1 = smallp.tile([1, 1], f32, tag="id1")
            nc.vector.memset(ident1[:], 1.0)
            w2big = smallp.tile([128, 256], bf16, tag="w2big")
            kwS = smallp.tile([128, 8 * CP4], bf16, tag="kwS")   # knn@Wc2 (+ones col)
            for t in range(8):
                nc.vector.memset(kwS[:, t * CP4 + C:t * CP4 + C + 1], 1.0)

            # ---- qprojT [128(a), 2*32] (unscale by 1/WSCALE on evict) ----
            qp_ps = psmi.tile([128, 2 * CP4], f32, tag="mi")
            for at in range(2):
                for dc in range(4):
                    nc.tensor.matmul(
                        qp_ps[:, at * BROWS:(at + 1) * BROWS],
                        lhsT=Wq8[dc][:, at * 128:(at + 1) * 128], rhs=qT[dc],
                        start=(dc == 0), stop=(dc == 3))
            qprojT = smallp.tile([128, 2 * BROWS], f32, tag="qprojT")
            nc.scalar.activation(qprojT[:], qp_ps[:, :2 * BROWS],
                                 mybir.ActivationFunctionType.Copy,
                                 scale=1.0 / WSCALE)

            # ---- y1 = relu(q) @ Wc1, shipped early (partitions 32..63) ----
            yy_ps = psy.tile([128, CP4], f32, tag="yy")
            for dc in range(4):
                nc.tensor.matmul(yy_ps[BROWS:2 * BROWS, :C], lhsT=qT[dc],
                                 rhs=Wc[dc], start=(dc == 0), stop=(dc == 3))
            osb = smallp.tile([2 * BROWS, CP4], f32, tag="osb")
            nc.scalar.copy(out=osb[:BROWS, :C], in_=yy_ps[BROWS:2 * BROWS, :C])
            nc.sync.dma_start(out=out_d[:BROWS, :], in_=osb[:BROWS, :])

            # ---- kprojT (fp8) + h = tanh(kp/WSCALE + qproj + bqm) ----
            kwcopies = []
            sc_ps = pssc.tile([128, NCD], f32, tag="sc")
            hT = [bigp.tile([128, NCD], bf16, tag=f"hT{at}", name=f"hTt{at}")
                  for at in range(2)]
            kph = {}
            for half in range(2):
                for at in range(2):
                    kp = pskp.tile([128, 512], f32, tag="kp")
                    kph[at, half] = kp
                    for dc in range(4):
                        nc.tensor.matmul(
                            kp[:],
                            lhsT=Wm8[dc][:, at * 128:(at + 1) * 128],
                            rhs=kt[:, half * 2048 + dc * 512:half * 2048 + (dc + 1) * 512],
                            start=(dc == 0), stop=(dc == 3))

            # ---- knnWc[2t,2t+1] pairs (PE work ahead of the scores chain) ----
            for tp in range(4):
                kw_ps = psmi.tile([128, 2 * CP4], f32, tag="mi")
                for tt in range(2):
                    t = 2 * tp + tt
                    half, blk = t // 4, t % 4
                    for dc in range(4):
                        nc.tensor.matmul(
                            kw_ps[:, tt * CP4:tt * CP4 + C],
                            lhsT=kt[:, half * 2048 + dc * 512 + blk * 128:
                                    half * 2048 + dc * 512 + (blk + 1) * 128],
                            rhs=Wc[4 + dc],
                            start=(dc == 0), stop=(dc == 3))
                kwcopies.append((tp, kw_ps))

            for half in range(2):
                for at in range(2):
                    cols = slice(half * 512, (half + 1) * 512)
                    qb = qprojT[:, at * BROWS + half * 16:at * BROWS + half * 16 + 16,
                                None].to_broadcast([128, 16, K])
                    nc.vector.scalar_tensor_tensor(
                        out=hT[at][:, cols].rearrange("p (b k) -> p b k", k=K),
                        in0=kph[at, half][:].rearrange("p (b k) -> p b k", k=K),
                        scalar=1.0 / WSCALE, in1=qb,
                        op0=mybir.AluOpType.mult, op1=mybir.AluOpType.add)
                    nc.scalar.activation(hT[at][:, cols], hT[at][:, cols],
                                         mybir.ActivationFunctionType.Tanh,
                                         bias=bqm[:, at:at + 1])
                for at in range(2):
                    nc.tensor.matmul(
                        sc_ps[:1, half * 512:(half + 1) * 512],
                        lhsT=Ws[at],
                        rhs=hT[at][:, half * 512:(half + 1) * 512],
                        start=(at == 0), stop=(at == 1))

            # ---- evict knnWc pairs to SBUF (DVE), between adds and muls ----
            for tp, kw_ps in kwcopies:
                nc.vector.tensor_copy(
                    kwS[:].rearrange("p (t c) -> p t c", t=8)[:, 2 * tp:2 * tp + 2, :C],
                    kw_ps[:].rearrange("p (two c) -> p two c", two=2)[:, :, :C])

            # ---- e row (exp per half) -> eT -> block-diag weights ----
            e_row = smallp.tile([1, NCD], f32, tag="e_row")
            ecT_ps = psmi.tile([128, 2 * CP4], f32, tag="mi")
            for half in range(2):
                nc.scalar.activation(e_row[:, half * 512:(half + 1) * 512],
                                     sc_ps[:1, half * 512:(half + 1) * 512],
                                     mybir.ActivationFunctionType.Exp)
                for tt in range(4):
                    t = half * 4 + tt
                    nc.tensor.transpose(ecT_ps[:, t:t + 1],
                                        e_row[:, t * 128:(t + 1) * 128], ident1[:])
                nc.vector.tensor_tensor(
                    w2big[:, half * 128:(half + 1) * 128]
                        .rearrange("p (t b) -> p t b", t=4),
                    mask256[:, half * 128:(half + 1) * 128]
                        .rearrange("p (t b) -> p t b", t=4),
                    ecT_ps[:, half * 4:half * 4 + 4, None].to_broadcast([128, 4, 32]),
                    mybir.AluOpType.mult)

            # ---- y2[b,:] = sum_k e * knnWc ; col C = sum_k e (den) ----
            for t in range(8):
                nc.tensor.matmul(yy_ps[:BROWS, :C + 1],
                                 lhsT=w2big[:, 32 * t:32 * t + 32],
                                 rhs=kwS[:, t * CP4:t * CP4 + C + 1],
                                 start=(t == 0), stop=(t == 7))
            nc.scalar.copy(out=osb[BROWS:, :C + 1], in_=yy_ps[:BROWS, :C + 1])
            nc.sync.dma_start(out=out_d[BROWS:, :], in_=osb[BROWS:, :])
    nc.finalize()
    return nc


def _phase1_nc():
    global _PH1
    if _PH1 is None:
        _PH1 = _build_phase1()
    return _PH1


def _phase2_nc():
    global _PH2
    if _PH2 is None:
        _PH2 = _build_phase2()
    return _PH2


def kernel(query_feat, memory_keys, Wq, bq, Wm, bm, Ws, bs, Wc, bc):
    query_feat = np.asarray(query_feat, np.float32)
    memory_keys = np.asarray(memory_keys, np.float32)

    # ---- host prep: pad + normalize + transpose + shard keys (bf16) ----
    kn = np.sqrt((memory_keys ** 2).sum(axis=1))
    khat = memory_keys * (KSCALE / kn)[:, None]
    pad = np.full((NPAD - N, D), -KSCALE / np.sqrt(D), np.float32)
    khat_pad = np.concatenate([khat.astype(np.float32), pad], axis=0)
    q32 = np.maximum(query_feat, 0)
    qT_full = np.ascontiguousarray((q32.T * QSCALE).astype(E4))  # [512, 256]

    ph1 = _phase1_nc()
    in_maps = []
    for c in range(NC_CORES):
        sh = khat_pad[c * SHARD:(c + 1) * SHARD]          # [12800, 512]
        arr = np.ascontiguousarray(
            sh.reshape(NCHUNK, CHUNK, 4, 128).transpose(0, 3, 2, 1).astype(E4)
        ).reshape(NCHUNK, 128, 4 * CHUNK)
        in_maps.append({"khatT": arr, "qT": qT_full})
    res1 = run_bass_kernel_spmd(ph1, in_maps, core_ids=list(range(NC_CORES)))

    # ---- host merge: recover indices, exact re-score of candidates ----
    all_gidx = np.zeros((B, NC_CORES, KLOC), np.int64)
    for c in range(NC_CORES):
        wp = np.asarray(res1.results[c]["wp"]).view(np.uint32)
        win, pos = wp[:, :KLOC], wp[:, KLOC:].astype(np.int64)
        within = (win & np.uint32(0xFFFF)).astype(np.int64)  # in-chunk index
        all_gidx[:, c, :] = (pos // 8) * CHUNK + within + c * SHARD
    gidx = all_gidx.reshape(B, CAND)
    safe = np.minimum(gidx, N - 1)
    cand_keys = memory_keys[safe]                       # [256, 256, 512]
    dots = np.einsum("bd,bcd->bc", q32, cand_keys, optimize=True)
    cos = dots / np.maximum(
        np.linalg.norm(q32, axis=1)[:, None] * kn[safe], np.float32(1e-8))
    cos[gidx >= N] = -np.inf                            # mask dummy-pad hits
    order = np.argsort(-cos, axis=1, kind="stable")[:, :K]
    top_idx = np.take_along_axis(safe, order, axis=1)   # [256, 32]

    # ---- phase 2 (batch sharded): pack one blob per core ----
    ph2 = _phase2_nc()
    bqm_f = (np.asarray(bq, np.float32) + np.asarray(bm, np.float32))
    Wm_8 = (np.asarray(Wm, np.float32).reshape(4, 128, A).transpose(1, 0, 2)
            .reshape(128, 1024) * WSCALE).astype(E4)
    Wq_8 = (np.asarray(Wq, np.float32).reshape(4, 128, A).transpose(1, 0, 2)
            .reshape(128, 1024) * WSCALE).astype(E4)
    Ws_b = np.asarray(Ws, np.float32)[:, 0].reshape(2, 128).T         # [128, 2]
    Wc_b = np.asarray(Wc, np.float32).reshape(8, 128, C).transpose(1, 0, 2).reshape(128, 800)
    bqm_u16 = np.ascontiguousarray(
        bqm_f.reshape(2, 128).T.astype(np.float32)).view(np.uint16)   # [128, 4]
    # mask256[p, 32t+b] = 1 iff b//4 == t and p//32 == b%4
    gg = np.arange(256)
    pp = np.arange(128)[:, None]
    m256 = (((gg % 32) // 4 == gg // 32) & (pp // 32 == gg % 4)).astype(np.float32)

    wpart = np.zeros((128, BLOB_W - P2_CONST), np.uint16)
    wpart[:, 0:2] = Ws_b.astype(BF).view(np.uint16)
    wpart[:, 2:6] = bqm_u16
    w8 = np.concatenate([np.asarray(Wm_8), np.asarray(Wq_8)], axis=1)  # [128,2048] fp8
    wpart[:, P2_W8 - P2_CONST:P2_W8 - P2_CONST + 1024] = \
        np.ascontiguousarray(w8).view(np.uint8).reshape(128, 2048).view(np.uint16)
    wpart[:, P2_MASK - P2_CONST:P2_MASK - P2_CONST + 256] = m256.astype(BF).view(np.uint16)
    wpart[:, P2_WC - P2_CONST:P2_WC - P2_CONST + 800] = Wc_b.astype(BF).view(np.uint16)

    in_maps2 = []
    for c in range(NC_CORES):
        rows = slice(c * BROWS, (c + 1) * BROWS)
        knn_rows = memory_keys[top_idx[rows]].reshape(NCD, D)
        kt8 = np.ascontiguousarray(
            knn_rows.reshape(2, 512, 4, 128).transpose(3, 0, 2, 1)
        ).reshape(128, 4096).astype(E4)
        qTc = np.ascontiguousarray(
            q32[rows].T.reshape(4, 128, BROWS).transpose(1, 0, 2)
        ).reshape(128, 128).astype(BF).view(np.uint16)
        blob = np.zeros((128, BLOB_W), np.uint16)
        blob[:, :2048] = np.asarray(kt8).view(np.uint8).reshape(128, 4096).view(np.uint16)
        blob[:, P2_CONST:] = wpart
        blob[:, P2_CONST + 6:P2_CONST + 134] = qTc
        in_maps2.append({"blob": blob.view(BF)})
    res2 = run_bass_kernel_spmd(ph2, in_maps2, core_ids=list(range(NC_CORES)))

    out = np.zeros((B, C), np.float32)
    for c in range(NC_CORES):
        r = np.asarray(res2.results[c]["out"], np.float32)   # [64, 104]
        y1 = r[:BROWS, :C]
        y2 = r[BROWS:, :C]
        den = r[BROWS:, C]
        out[c * BROWS:(c + 1) * BROWS] = y1 + y2 / den[:, None]
    return (out + np.asarray(bc, np.float32)[None, :]).astype(np.float32)
